# revision 61
# baseline (speedup 1.0000x reference)
"""Trainium2 Bass kernel for nn_BertCNN (3x BERT-small encoder + CNN maxpool head).

Strategy: data-parallel over *sequences* across 8 NeuronCores. The 96
sequences (32 samples x 3 branches) are classified by ragged length into
"short" (fits one 128-token tile) and "long" (two tiles), then dealt to
cores so every core gets the same composition (e.g. 6 short + 6 long)
and runs an identical SPMD program. Each core runs the 4-layer BERT
encoder + conv/maxpool head on its 12 sequences and emits per-sequence
partial logits [4cls, 3branch-hypotheses, 12seq]; the host selects the
real branch row per sequence, sums the 3 branch contributions of every
sample and adds fc_b (pure gather/unshard arithmetic).

Dataflow per core (all big matmuls bf16 operands, fp32 PSUM accumulation):
  - token embeddings gathered on-device via indirect DMA from word_emb
  - residual stream token-major fp32->bf16 in SBUF; a feature-major bf16
    transposed copy (PE-free DMA transpose, one XBAR call per pair)
    feeds the QKV / FFN matmuls
  - sequences processed in pairs (short pair = 2x1 tile, long = 2x2) so
    projection / FFN1 matmuls run at the widest moving-operand width
  - attention in S^T = [key, query] layout: the ragged-length mask folds
    into the Exp activation as a per-partition bias; exp(S^T) is the
    lhsT of the context matmul; softmax denominators come from a ones
    column appended to V
  - LayerNorm rstd via Quake-style bit-trick rsqrt + 2 Newton steps on
    the DVE: keeps Sqrt off the ACT engine so the only ACT table loads
    are the per-iteration Exp<->Gelu switch
  - conv head runs feature-major ([filters, windows]); global maxpool is
    a per-partition free-axis reduce_max; ragged window validity is a
    -1e30 penalty added before the max; short sequences use 129-wide
    windows instead of 257

Engine-queue discipline (from trace analysis of the v0 kernel):
  - sync queue: ONLY DMA transposes (+ final output store)
  - scalar queue (ACT HWDGE): prologue constants + layer-0 weights
  - gpsimd queue (SWDGE): embedding gathers, layer 1..3 / conv / fc
    weight prefetch, conv masks
  - per-iteration emission order keeps the PE queue dependency-clean:
    ctx -> [next-pair QKV backfill] -> WO -> [next V] -> [next scores g0]
    -> LN1 -> FFN1 -> FFN2 (ft-outer, chases the Gelu chain) ->
    [next scores g1] -> LN2 -> pair transpose -> conv (last layer)
"""

import numpy as np
import ml_dtypes

V, D, H, DH, NL, FF = 30522, 512, 8, 64, 4, 2048
NF, NCLS, B, S = 256, 4, 32, 256
NCORES, SPC = 8, 4
NSEQ = 3 * SPC          # 12 sequences per core
NPAIR = NSEQ // 2       # 6 sequence pairs
ND = D // 128            # 4 feature tiles
NFT = FF // 128          # 16 FFN hidden tiles
NCH = 6                  # per-branch fc chunks of 128 (3 kernels x 2 ftiles)

BF = ml_dtypes.bfloat16
F8 = ml_dtypes.float8_e4m3
_CACHE = {}
FP8_FFN = True           # fp8e4 DoubleRow matmuls for FFN1/FFN2
FFN_XS, FFN_WS = 8.0, 16.0   # fp8 quantization scales (powers of 2)
FP8_QKV = False          # fp8e4 DoubleRow for the QKV/WO projections too
#   (tried: saves ~90us PE but congests DVE/ACT and costs 0.4e-2 rel err)
#   (scores / softmax / ctx stay bf16; only the projections quantize)
QKV_XS, CTX_S = 16.0, 32.0   # x / ctx activation scales for fp8
QUAKE_LN = True          # DVE-only rsqrt (bit trick + Newton steps)
QUAKE_ITERS = 1          # Newton steps (1 -> 1.8e-3 rstd rel err, plenty here)
BCAST_NORM = True        # one stride-0-broadcast DVE op per ctx 4-head group
BF16_R = True            # residual/LN scratch tiles in bf16 (2x DVE modes)


def _flags(inputs):
    z = lambda a: bool(np.all(np.asarray(a) == 0))
    o = lambda a: bool(np.all(np.asarray(a) == 1))
    return {
        "bqk": z(inputs["bq"]) and z(inputs["bk"]),
        "bv": z(inputs["bv"]),
        "bo": z(inputs["bo"]),
        "bi": z(inputs["bi"]),
        "bo2": z(inputs["bo2"]),
        "ln": all(o(inputs[k]) for k in ("emb_ln_s", "ln1_s", "ln2_s"))
        and all(z(inputs[k]) for k in ("emb_ln_b", "ln1_b", "ln2_b")),
        "cb": z(inputs["conv_b1"]) and z(inputs["conv_b2"]) and z(inputs["conv_b3"]),
    }


def _pair_nts(ns):
    """Pair tile-counts for a core with ns short seqs: one short pair
    first (fast PE start), longs in the middle, shorts at the tail (small
    final conv)."""
    nps = ns // 2
    pn = []
    if nps > 0:
        pn.append(1)
    pn += [2] * (NPAIR - nps)
    pn += [1] * (nps - 1 if nps > 0 else 0)
    return tuple(pn)


def _build_program(fl, pn):
    import contextlib
    import concourse.bass as bass
    import concourse.mybir as mybir
    import concourse.tile as tile
    from concourse import bacc
    from concourse.masks import make_identity

    F32, BF16, I32 = mybir.dt.float32, mybir.dt.bfloat16, mybir.dt.int32
    U32 = mybir.dt.uint32
    AL, AF = mybir.AluOpType, mybir.ActivationFunctionType

    seq_nt = []
    for p in range(NPAIR):
        seq_nt += [pn[p], pn[p]]
    tbase = np.concatenate([[0], np.cumsum(seq_nt)]).astype(int)
    NT_TOT = int(tbase[-1])

    nc = bacc.Bacc("TRN2", target_bir_lowering=False, debug=False,
                   num_devices=NCORES)

    di = lambda n, s, d: nc.dram_tensor(n, s, d, kind="ExternalInput").ap()
    F8D = mybir.dt.float8e4
    PDT = F8D if FP8_QKV else BF16
    word = di("word_emb", [V, D], F32)
    ids_d = di("ids", [128, NT_TOT], I32)
    mb_d = di("maskbias", [128, NT_TOT], F32)
    posty_d = di("posty", [2, 128, D], F32)
    cmask_d = di("convmask", [NSEQ, S + 1], BF16)
    cpen_d = di("convpen", [NSEQ, 3, S + 1], F32)
    wq_d = [di(f"wq{l}", [128, ND, D], PDT) for l in range(NL)]
    wk_d = [di(f"wk{l}", [128, ND, D], PDT) for l in range(NL)]
    wv_d = [di(f"wv{l}", [128, ND, D], PDT) for l in range(NL)]
    wo_d = [di(f"wo{l}", [128, ND, D], PDT) for l in range(NL)]
    WDT = F8D if FP8_FFN else BF16
    wi_d = [di(f"wi{l}", [128, ND, FF], WDT) for l in range(NL)]
    wo2_d = [di(f"wo2{l}", [128, NFT, D], WDT) for l in range(NL)]
    cw_d = [di(f"cw{k}", [k, 128, ND, NF], BF16) for k in (1, 2, 3)]
    fcw_d = di("fcw", [128, 3, NCH, NCLS], F32)
    if not fl["bqk"]:
        bq_d = [di(f"bq{l}", [ND, 128], F32) for l in range(NL)]
        bk_d = [di(f"bk{l}", [ND, 128], F32) for l in range(NL)]
    if not fl["bv"]:
        bv_d = [di(f"bv{l}", [D], F32) for l in range(NL)]
    if not fl["bo"]:
        bo_d = [di(f"bo{l}", [D], F32) for l in range(NL)]
    if not fl["bi"]:
        bi_d = [di(f"bi{l}", [NFT, 128], F32) for l in range(NL)]
    if not fl["bo2"]:
        bo2_d = [di(f"bo2{l}", [D], F32) for l in range(NL)]
    if not fl["ln"]:
        elns_d = di("lnes", [D], F32)
        elnb_d = di("lneb", [D], F32)
        ln1s_d = [di(f"ln1s{l}", [D], F32) for l in range(NL)]
        ln1b_d = [di(f"ln1b{l}", [D], F32) for l in range(NL)]
        ln2s_d = [di(f"ln2s{l}", [D], F32) for l in range(NL)]
        ln2b_d = [di(f"ln2b{l}", [D], F32) for l in range(NL)]
    if not fl["cb"]:
        cb_d = di("convb", [3, 2, 128], F32)

    out_d = nc.dram_tensor("out", [NCLS, 3, NSEQ], F32,
                           kind="ExternalOutput").ap()

    with tile.TileContext(nc) as tc, contextlib.ExitStack() as ctx:
        consts = ctx.enter_context(tc.tile_pool(name="consts", bufs=1))
        state = ctx.enter_context(tc.tile_pool(name="state", bufs=1))
        wts = ctx.enter_context(tc.tile_pool(name="wts", bufs=1))
        big = ctx.enter_context(tc.tile_pool(name="big", bufs=1))
        work = ctx.enter_context(tc.tile_pool(name="work", bufs=2))
        small = ctx.enter_context(tc.tile_pool(name="small", bufs=4))
        ps_mm = ctx.enter_context(tc.tile_pool(name="ps_mm", bufs=6, space="PSUM"))
        ps_ctx = ctx.enter_context(tc.tile_pool(name="ps_ctx", bufs=2, space="PSUM"))

        # ---- prologue constants: scalar (ACT) HWDGE queue, critical first ----
        ids_sb = consts.tile([128, NT_TOT], I32, tag="ids")
        nc.scalar.dma_start(out=ids_sb[:], in_=ids_d)
        posty = consts.tile([128, 2, D], F32, tag="posty")
        nc.scalar.dma_start(out=posty[:], in_=posty_d.rearrange("t p d -> p t d"))
        mb_sb = consts.tile([128, NT_TOT], F32, tag="mb")
        nc.scalar.dma_start(out=mb_sb[:], in_=mb_d)
        ident = consts.tile([128, 128], BF16, tag="ident")
        magic = consts.tile([128, 8], I32, tag="magic")
        nc.vector.memset(magic[:], 0x5F3759DF)
        if not QUAKE_LN:
            eps_t = consts.tile([128, 1], F32, tag="eps")
            nc.vector.memset(eps_t[:], 1e-12)

        bcast = lambda ap, n: ap[None, :].to_broadcast([128, n])
        if not fl["ln"]:
            elns = consts.tile([128, D], F32, tag="elns")
            nc.scalar.dma_start(out=elns[:], in_=bcast(elns_d, D))
            elnb = consts.tile([128, D], F32, tag="elnb")
            nc.scalar.dma_start(out=elnb[:], in_=bcast(elnb_d, D))

        # persistent per-pair state: token-major residual + feature-major copy
        RDT = BF16 if BF16_R else F32
        XDT = F8D if FP8_QKV else BF16
        x_p = [state.tile([128, 2, pn[q], D], BF16, tag=f"xp{q}",
                          name=f"xp{q}") for q in range(NPAIR)]
        if FP8_QKV:
            xT = [state.tile([128, ND, 2 * pn[q] * 128], F8D, tag=f"xT{q}",
                             name=f"xT{q}") for q in range(NPAIR)]
        else:
            xT = [state.tile([128, 2, pn[q], ND, 128], BF16, tag=f"xT{q}",
                             name=f"xT{q}") for q in range(NPAIR)]
        rep = state.tile([128, NCH, NSEQ], F32, tag="rep")

        def ln_stats_batch(rs):
            """Pipelined LN stats for a list of [128, D] sources; returns
            (rcp, nmb) where rcp[:, i] = rstd_i, nmb[:, i] = -mean_i*rstd_i."""
            n = len(rs)
            mvt = small.tile([128, n, 2], F32, tag="mvt")
            for i, r in enumerate(rs):
                st = small.tile([128, 6], F32, tag="st", name=f"st{i}")
                nc.vector.bn_stats(out=st[:], in_=r)
                nc.vector.bn_aggr(out=mvt[:, i, :], in_=st[:])
            rcp = small.tile([128, n], F32, tag="rcp2")
            if QUAKE_LN:
                # rstd = rsqrt(var + eps): Quake bit-trick + 2 Newton steps,
                # all on the DVE (no ACT Sqrt -> no act-table thrash)
                vv = small.tile([128, n], F32, tag="vv")
                nc.vector.tensor_scalar_add(out=vv[:], in0=mvt[:, :, 1],
                                            scalar1=1e-12)
                nc.vector.tensor_scalar(
                    out=rcp[:].bitcast(I32), in0=vv[:].bitcast(I32),
                    scalar1=1, scalar2=0, op0=AL.logical_shift_right,
                    op1=AL.bypass)
                nc.vector.tensor_tensor(
                    out=rcp[:].bitcast(I32), in0=magic[:, 0:n],
                    in1=rcp[:].bitcast(I32), op=AL.subtract)
                t = small.tile([128, n], F32, tag="qt")
                for _ in range(QUAKE_ITERS):
                    nc.vector.tensor_tensor(out=t[:], in0=rcp[:], in1=rcp[:],
                                            op=AL.mult)
                    nc.vector.tensor_tensor(out=t[:], in0=t[:], in1=vv[:],
                                            op=AL.mult)
                    nc.vector.tensor_scalar(out=t[:], in0=t[:], scalar1=-0.5,
                                            scalar2=1.5, op0=AL.mult,
                                            op1=AL.add)
                    nc.vector.tensor_tensor(out=rcp[:], in0=rcp[:], in1=t[:],
                                            op=AL.mult)
            else:
                sd = small.tile([128, n], F32, tag="sd")
                nc.scalar.activation(out=sd[:], in_=mvt[:, :, 1],
                                     func=AF.Sqrt, bias=eps_t[:], scale=1.0)
                nc.vector.reciprocal(out=rcp[:], in_=sd[:])
            nmb = small.tile([128, n], F32, tag="nmb")
            # nmb = (mean * -1) * rstd
            nc.vector.scalar_tensor_tensor(
                out=nmb[:], in0=mvt[:, :, 0], scalar=-1.0, in1=rcp[:],
                op0=AL.mult, op1=AL.mult)
            return rcp, nmb

        def ln_apply_batch(rs, dsts, rcp, nmb, s_tile, b_tile):
            for i in range(len(rs)):
                if False and s_tile is None and b_tile is None and i % 2 == 1:
                    # odd tiles apply on ACT (Copy: in every table, no load)
                    # so the two engines drain the batch in parallel
                    nc.scalar.activation(
                        out=dsts[i], in_=rs[i], func=AF.Identity,
                        bias=nmb[:, i:i + 1], scale=rcp[:, i:i + 1])
                    continue
                nc.vector.tensor_scalar(
                    out=dsts[i], in0=rs[i], scalar1=rcp[:, i:i + 1],
                    scalar2=nmb[:, i:i + 1], op0=AL.mult, op1=AL.add)
                if s_tile is not None:
                    nc.vector.tensor_tensor(out=dsts[i], in0=dsts[i],
                                            in1=s_tile[:], op=AL.mult)
                if b_tile is not None:
                    nc.vector.tensor_tensor(out=dsts[i], in0=dsts[i],
                                            in1=b_tile[:], op=AL.add)

        def feat_major(pr, dst, scale):
            """PE-transpose x_p[pr] into a feature-major copy dst with a
            fused scale+cast drain. Transposes go tile-outer so they chase
            the LN applies tile-by-tile instead of waiting for the batch."""
            nt = pn[pr]
            sw = 2 * nt * 128
            for dt in range(ND):
                tps = ps_mm.tile([128, sw], BF16, tag="mm",
                                 name=f"fm{pr}_{dt}")
                for i in range(2 * nt):
                    si, tt = i // nt, i % nt
                    nc.tensor.transpose(
                        tps[:, i * 128:(i + 1) * 128],
                        x_p[pr][:, si, tt, dt * 128:(dt + 1) * 128],
                        ident[:])
                if scale == 1.0:
                    nc.vector.tensor_copy(out=dst[:, dt, :], in_=tps[:])
                else:
                    nc.vector.tensor_scalar_mul(out=dst[:, dt, :],
                                                in0=tps[:], scalar1=scale)

        def to_feat(pr, eng=None):
            """Refresh the feature-major x copy after an LN2 update."""
            if FP8_QKV:
                feat_major(pr, xT[pr], QKV_XS)
            else:
                (eng or nc.sync).dma_start_transpose(xT[pr][:, :, :, :, :],
                                                     x_p[pr][:, :, :, :])

        def embed_pair(p, pt_eng=None):
            nt = pn[p]
            t0 = int(tbase[2 * p])
            gb = work.tile([128, 2 * nt, D], F32, tag="r", name=f"g{p}",
                           bufs=2)
            tiles = []
            for si in range(2):
                for tt in range(nt):
                    j = si * nt + tt
                    nc.gpsimd.indirect_dma_start(
                        out=gb[:, j, :], out_offset=None, in_=word[:],
                        in_offset=bass.IndirectOffsetOnAxis(
                            ap=ids_sb[:, t0 + j:t0 + j + 1], axis=0))
                    nc.vector.tensor_tensor(out=gb[:, j, :], in0=gb[:, j, :],
                                            in1=posty[:, tt, :], op=AL.add)
                    tiles.append((si, tt))
            rcp, nmb = ln_stats_batch([gb[:, si * nt + tt, :]
                                       for si, tt in tiles])
            ln_apply_batch([gb[:, si * nt + tt, :] for si, tt in tiles],
                           [x_p[p][:, si, tt, :] for si, tt in tiles],
                           rcp, nmb,
                           None if fl["ln"] else elns,
                           None if fl["ln"] else elnb)
            to_feat(p, pt_eng)

        def load_layer_weights_A(l, q):
            """QKV weights (+ small per-layer consts) for layer l."""
            w = {}
            for nm, dd in (("wq", wq_d), ("wk", wk_d), ("wv", wv_d)):
                w[nm] = wts.tile([128, ND, D], PDT, tag=nm, name=f"{nm}_{l}")
                for dt in range(0, ND, 2):
                    q.dma_start(out=w[nm][:, dt:dt + 2, :],
                                in_=dd[l][:, dt:dt + 2, :])
            if not fl["bqk"]:
                w["bq"] = consts.tile([128, ND], F32, tag="bq", name=f"bq_{l}")
                q.dma_start(out=w["bq"][:], in_=bq_d[l].rearrange("t p -> p t"))
                w["bk"] = consts.tile([128, ND], F32, tag="bk", name=f"bk_{l}")
                q.dma_start(out=w["bk"][:], in_=bk_d[l].rearrange("t p -> p t"))
            if not fl["bv"]:
                w["bv"] = consts.tile([128, D], F32, tag="bv", name=f"bv_{l}")
                q.dma_start(out=w["bv"][:], in_=bcast(bv_d[l], D))
            if not fl["ln"]:
                for nm, dd in (("ln1s", ln1s_d), ("ln1b", ln1b_d),
                               ("ln2s", ln2s_d), ("ln2b", ln2b_d)):
                    w[nm] = consts.tile([128, D], F32, tag=nm, name=f"{nm}_{l}")
                    q.dma_start(out=w[nm][:], in_=bcast(dd[l], D))
            return w

        def load_layer_weights_B(l, w, q):
            """WO / FFN weights for layer l (emit after last layer-(l-1) use)."""
            w["wo"] = wts.tile([128, ND, D], PDT, tag="wo", name=f"wo_{l}")
            for dt in range(0, ND, 2):
                q.dma_start(out=w["wo"][:, dt:dt + 2, :],
                            in_=wo_d[l][:, dt:dt + 2, :])
            w["wi"] = wts.tile([128, ND, FF], WDT, tag="wi", name=f"wi_{l}")
            for dt in range(0, ND, 2):
                q.dma_start(out=w["wi"][:, dt:dt + 2, :],
                            in_=wi_d[l][:, dt:dt + 2, :])
            w["wo2"] = wts.tile([128, NFT, D], WDT, tag="wo2", name=f"wo2_{l}")
            for ft in range(0, NFT, 8):
                q.dma_start(out=w["wo2"][:, ft:ft + 8, :],
                            in_=wo2_d[l][:, ft:ft + 8, :])
            if not fl["bo"]:
                w["bo"] = consts.tile([128, D], F32, tag="bo", name=f"bo_{l}")
                q.dma_start(out=w["bo"][:], in_=bcast(bo_d[l], D))
            if not fl["bi"]:
                w["bi"] = consts.tile([128, NFT], F32, tag="bi", name=f"bi_{l}")
                q.dma_start(out=w["bi"][:], in_=bi_d[l].rearrange("t p -> p t"))
            if not fl["bo2"]:
                w["bo2"] = consts.tile([128, D], F32, tag="bo2", name=f"bo2_{l}")
                q.dma_start(out=w["bo2"][:], in_=bcast(bo2_d[l], D))
            return w

        # fp8 dequant scales for the projection drains (wq carries no /8 on
        # the host in fp8 mode; the score scaling folds in here instead)
        P_SC = 1.0 / (QKV_XS * FFN_WS) if FP8_QKV else 1.0
        Q_SC = P_SC * 0.125 if FP8_QKV else 1.0
        O_SC = 1.0 / (CTX_S * FFN_WS) if FP8_QKV else 1.0

        def proj_mm(ps, xts, wt, ocols):
            """One QKV-projection matmul chain (DR fp8 or bf16)."""
            if FP8_QKV:
                for dp in range(2):
                    nc.tensor.matmul(
                        ps[:], wt[:, 2 * dp:2 * dp + 2, ocols],
                        xts[:, 2 * dp:2 * dp + 2, :],
                        start=dp == 0, stop=dp == 1,
                        perf_mode=mybir.MatmulPerfMode.DoubleRow)
            else:
                for dt in range(ND):
                    nc.tensor.matmul(
                        ps[:], wt[:, dt, ocols], xts[:, :, :, dt, :],
                        start=dt == 0, stop=dt == ND - 1)

        def qkv_pair(pr, w):
            """qT/kT feature-major bf16 for both seqs."""
            nt = pn[pr]
            sw = 2 * nt * 128
            xts = xT[pr]
            qT = work.tile([128, ND, sw], BF16, tag="qT", name=f"qT{pr}")
            kT = work.tile([128, ND, sw], BF16, tag="kT", name=f"kT{pr}")
            for dst_t, wt, which in ((qT, w["wq"], "q"), (kT, w["wk"], "k")):
                sc = Q_SC if which == "q" else P_SC
                for ot in range(ND):
                    ps = ps_mm.tile([128, sw], F32, tag="mm")
                    proj_mm(ps, xts, wt, slice(ot * 128, (ot + 1) * 128))
                    if fl["bqk"]:
                        # split PSUM drain across ACT and DVE so neither
                        # engine's queue gates PSUM recycling
                        if which == "q":
                            nc.scalar.mul(out=dst_t[:, ot, :], in_=ps[:],
                                          mul=sc)
                        elif sc == 1.0:
                            nc.vector.tensor_copy(out=dst_t[:, ot, :],
                                                  in_=ps[:])
                        else:
                            nc.vector.tensor_scalar_mul(
                                out=dst_t[:, ot, :], in0=ps[:], scalar1=sc)
                    else:
                        bt = w["bq"] if which == "q" else w["bk"]
                        nc.vector.tensor_scalar(
                            out=dst_t[:, ot, :], in0=ps[:], scalar1=sc,
                            scalar2=bt[:, ot:ot + 1], op0=AL.mult, op1=AL.add)
            return qT, kT

        def v_pair(pr, w):
            """V token-major bf16 with a ones column per head, both seqs."""
            nt = pn[pr]
            xts = xT[pr]
            vAs = []
            for si in range(2):
                vA = work.tile([128, nt, H, DH + 1], BF16, tag="vA",
                               name=f"vA{pr}_{si}")
                nc.vector.memset(vA[:, :, :, DH:DH + 1], 1.0)
                for tt in range(nt):
                    tok = (si * nt + tt) * 128
                    ps = ps_mm.tile([128, D], F32, tag="mm")
                    if FP8_QKV:
                        for dp in range(2):
                            nc.tensor.matmul(
                                ps[:], xts[:, 2 * dp:2 * dp + 2, tok:tok + 128],
                                w["wv"][:, 2 * dp:2 * dp + 2, :],
                                start=dp == 0, stop=dp == 1,
                                perf_mode=mybir.MatmulPerfMode.DoubleRow)
                    else:
                        for dt in range(ND):
                            nc.tensor.matmul(
                                ps[:], xts[:, si, tt, dt, :],
                                w["wv"][:, dt, :], start=dt == 0,
                                stop=dt == ND - 1)
                    if fl["bv"]:
                        if P_SC == 1.0:
                            nc.vector.tensor_copy(
                                out=vA[:, tt, :, 0:DH],
                                in_=ps.rearrange("p (h d) -> p h d", h=H))
                        else:
                            nc.vector.tensor_scalar_mul(
                                out=vA[:, tt, :, 0:DH],
                                in0=ps.rearrange("p (h d) -> p h d", h=H),
                                scalar1=P_SC)
                    else:
                        nc.vector.scalar_tensor_tensor(
                            out=vA[:, tt, :, 0:DH],
                            in0=ps.rearrange("p (h d) -> p h d", h=H),
                            scalar=P_SC,
                            in1=w["bv"].rearrange("p (h d) -> p h d", h=H),
                            op0=AL.mult, op1=AL.add)
                vAs.append(vA)
            return vAs

        att_state = {}

        def att_scores(pr, g_, qk):
            """S^T + exp for head-group g_ of both seqs of pair pr."""
            nt = pn[pr]
            sq = nt * 128       # queries per seq
            qT, kT = qk
            for si in range(2):
                seq = 2 * pr + si
                so = si * sq
                eT = work.tile([128, nt, 4, sq], BF16, tag="eT",
                               name=f"eT{pr}_{g_}_{si}", bufs=4)
                att_state[(pr, g_, si)] = eT
                for kt in range(nt):
                    ti = int(tbase[seq]) + kt
                    for hi in range(4):
                        h = g_ * 4 + hi
                        ot, hh = h // 2, (h % 2) * DH
                        ps = ps_mm.tile([128, sq], F32, tag="mm")
                        nc.tensor.matmul(
                            ps[:],
                            kT[hh:hh + DH, ot, so + kt * 128:so + (kt + 1) * 128],
                            qT[hh:hh + DH, ot, so:so + sq],
                            start=True, stop=True)
                        nc.scalar.activation(
                            out=eT[:, kt, hi, :], in_=ps[:],
                            func=AF.Exp, bias=mb_sb[:, ti:ti + 1], scale=1.0)

        def att_ctx(pr, g_, vAs, ctxb):
            """ctx (+denominator) matmuls and DVE normalization for group g_."""
            nt = pn[pr]
            for si in range(2):
                eT, vA = att_state.pop((pr, g_, si)), vAs[si]
                for qt in range(nt):
                    cps = ps_ctx.tile([128, 4 * (DH + 1)], F32, tag="ctx",
                                      name=f"ctx{si}_{qt}_{g_}")
                    for hi in range(4):
                        h = g_ * 4 + hi
                        sl = slice(hi * (DH + 1), (hi + 1) * (DH + 1))
                        for kt in range(nt):
                            nc.tensor.matmul(
                                cps[:, sl],
                                eT[:, kt, hi, qt * 128:(qt + 1) * 128],
                                vA[:, kt, h, :], start=kt == 0,
                                stop=kt == nt - 1)
                    cph = cps.rearrange("p (h c) -> p h c", c=DH + 1)
                    if BCAST_NORM:
                        rcp = small.tile([128, 4, 1], F32, tag="rcp")
                        nc.vector.reciprocal(out=rcp[:],
                                             in_=cph[:, :, DH:DH + 1])
                        nc.vector.scalar_tensor_tensor(
                            out=ctxb[:, si, qt,
                                     g_ * 4 * DH:(g_ + 1) * 4 * DH].rearrange(
                                         "p (h d) -> p h d", d=DH),
                            in0=cph[:, :, 0:DH], scalar=1.0,
                            in1=rcp[:, :, 0:1].broadcast_to([128, 4, DH]),
                            op0=AL.mult, op1=AL.mult)
                    else:
                        rcp = small.tile([128, 4], F32, tag="rcp")
                        nc.vector.reciprocal(out=rcp[:], in_=cph[:, :, DH])
                        for hi in range(4):
                            h = g_ * 4 + hi
                            base = hi * (DH + 1)
                            nc.vector.tensor_scalar_mul(
                                out=ctxb[:, si, qt, h * DH:(h + 1) * DH],
                                in0=cps[:, base:base + DH],
                                scalar1=rcp[:, hi:hi + 1])

        def residual_ln(rs, dsts, s_tile, b_tile):
            rcp, nmb = ln_stats_batch(rs)
            ln_apply_batch(rs, dsts, rcp, nmb, s_tile, b_tile)

        def ctx_half(pr, g_, vAs, ctxb, ct):
            """ctx matmuls + norm for head-group g_, then PE-transpose that
            group's two feature tiles into ct with ACT copy drains (copy is
            in every act table -> no table load, ~1us latency to WO)."""
            nt = pn[pr]
            sw = 2 * nt * 128
            att_ctx(pr, g_, vAs, ctxb)
            for dt in (2 * g_, 2 * g_ + 1):
                tpx = ps_mm.tile([128, sw], BF16, tag="mm",
                                 name=f"ctp{pr}_{dt}")
                for i in range(2 * nt):
                    si, tt = i // nt, i % nt
                    nc.tensor.transpose(
                        tpx[:, i * 128:(i + 1) * 128],
                        ctxb[:, si, tt, dt * 128:(dt + 1) * 128],
                        ident[:])
                if FP8_QKV:
                    nc.scalar.mul(out=ct[:, dt, :], in_=tpx[:], mul=CTX_S)
                else:
                    nc.scalar.copy(out=ct[:, dt, :], in_=tpx[:])

        def ctx_start(pr, vAs):
            """Allocate this pair's ctx tiles and run head-group 0."""
            nt = pn[pr]
            ctxb = work.tile([128, 2, nt, D], BF16, tag="ctxb",
                             name=f"cb{pr}")
            ct = work.tile([128, ND, 2 * nt * 128], XDT, tag="cT",
                           name=f"cT{pr}")
            ctx_half(pr, 0, vAs, ctxb, ct)
            return ctxb, ct

        cw = {}

        def load_conv_weights(q):
            ti = 0
            for ki, k in enumerate((1, 2, 3)):
                for j in range(k):
                    t = wts.tile([128, ND, NF], BF16, tag=f"cw{ti}",
                                 name=f"cwt{k}_{j}")
                    q.dma_start(out=t[:], in_=cw_d[ki][j])
                    cw[(k, j)] = t
                    ti += 1
            fcw = consts.tile([128, 3, NCH, NCLS], F32, tag="fcw")
            q.dma_start(out=fcw[:], in_=fcw_d)
            cb = None
            if not fl["cb"]:
                cb = consts.tile([128, 3, 2], F32, tag="cb")
                q.dma_start(out=cb[:], in_=cb_d.rearrange("k t p -> p k t"))
            return fcw, cb

        def conv_prefetch(pr):
            """Issue the conv mask / window-penalty DMAs for pair pr early."""
            nt = pn[pr]
            swc = nt * 128 + 1
            cms, pens = [], []
            for si in range(2):
                seq = 2 * pr + si
                cm = work.tile([128, swc], BF16, tag="cm", name=f"cm{seq}")
                nc.gpsimd.dma_start(
                    out=cm[:],
                    in_=cmask_d[seq, 0:swc][None, :].to_broadcast([128, swc]))
                cms.append(cm)
                ps_ = []
                for ki in range(3):
                    pen = work.tile([128, swc], F32, tag="pen",
                                    name=f"pen{seq}_{ki}", bufs=6)
                    nc.gpsimd.dma_start(
                        out=pen[:],
                        in_=cpen_d[seq, ki, 0:swc][None, :].to_broadcast(
                            [128, swc]))
                    ps_.append(pen)
                pens.append(ps_)
            return cms, pens

        def conv_pair(pr, cms, pens):
            """Conv head for both seqs of pair pr straight off x_p: PE
            transposes feature tiles into PSUM, the drain fuses the
            token-mask multiply (no DMA transpose on the conv path)."""
            nt = pn[pr]
            swc = nt * 128 + 1
            for si in range(2):
                seq = 2 * pr + si
                xcv = work.tile([128, ND, swc], BF16, tag="xcv",
                                name=f"xcv{seq}")
                nc.vector.memset(xcv[:, :, nt * 128:nt * 128 + 1], 0.0)
                for dt in range(ND):
                    tpc = ps_mm.tile([128, nt * 128], BF16, tag="mm",
                                     name=f"cvtp{seq}_{dt}")
                    for tt in range(nt):
                        nc.tensor.transpose(
                            tpc[:, tt * 128:(tt + 1) * 128],
                            x_p[pr][:, si, tt, dt * 128:(dt + 1) * 128],
                            ident[:])
                    nc.vector.tensor_tensor(out=xcv[:, dt, 0:nt * 128],
                                            in0=tpc[:],
                                            in1=cms[si][:, 0:nt * 128],
                                            op=AL.mult)
                for ki, k in enumerate((1, 2, 3)):
                    nw = swc - k + 1
                    for ft in range(2):
                        ps = ps_mm.tile([128, swc], F32, tag="mm")
                        idx = 0
                        for dt in range(ND):
                            for j in range(k):
                                nc.tensor.matmul(
                                    ps[:, 0:nw],
                                    cw[(k, j)][:, dt, ft * 128:(ft + 1) * 128],
                                    xcv[:, dt, j:j + nw],
                                    start=idx == 0, stop=idx == ND * k - 1)
                                idx += 1
                        cvt = work.tile([128, swc], F32, tag="cvt",
                                        name=f"cv{seq}_{k}_{ft}")
                        nc.vector.tensor_tensor(out=cvt[:, 0:nw],
                                                in0=ps[:, 0:nw],
                                                in1=pens[si][ki][:, 0:nw],
                                                op=AL.add)
                        nc.vector.tensor_reduce(
                            out=rep[:, ki * 2 + ft, seq:seq + 1],
                            in_=cvt[:, 0:nw],
                            axis=mybir.AxisListType.X, op=AL.max)

        # ---- main schedule ----
        # prologue: embed gathers lead the DGE, layer-0 weights trail them
        # on the gpsimd queue in need order (wq/wk -> wv -> wo/wi/wo2)
        make_identity(nc, ident[:])
        # p-state warmup: keep the PE streaming while the embed/weight
        # chain runs so the first real matmuls start at full clock
        warm = consts.tile([128, 512], BF16, tag="warm")
        nc.vector.memset(warm[:], 0.5)
        wps = ps_mm.tile([128, 512], F32, tag="mm", name="warm")
        for _ in range(30):
            nc.tensor.matmul(wps[:], ident[:], warm[:], start=True, stop=True)
        embed_pair(0, pt_eng=nc.scalar)
        w_cur = {}
        for nm, dd in (("wq", wq_d), ("wk", wk_d)):
            w_cur[nm] = wts.tile([128, ND, D], PDT, tag=nm, name=f"{nm}_0")
            for dt in range(0, ND, 2):
                nc.gpsimd.dma_start(out=w_cur[nm][:, dt:dt + 2, :],
                                    in_=dd[0][:, dt:dt + 2, :])
        embed_pair(1, pt_eng=nc.scalar)
        w_cur["wv"] = wts.tile([128, ND, D], PDT, tag="wv", name="wv_0")
        for dt in range(0, ND, 2):
            nc.gpsimd.dma_start(out=w_cur["wv"][:, dt:dt + 2, :],
                                in_=wv_d[0][:, dt:dt + 2, :])
        if not fl["bqk"]:
            w_cur["bq"] = consts.tile([128, ND], F32, tag="bq", name="bq_0")
            nc.gpsimd.dma_start(out=w_cur["bq"][:],
                                in_=bq_d[0].rearrange("t p -> p t"))
            w_cur["bk"] = consts.tile([128, ND], F32, tag="bk", name="bk_0")
            nc.gpsimd.dma_start(out=w_cur["bk"][:],
                                in_=bk_d[0].rearrange("t p -> p t"))
        if not fl["bv"]:
            w_cur["bv"] = consts.tile([128, D], F32, tag="bv", name="bv_0")
            nc.gpsimd.dma_start(out=w_cur["bv"][:], in_=bcast(bv_d[0], D))
        if not fl["ln"]:
            for nm, dd in (("ln1s", ln1s_d), ("ln1b", ln1b_d),
                           ("ln2s", ln2s_d), ("ln2b", ln2b_d)):
                w_cur[nm] = consts.tile([128, D], F32, tag=nm, name=f"{nm}_0")
                nc.gpsimd.dma_start(out=w_cur[nm][:], in_=bcast(dd[0], D))
        w_cur = load_layer_weights_B(0, w_cur, nc.gpsimd)
        qk_cur = qkv_pair(0, w_cur)
        v_cur = v_pair(0, w_cur)
        att_scores(0, 0, qk_cur)
        att_scores(0, 1, qk_cur)
        ctx_carry = ctx_start(0, v_cur)
        fcw = cb = None
        for l in range(NL):
            for pr in range(NPAIR):
                nt = pn[pr]
                w = w_cur
                if pr + 1 < NPAIR:
                    nxt_l, nxt_pr = l, pr + 1
                elif l + 1 < NL:
                    nxt_l, nxt_pr = l + 1, 0
                else:
                    nxt_l = nxt_pr = None
                cross = nxt_pr is not None and nxt_l != l

                if cross:
                    # QKV weights of the next layer: all layer-l readers of
                    # wq/wk/wv were emitted by the previous iteration
                    w_nxt = load_layer_weights_A(nxt_l, nc.gpsimd)
                elif nxt_pr is not None:
                    w_nxt = w
                if l == NL - 1:
                    cms, pens = conv_prefetch(pr)

                sw = 2 * nt * 128
                # head-group 0 of this pair's ctx ran at the tail of the
                # previous iteration (ctx_carry); finish group 1 here
                ctxb, ct = ctx_carry
                ctx_half(pr, 1, v_cur, ctxb, ct)
                # PE backfill: the next pair's QKV projections
                if nxt_pr is not None:
                    qk_nxt = qkv_pair(nxt_pr, w_nxt)
                # attention out projection + residual
                rs = []
                for i in range(2 * nt):
                    si, tt = i // nt, i % nt
                    ps = ps_mm.tile([128, D], F32, tag="mm")
                    if FP8_QKV:
                        for dp in range(2):
                            nc.tensor.matmul(
                                ps[:], ct[:, 2 * dp:2 * dp + 2,
                                           i * 128:(i + 1) * 128],
                                w["wo"][:, 2 * dp:2 * dp + 2, :],
                                start=dp == 0, stop=dp == 1,
                                perf_mode=mybir.MatmulPerfMode.DoubleRow)
                    else:
                        for dt in range(ND):
                            nc.tensor.matmul(
                                ps[:], ct[:, dt, i * 128:(i + 1) * 128],
                                w["wo"][:, dt, :], start=dt == 0,
                                stop=dt == ND - 1)
                    r = work.tile([128, D], RDT, tag="rln", name=f"r{i}", bufs=4)
                    if O_SC == 1.0:
                        nc.vector.tensor_tensor(out=r[:], in0=ps[:],
                                                in1=x_p[pr][:, si, tt, :],
                                                op=AL.add)
                    else:
                        nc.vector.scalar_tensor_tensor(
                            out=r[:], in0=ps[:], scalar=O_SC,
                            in1=x_p[pr][:, si, tt, :], op0=AL.mult, op1=AL.add)
                    if not fl["bo"]:
                        nc.vector.tensor_tensor(out=r[:], in0=r[:],
                                                in1=w["bo"][:], op=AL.add)
                    rs.append(r[:])
                # more PE backfill: next pair's V and both score groups run
                # while the LN1 chain (pure DVE now) drains
                if nxt_pr is not None:
                    v_nxt = v_pair(nxt_pr, w_nxt)
                    att_scores(nxt_pr, 0, qk_nxt)
                    att_scores(nxt_pr, 1, qk_nxt)
                residual_ln(rs, [x_p[pr][:, i // nt, i % nt, :]
                                 for i in range(2 * nt)],
                            None if fl["ln"] else w["ln1s"],
                            None if fl["ln"] else w["ln1b"])
                # PE-transpose the LN1 output straight into PSUM (bf16),
                # then one drain per dt does the fp8 cast + scale
                y1 = work.tile([128, ND, sw], WDT, tag="y1f8",
                               name=f"y1f8{pr}")
                feat_major(pr, y1, FFN_XS if FP8_FFN else 1.0)
                # FFN1: hidden feature-major, gelu fused with bias
                hT = big.tile([128, NFT, sw], F8D if FP8_FFN else BF16,
                              tag="hT")
                for ft in range(NFT):
                    ps = ps_mm.tile([128, sw], F32, tag="mm")
                    if FP8_FFN:
                        for dp in range(2):
                            nc.tensor.matmul(
                                ps[:],
                                w["wi"][:, 2 * dp:2 * dp + 2,
                                        ft * 128:(ft + 1) * 128],
                                y1[:, 2 * dp:2 * dp + 2, :],
                                start=dp == 0, stop=dp == 1,
                                perf_mode=mybir.MatmulPerfMode.DoubleRow)
                    else:
                        for dt in range(ND):
                            nc.tensor.matmul(
                                ps[:], w["wi"][:, dt, ft * 128:(ft + 1) * 128],
                                y1[:, dt, :], start=dt == 0,
                                stop=dt == ND - 1)
                    nc.scalar.activation(
                        out=hT[:, ft, :], in_=ps[:], func=AF.Gelu,
                        bias=0.0 if fl["bi"] else w["bi"][:, ft:ft + 1],
                        scale=1.0 / (FFN_XS * FFN_WS) if FP8_FFN else 1.0)
                # embeds of the remaining pairs ride the FFN window (their
                # DVE chain slots between the y1f8 and FFN2 drains)
                if l == 0 and pr < 4:
                    embed_pair(pr + 2)
                # FFN2 + residual: ft-outer with per-token-tile PSUM so the
                # first matmuls chase the Gelu chain instead of waiting on it
                pss = [ps_mm.tile([128, D], F32, tag="mm",
                                  name=f"f2_{l}_{pr}_{i}")
                       for i in range(2 * nt)]
                if FP8_FFN:
                    for fp_ in range(NFT // 2):
                        for i in range(2 * nt):
                            si, tt = i // nt, i % nt
                            so = si * nt * 128
                            nc.tensor.matmul(
                                pss[i][:],
                                hT[:, 2 * fp_:2 * fp_ + 2,
                                   so + tt * 128:so + (tt + 1) * 128],
                                w["wo2"][:, 2 * fp_:2 * fp_ + 2, :],
                                start=fp_ == 0, stop=fp_ == NFT // 2 - 1,
                                perf_mode=mybir.MatmulPerfMode.DoubleRow)
                else:
                    for ft in range(NFT):
                        for i in range(2 * nt):
                            si, tt = i // nt, i % nt
                            so = si * nt * 128
                            nc.tensor.matmul(
                                pss[i][:],
                                hT[:, ft, so + tt * 128:so + (tt + 1) * 128],
                                w["wo2"][:, ft, :], start=ft == 0,
                                stop=ft == NFT - 1)
                rs = []
                for i in range(2 * nt):
                    si, tt = i // nt, i % nt
                    r = work.tile([128, D], RDT, tag="rln", name=f"r2{i}", bufs=4)
                    if FP8_FFN:
                        nc.vector.scalar_tensor_tensor(
                            out=r[:], in0=pss[i][:], scalar=1.0 / FFN_WS,
                            in1=x_p[pr][:, si, tt, :], op0=AL.mult, op1=AL.add)
                    else:
                        nc.vector.tensor_tensor(out=r[:], in0=pss[i][:],
                                                in1=x_p[pr][:, si, tt, :],
                                                op=AL.add)
                    if not fl["bo2"]:
                        nc.vector.tensor_tensor(out=r[:], in0=r[:],
                                                in1=w["bo2"][:], op=AL.add)
                    rs.append(r[:])
                if cross:
                    # WO/FFN weights of the next layer: all layer-l readers
                    # of wo/wi/wo2 are emitted above
                    w_nxt = load_layer_weights_B(nxt_l, w_nxt, nc.gpsimd)
                if l == 1 and pr == 0:
                    fcw, cb = load_conv_weights(nc.gpsimd)
                residual_ln(rs, [x_p[pr][:, i // nt, i % nt, :]
                                 for i in range(2 * nt)],
                            None if fl["ln"] else w["ln2s"],
                            None if fl["ln"] else w["ln2b"])
                if l < NL - 1:
                    to_feat(pr)
                # PE backfill while the LN2 chain runs: the NEXT pair's
                # first ctx half (its exps cleared the ACT queue before
                # this pair's gelus)
                if nxt_pr is not None:
                    ctx_carry = ctx_start(nxt_pr, v_nxt)
                    qk_cur, v_cur, w_cur = qk_nxt, v_nxt, w_nxt
                if l == NL - 1:
                    conv_pair(pr, cms, pens)

        if not fl["cb"]:
            for ki in range(3):
                for ft in range(2):
                    co = ki * 2 + ft
                    nc.vector.tensor_scalar_add(
                        out=rep[:, co, :], in0=rep[:, co, :],
                        scalar1=cb[:, ki, ft:ft + 1])
        nc.scalar.activation(out=rep[:], in_=rep[:], func=AF.Relu)

        # partial logits per branch hypothesis: fps[:, b, :] = fcw_b^T @ rep
        fps = ps_mm.tile([128, 3, NSEQ], F32, tag="mm", name="fps")
        for b_ in range(3):
            for co in range(NCH):
                nc.tensor.matmul(fps[0:NCLS, b_, :], fcw[:, b_, co, :],
                                 rep[:, co, :],
                                 start=co == 0, stop=co == NCH - 1)
        ob = small.tile([NCLS, 3, NSEQ], F32, tag="ob")
        nc.scalar.copy(out=ob[:], in_=fps[0:NCLS, :, :])
        nc.sync.dma_start(out=out_d[:], in_=ob[:])

    nc.compile()
    return nc


def _classify(inputs):
    """Compute per-core composition and the seq->(core, slot) assignment.

    Returns (ns, assign) where assign[core] is a list of NSEQ global
    sequence ids (branch*32 + sample) in slot order."""
    lens = []
    for p in ("q", "a", "b"):
        lens.append(np.asarray(inputs[p + "_attention_mask"]).sum(1))
    lens = np.concatenate(lens)          # [96], id = branch*32+sample
    short_ids = np.where(lens <= 128)[0]
    ns = min(len(short_ids) // NCORES, NSEQ)
    ns -= ns % 2
    n_short = ns * NCORES
    order = np.argsort(lens, kind="stable")
    shorts = [i for i in order if lens[i] <= 128][:n_short]
    short_set = set(shorts)
    longs = [i for i in order[::-1] if i not in short_set]
    pnt = _pair_nts(ns)
    assign = []
    for c in range(NCORES):
        my_s = shorts[c * ns:(c + 1) * ns]
        my_l = longs[c * (NSEQ - ns):(c + 1) * (NSEQ - ns)]
        si, li = 0, 0
        slots = []
        for p in range(NPAIR):
            for _ in range(2):
                if pnt[p] == 1:
                    slots.append(my_s[si]); si += 1
                else:
                    slots.append(my_l[li]); li += 1
        assign.append(slots)
    return ns, assign


def _core_inputs(inputs, fl, ns, assign):
    f32 = lambda a: np.ascontiguousarray(np.asarray(a, dtype=np.float32))
    tile_w = lambda w: np.ascontiguousarray(
        f32(w).reshape(w.shape[0] // 128, 128, w.shape[1])
        .transpose(1, 0, 2).astype(BF))
    tile_w8 = lambda w: np.ascontiguousarray(
        (f32(w) * FFN_WS).reshape(w.shape[0] // 128, 128, w.shape[1])
        .transpose(1, 0, 2).astype(F8))

    pnt = _pair_nts(ns)
    seq_nt = []
    for p in range(NPAIR):
        seq_nt += [pnt[p], pnt[p]]

    shared = {}
    shared["posty"] = np.ascontiguousarray(
        (f32(inputs["pos_emb"][:S]) + f32(inputs["type_emb"][0])).reshape(
            2, 128, D))
    for l in range(NL):
        if FP8_QKV:
            # no host /8 on Wq in fp8 (subnormal risk); folded in the drain
            shared[f"wq{l}"] = tile_w8(inputs["Wq"][l])
            shared[f"wk{l}"] = tile_w8(inputs["Wk"][l])
            shared[f"wv{l}"] = tile_w8(inputs["Wv"][l])
            shared[f"wo{l}"] = tile_w8(inputs["Wo"][l])
        else:
            shared[f"wq{l}"] = tile_w(f32(inputs["Wq"][l]) / 8.0)
            shared[f"wk{l}"] = tile_w(inputs["Wk"][l])
            shared[f"wv{l}"] = tile_w(inputs["Wv"][l])
            shared[f"wo{l}"] = tile_w(inputs["Wo"][l])
        if FP8_FFN:
            shared[f"wi{l}"] = tile_w8(inputs["Wi"][l])
            shared[f"wo2{l}"] = tile_w8(inputs["Wo2"][l])
        else:
            shared[f"wi{l}"] = tile_w(inputs["Wi"][l])
            shared[f"wo2{l}"] = tile_w(inputs["Wo2"][l])
        if not fl["bqk"]:
            shared[f"bq{l}"] = f32(inputs["bq"][l]).reshape(ND, 128) / 8.0
            shared[f"bk{l}"] = f32(inputs["bk"][l]).reshape(ND, 128)
        if not fl["bv"]:
            shared[f"bv{l}"] = f32(inputs["bv"][l])
        if not fl["bo"]:
            shared[f"bo{l}"] = f32(inputs["bo"][l])
        if not fl["bi"]:
            shared[f"bi{l}"] = f32(inputs["bi"][l]).reshape(NFT, 128)
        if not fl["bo2"]:
            shared[f"bo2{l}"] = f32(inputs["bo2"][l])
        if not fl["ln"]:
            shared[f"ln1s{l}"] = f32(inputs["ln1_s"][l])
            shared[f"ln1b{l}"] = f32(inputs["ln1_b"][l])
            shared[f"ln2s{l}"] = f32(inputs["ln2_s"][l])
            shared[f"ln2b{l}"] = f32(inputs["ln2_b"][l])
    if not fl["ln"]:
        shared["lnes"] = f32(inputs["emb_ln_s"])
        shared["lneb"] = f32(inputs["emb_ln_b"])
    for ki, k in enumerate((1, 2, 3)):
        w = f32(inputs[f"conv_w{k}"])          # [NF, k, D]
        wt = np.ascontiguousarray(w.transpose(1, 2, 0))  # [k, D, NF]
        shared[f"cw{k}"] = np.ascontiguousarray(
            wt.reshape(k, ND, 128, NF).transpose(0, 2, 1, 3).astype(BF))
    # fc blocks in reference concat order (q_rep, b_rep, a_rep); index by
    # input branch id 0=q 1=a 2=b
    fcw = f32(inputs["fc_w"]).reshape(3, NCH, 128, NCLS)
    shared["fcw"] = np.ascontiguousarray(fcw[[0, 2, 1]].transpose(2, 0, 1, 3))
    if not fl["cb"]:
        shared["convb"] = np.stack(
            [f32(inputs[f"conv_b{k}"]).reshape(2, 128) for k in (1, 2, 3)])
    shared["word_emb"] = f32(inputs["word_emb"])

    all_ids = np.concatenate([np.asarray(inputs[p + "_input_ids"])
                              for p in ("q", "a", "b")]).astype(np.int32)
    all_masks = np.concatenate([np.asarray(inputs[p + "_attention_mask"])
                                for p in ("q", "a", "b")]).astype(np.int32)
    all_lens = all_masks.sum(1)

    in_maps = []
    for c in range(NCORES):
        sids = assign[c]
        ids_tiles, mb_tiles = [], []
        cmask = np.zeros((NSEQ, S + 1), dtype=np.float32)
        pen = np.zeros((NSEQ, 3, S + 1), dtype=np.float32)
        for j, gid in enumerate(sids):
            nt = seq_nt[j]
            for tt in range(nt):
                ids_tiles.append(all_ids[gid, tt * 128:(tt + 1) * 128])
                mb_tiles.append(
                    (all_masks[gid, tt * 128:(tt + 1) * 128] - 1) * 10000.0)
            cmask[j, 0:S] = all_masks[gid]
            L = all_lens[gid]
            swj = nt * 128 + 1
            for ki, k in enumerate((1, 2, 3)):
                valid = (np.arange(S + 1) + k - 1) <= L
                valid[swj - k + 1:] = False
                pen[j, ki] = np.where(valid, 0.0, -1e30)
        m = dict(shared)
        m["ids"] = np.ascontiguousarray(
            np.stack(ids_tiles).astype(np.int32).T)
        m["maskbias"] = np.ascontiguousarray(
            np.stack(mb_tiles).astype(np.float32).T)
        m["convmask"] = np.ascontiguousarray(cmask.astype(BF))
        m["convpen"] = np.ascontiguousarray(pen)
        in_maps.append(m)
    return in_maps


def _get_program(fl, pnt):
    key = (tuple(sorted(fl.items())), pnt)
    if key not in _CACHE:
        _CACHE[key] = _build_program(fl, pnt)
    return _CACHE[key]


def run_sharded(inputs, debug=False, **run_kwargs):
    """Shard, run on 8 cores, gather. Returns (output, BassKernelResults)."""
    from concourse.bass_utils import run_bass_kernel_spmd
    fl = _flags(inputs)
    ns, assign = _classify(inputs)
    nc = _get_program(fl, _pair_nts(ns))
    in_maps = _core_inputs(inputs, fl, ns, assign)
    res = run_bass_kernel_spmd(nc, in_maps, core_ids=list(range(NCORES)),
                               **run_kwargs)
    out = np.zeros((B, NCLS), dtype=np.float32)
    for c in range(NCORES):
        part = np.asarray(res.results[c]["out"], dtype=np.float32)  # [4,3,12]
        for j, gid in enumerate(assign[c]):
            br, sample = gid // B, gid % B
            out[sample] += part[:, br, j]
    out += np.asarray(inputs["fc_b"], dtype=np.float32)[None, :]
    return out, res


def kernel(**inputs):
    out, _ = run_sharded(inputs)
    return out


# revision 65
# speedup vs baseline: 1.0024x; 1.0024x over previous
"""Trainium2 Bass kernel for nn_BertCNN (3x BERT-small encoder + CNN maxpool head).

Strategy: data-parallel over *sequences* across 8 NeuronCores. The 96
sequences (32 samples x 3 branches) are classified by ragged length into
"short" (fits one 128-token tile) and "long" (two tiles), then dealt to
cores so every core gets the same composition (e.g. 6 short + 6 long)
and runs an identical SPMD program. Each core runs the 4-layer BERT
encoder + conv/maxpool head on its 12 sequences and emits per-sequence
partial logits [4cls, 3branch-hypotheses, 12seq]; the host selects the
real branch row per sequence, sums the 3 branch contributions of every
sample and adds fc_b (pure gather/unshard arithmetic).

Dataflow per core (all big matmuls bf16 operands, fp32 PSUM accumulation):
  - token embeddings gathered on-device via indirect DMA from word_emb
  - residual stream token-major fp32->bf16 in SBUF; a feature-major bf16
    transposed copy (PE-free DMA transpose, one XBAR call per pair)
    feeds the QKV / FFN matmuls
  - sequences processed in pairs (short pair = 2x1 tile, long = 2x2) so
    projection / FFN1 matmuls run at the widest moving-operand width
  - attention in S^T = [key, query] layout: the ragged-length mask folds
    into the Exp activation as a per-partition bias; exp(S^T) is the
    lhsT of the context matmul; softmax denominators come from a ones
    column appended to V
  - LayerNorm rstd via Quake-style bit-trick rsqrt + 2 Newton steps on
    the DVE: keeps Sqrt off the ACT engine so the only ACT table loads
    are the per-iteration Exp<->Gelu switch
  - conv head runs feature-major ([filters, windows]); global maxpool is
    a per-partition free-axis reduce_max; ragged window validity is a
    -1e30 penalty added before the max; short sequences use 129-wide
    windows instead of 257

Engine-queue discipline (from trace analysis of the v0 kernel):
  - sync queue: ONLY DMA transposes (+ final output store)
  - scalar queue (ACT HWDGE): prologue constants + layer-0 weights
  - gpsimd queue (SWDGE): embedding gathers, layer 1..3 / conv / fc
    weight prefetch, conv masks
  - per-iteration emission order keeps the PE queue dependency-clean:
    ctx -> [next-pair QKV backfill] -> WO -> [next V] -> [next scores g0]
    -> LN1 -> FFN1 -> FFN2 (ft-outer, chases the Gelu chain) ->
    [next scores g1] -> LN2 -> pair transpose -> conv (last layer)
"""

import numpy as np
import ml_dtypes

V, D, H, DH, NL, FF = 30522, 512, 8, 64, 4, 2048
NF, NCLS, B, S = 256, 4, 32, 256
NCORES, SPC = 8, 4
NSEQ = 3 * SPC          # 12 sequences per core
NPAIR = NSEQ // 2       # 6 sequence pairs
ND = D // 128            # 4 feature tiles
NFT = FF // 128          # 16 FFN hidden tiles
NCH = 6                  # per-branch fc chunks of 128 (3 kernels x 2 ftiles)

BF = ml_dtypes.bfloat16
F8 = ml_dtypes.float8_e4m3
_CACHE = {}
FP8_FFN = True           # fp8e4 DoubleRow matmuls for FFN1/FFN2
FFN_XS, FFN_WS = 8.0, 16.0   # fp8 quantization scales (powers of 2)
FP8_QKV = False          # fp8e4 DoubleRow for the QKV/WO projections too
#   (tried: saves ~90us PE but congests DVE/ACT and costs 0.4e-2 rel err)
#   (scores / softmax / ctx stay bf16; only the projections quantize)
QKV_XS, CTX_S = 16.0, 32.0   # x / ctx activation scales for fp8
QUAKE_LN = True          # DVE-only rsqrt (bit trick + Newton steps)
QUAKE_ITERS = 1          # Newton steps (1 -> 1.8e-3 rstd rel err, plenty here)
BCAST_NORM = True        # one stride-0-broadcast DVE op per ctx 4-head group
BF16_R = True            # residual/LN scratch tiles in bf16 (2x DVE modes)


def _flags(inputs):
    z = lambda a: bool(np.all(np.asarray(a) == 0))
    o = lambda a: bool(np.all(np.asarray(a) == 1))
    return {
        "bqk": z(inputs["bq"]) and z(inputs["bk"]),
        "bv": z(inputs["bv"]),
        "bo": z(inputs["bo"]),
        "bi": z(inputs["bi"]),
        "bo2": z(inputs["bo2"]),
        "ln": all(o(inputs[k]) for k in ("emb_ln_s", "ln1_s", "ln2_s"))
        and all(z(inputs[k]) for k in ("emb_ln_b", "ln1_b", "ln2_b")),
        "cb": z(inputs["conv_b1"]) and z(inputs["conv_b2"]) and z(inputs["conv_b3"]),
    }


def _pair_nts(ns):
    """Pair tile-counts for a core with ns short seqs: one short pair
    first (fast PE start), longs in the middle, shorts at the tail (small
    final conv)."""
    nps = ns // 2
    pn = []
    if nps > 0:
        pn.append(1)
    pn += [2] * (NPAIR - nps)
    pn += [1] * (nps - 1 if nps > 0 else 0)
    return tuple(pn)


def _build_program(fl, pn):
    import contextlib
    import concourse.bass as bass
    import concourse.mybir as mybir
    import concourse.tile as tile
    from concourse import bacc
    from concourse.masks import make_identity

    F32, BF16, I32 = mybir.dt.float32, mybir.dt.bfloat16, mybir.dt.int32
    U32 = mybir.dt.uint32
    AL, AF = mybir.AluOpType, mybir.ActivationFunctionType

    seq_nt = []
    for p in range(NPAIR):
        seq_nt += [pn[p], pn[p]]
    tbase = np.concatenate([[0], np.cumsum(seq_nt)]).astype(int)
    NT_TOT = int(tbase[-1])

    nc = bacc.Bacc("TRN2", target_bir_lowering=False, debug=False,
                   num_devices=NCORES)

    di = lambda n, s, d: nc.dram_tensor(n, s, d, kind="ExternalInput").ap()
    F8D = mybir.dt.float8e4
    PDT = F8D if FP8_QKV else BF16
    word = di("word_emb", [V, D], F32)
    ids_d = di("ids", [128, NT_TOT], I32)
    mb_d = di("maskbias", [128, NT_TOT], F32)
    posty_d = di("posty", [2, 128, D], F32)
    cmask_d = di("convmask", [NSEQ, S + 1], BF16)
    cpen_d = di("convpen", [NSEQ, 3, S + 1], F32)
    wq_d = [di(f"wq{l}", [128, ND, D], PDT) for l in range(NL)]
    wk_d = [di(f"wk{l}", [128, ND, D], PDT) for l in range(NL)]
    wv_d = [di(f"wv{l}", [128, ND, D], PDT) for l in range(NL)]
    wo_d = [di(f"wo{l}", [128, ND, D], PDT) for l in range(NL)]
    WDT = F8D if FP8_FFN else BF16
    wi_d = [di(f"wi{l}", [128, ND, FF], WDT) for l in range(NL)]
    wo2_d = [di(f"wo2{l}", [128, NFT, D], WDT) for l in range(NL)]
    cw_d = [di(f"cw{k}", [k, 128, ND, NF], BF16) for k in (1, 2, 3)]
    fcw_d = di("fcw", [128, 3, NCH, NCLS], F32)
    if not fl["bqk"]:
        bq_d = [di(f"bq{l}", [ND, 128], F32) for l in range(NL)]
        bk_d = [di(f"bk{l}", [ND, 128], F32) for l in range(NL)]
    if not fl["bv"]:
        bv_d = [di(f"bv{l}", [D], F32) for l in range(NL)]
    if not fl["bo"]:
        bo_d = [di(f"bo{l}", [D], F32) for l in range(NL)]
    if not fl["bi"]:
        bi_d = [di(f"bi{l}", [NFT, 128], F32) for l in range(NL)]
    if not fl["bo2"]:
        bo2_d = [di(f"bo2{l}", [D], F32) for l in range(NL)]
    if not fl["ln"]:
        elns_d = di("lnes", [D], F32)
        elnb_d = di("lneb", [D], F32)
        ln1s_d = [di(f"ln1s{l}", [D], F32) for l in range(NL)]
        ln1b_d = [di(f"ln1b{l}", [D], F32) for l in range(NL)]
        ln2s_d = [di(f"ln2s{l}", [D], F32) for l in range(NL)]
        ln2b_d = [di(f"ln2b{l}", [D], F32) for l in range(NL)]
    if not fl["cb"]:
        cb_d = di("convb", [3, 2, 128], F32)

    out_d = nc.dram_tensor("out", [NCLS, 3, NSEQ], F32,
                           kind="ExternalOutput").ap()

    with tile.TileContext(nc) as tc, contextlib.ExitStack() as ctx:
        consts = ctx.enter_context(tc.tile_pool(name="consts", bufs=1))
        state = ctx.enter_context(tc.tile_pool(name="state", bufs=1))
        wts = ctx.enter_context(tc.tile_pool(name="wts", bufs=1))
        big = ctx.enter_context(tc.tile_pool(name="big", bufs=1))
        work = ctx.enter_context(tc.tile_pool(name="work", bufs=2))
        small = ctx.enter_context(tc.tile_pool(name="small", bufs=4))
        ps_mm = ctx.enter_context(tc.tile_pool(name="ps_mm", bufs=6, space="PSUM"))
        ps_ctx = ctx.enter_context(tc.tile_pool(name="ps_ctx", bufs=2, space="PSUM"))

        # ---- prologue constants: scalar (ACT) HWDGE queue, critical first ----
        ids_sb = consts.tile([128, NT_TOT], I32, tag="ids")
        nc.scalar.dma_start(out=ids_sb[:], in_=ids_d)
        posty = consts.tile([128, 2, D], F32, tag="posty")
        nc.scalar.dma_start(out=posty[:], in_=posty_d.rearrange("t p d -> p t d"))
        mb_sb = consts.tile([128, NT_TOT], F32, tag="mb")
        nc.scalar.dma_start(out=mb_sb[:], in_=mb_d)
        ident = consts.tile([128, 128], BF16, tag="ident")
        magic = consts.tile([128, 8], I32, tag="magic")
        nc.vector.memset(magic[:], 0x5F3759DF)
        if not QUAKE_LN:
            eps_t = consts.tile([128, 1], F32, tag="eps")
            nc.vector.memset(eps_t[:], 1e-12)

        bcast = lambda ap, n: ap[None, :].to_broadcast([128, n])
        if not fl["ln"]:
            elns = consts.tile([128, D], F32, tag="elns")
            nc.scalar.dma_start(out=elns[:], in_=bcast(elns_d, D))
            elnb = consts.tile([128, D], F32, tag="elnb")
            nc.scalar.dma_start(out=elnb[:], in_=bcast(elnb_d, D))

        # persistent per-pair state: token-major residual + feature-major copy
        RDT = BF16 if BF16_R else F32
        XDT = F8D if FP8_QKV else BF16
        x_p = [state.tile([128, 2, pn[q], D], BF16, tag=f"xp{q}",
                          name=f"xp{q}") for q in range(NPAIR)]
        if FP8_QKV:
            xT = [state.tile([128, ND, 2 * pn[q] * 128], F8D, tag=f"xT{q}",
                             name=f"xT{q}") for q in range(NPAIR)]
        else:
            xT = [state.tile([128, 2, pn[q], ND, 128], BF16, tag=f"xT{q}",
                             name=f"xT{q}") for q in range(NPAIR)]
        rep = state.tile([128, NCH, NSEQ], F32, tag="rep")

        def ln_stats_batch(rs):
            """Pipelined LN stats for a list of [128, D] sources; returns
            (rcp, nmb) where rcp[:, i] = rstd_i, nmb[:, i] = -mean_i*rstd_i."""
            n = len(rs)
            mvt = small.tile([128, n, 2], F32, tag="mvt")
            for i, r in enumerate(rs):
                st = small.tile([128, 6], F32, tag="st", name=f"st{i}")
                nc.vector.bn_stats(out=st[:], in_=r)
                nc.vector.bn_aggr(out=mvt[:, i, :], in_=st[:])
            rcp = small.tile([128, n], F32, tag="rcp2")
            if QUAKE_LN:
                # rstd = rsqrt(var + eps): Quake bit-trick + 2 Newton steps,
                # all on the DVE (no ACT Sqrt -> no act-table thrash)
                vv = small.tile([128, n], F32, tag="vv")
                nc.vector.tensor_scalar_add(out=vv[:], in0=mvt[:, :, 1],
                                            scalar1=1e-12)
                nc.vector.tensor_scalar(
                    out=rcp[:].bitcast(I32), in0=vv[:].bitcast(I32),
                    scalar1=1, scalar2=0, op0=AL.logical_shift_right,
                    op1=AL.bypass)
                nc.vector.tensor_tensor(
                    out=rcp[:].bitcast(I32), in0=magic[:, 0:n],
                    in1=rcp[:].bitcast(I32), op=AL.subtract)
                t = small.tile([128, n], F32, tag="qt")
                for _ in range(QUAKE_ITERS):
                    nc.vector.tensor_tensor(out=t[:], in0=rcp[:], in1=rcp[:],
                                            op=AL.mult)
                    nc.vector.tensor_tensor(out=t[:], in0=t[:], in1=vv[:],
                                            op=AL.mult)
                    nc.vector.tensor_scalar(out=t[:], in0=t[:], scalar1=-0.5,
                                            scalar2=1.5, op0=AL.mult,
                                            op1=AL.add)
                    nc.vector.tensor_tensor(out=rcp[:], in0=rcp[:], in1=t[:],
                                            op=AL.mult)
            else:
                sd = small.tile([128, n], F32, tag="sd")
                nc.scalar.activation(out=sd[:], in_=mvt[:, :, 1],
                                     func=AF.Sqrt, bias=eps_t[:], scale=1.0)
                nc.vector.reciprocal(out=rcp[:], in_=sd[:])
            nmb = small.tile([128, n], F32, tag="nmb")
            # nmb = (mean * -1) * rstd
            nc.vector.scalar_tensor_tensor(
                out=nmb[:], in0=mvt[:, :, 0], scalar=-1.0, in1=rcp[:],
                op0=AL.mult, op1=AL.mult)
            return rcp, nmb

        def ln_apply_batch(rs, dsts, rcp, nmb, s_tile, b_tile):
            for i in range(len(rs)):
                if False and s_tile is None and b_tile is None and i % 2 == 1:
                    # odd tiles apply on ACT (Copy: in every table, no load)
                    # so the two engines drain the batch in parallel
                    nc.scalar.activation(
                        out=dsts[i], in_=rs[i], func=AF.Identity,
                        bias=nmb[:, i:i + 1], scale=rcp[:, i:i + 1])
                    continue
                nc.vector.tensor_scalar(
                    out=dsts[i], in0=rs[i], scalar1=rcp[:, i:i + 1],
                    scalar2=nmb[:, i:i + 1], op0=AL.mult, op1=AL.add)
                if s_tile is not None:
                    nc.vector.tensor_tensor(out=dsts[i], in0=dsts[i],
                                            in1=s_tile[:], op=AL.mult)
                if b_tile is not None:
                    nc.vector.tensor_tensor(out=dsts[i], in0=dsts[i],
                                            in1=b_tile[:], op=AL.add)

        def feat_major(pr, dst, scale):
            """PE-transpose x_p[pr] into a feature-major copy dst with a
            fused scale+cast drain. Transposes go tile-outer so they chase
            the LN applies tile-by-tile instead of waiting for the batch."""
            nt = pn[pr]
            sw = 2 * nt * 128
            for dt in range(ND):
                tps = ps_mm.tile([128, sw], BF16, tag="mm",
                                 name=f"fm{pr}_{dt}")
                for i in range(2 * nt):
                    si, tt = i // nt, i % nt
                    nc.tensor.transpose(
                        tps[:, i * 128:(i + 1) * 128],
                        x_p[pr][:, si, tt, dt * 128:(dt + 1) * 128],
                        ident[:])
                if scale == 1.0:
                    nc.vector.tensor_copy(out=dst[:, dt, :], in_=tps[:])
                else:
                    nc.vector.tensor_scalar_mul(out=dst[:, dt, :],
                                                in0=tps[:], scalar1=scale)

        def to_feat(pr, eng=None):
            """Refresh the feature-major x copy after an LN2 update."""
            if FP8_QKV:
                feat_major(pr, xT[pr], QKV_XS)
            else:
                (eng or nc.sync).dma_start_transpose(xT[pr][:, :, :, :, :],
                                                     x_p[pr][:, :, :, :])

        def embed_pair(p, pt_eng=None):
            nt = pn[p]
            t0 = int(tbase[2 * p])
            gb = work.tile([128, 2 * nt, D], F32, tag="r", name=f"g{p}",
                           bufs=2)
            tiles = []
            for si in range(2):
                for tt in range(nt):
                    j = si * nt + tt
                    nc.gpsimd.indirect_dma_start(
                        out=gb[:, j, :], out_offset=None, in_=word[:],
                        in_offset=bass.IndirectOffsetOnAxis(
                            ap=ids_sb[:, t0 + j:t0 + j + 1], axis=0))
                    nc.vector.tensor_tensor(out=gb[:, j, :], in0=gb[:, j, :],
                                            in1=posty[:, tt, :], op=AL.add)
                    tiles.append((si, tt))
            rcp, nmb = ln_stats_batch([gb[:, si * nt + tt, :]
                                       for si, tt in tiles])
            ln_apply_batch([gb[:, si * nt + tt, :] for si, tt in tiles],
                           [x_p[p][:, si, tt, :] for si, tt in tiles],
                           rcp, nmb,
                           None if fl["ln"] else elns,
                           None if fl["ln"] else elnb)
            to_feat(p, pt_eng)

        def load_layer_weights_A(l, q):
            """QKV weights (+ small per-layer consts) for layer l."""
            w = {}
            for nm, dd in (("wq", wq_d), ("wk", wk_d), ("wv", wv_d)):
                w[nm] = wts.tile([128, ND, D], PDT, tag=nm, name=f"{nm}_{l}")
                for dt in range(0, ND, 2):
                    q.dma_start(out=w[nm][:, dt:dt + 2, :],
                                in_=dd[l][:, dt:dt + 2, :])
            if not fl["bqk"]:
                w["bq"] = consts.tile([128, ND], F32, tag="bq", name=f"bq_{l}")
                q.dma_start(out=w["bq"][:], in_=bq_d[l].rearrange("t p -> p t"))
                w["bk"] = consts.tile([128, ND], F32, tag="bk", name=f"bk_{l}")
                q.dma_start(out=w["bk"][:], in_=bk_d[l].rearrange("t p -> p t"))
            if not fl["bv"]:
                w["bv"] = consts.tile([128, D], F32, tag="bv", name=f"bv_{l}")
                q.dma_start(out=w["bv"][:], in_=bcast(bv_d[l], D))
            if not fl["ln"]:
                for nm, dd in (("ln1s", ln1s_d), ("ln1b", ln1b_d),
                               ("ln2s", ln2s_d), ("ln2b", ln2b_d)):
                    w[nm] = consts.tile([128, D], F32, tag=nm, name=f"{nm}_{l}")
                    q.dma_start(out=w[nm][:], in_=bcast(dd[l], D))
            return w

        def load_layer_weights_B(l, w, q):
            """WO / FFN weights for layer l (emit after last layer-(l-1) use)."""
            w["wo"] = wts.tile([128, ND, D], PDT, tag="wo", name=f"wo_{l}")
            for dt in range(0, ND, 2):
                q.dma_start(out=w["wo"][:, dt:dt + 2, :],
                            in_=wo_d[l][:, dt:dt + 2, :])
            w["wi"] = wts.tile([128, ND, FF], WDT, tag="wi", name=f"wi_{l}")
            for dt in range(0, ND, 2):
                q.dma_start(out=w["wi"][:, dt:dt + 2, :],
                            in_=wi_d[l][:, dt:dt + 2, :])
            w["wo2"] = wts.tile([128, NFT, D], WDT, tag="wo2", name=f"wo2_{l}")
            for ft in range(0, NFT, 8):
                q.dma_start(out=w["wo2"][:, ft:ft + 8, :],
                            in_=wo2_d[l][:, ft:ft + 8, :])
            if not fl["bo"]:
                w["bo"] = consts.tile([128, D], F32, tag="bo", name=f"bo_{l}")
                q.dma_start(out=w["bo"][:], in_=bcast(bo_d[l], D))
            if not fl["bi"]:
                w["bi"] = consts.tile([128, NFT], F32, tag="bi", name=f"bi_{l}")
                q.dma_start(out=w["bi"][:], in_=bi_d[l].rearrange("t p -> p t"))
            if not fl["bo2"]:
                w["bo2"] = consts.tile([128, D], F32, tag="bo2", name=f"bo2_{l}")
                q.dma_start(out=w["bo2"][:], in_=bcast(bo2_d[l], D))
            return w

        # fp8 dequant scales for the projection drains (wq carries no /8 on
        # the host in fp8 mode; the score scaling folds in here instead)
        P_SC = 1.0 / (QKV_XS * FFN_WS) if FP8_QKV else 1.0
        Q_SC = P_SC * 0.125 if FP8_QKV else 1.0
        O_SC = 1.0 / (CTX_S * FFN_WS) if FP8_QKV else 1.0

        def proj_mm(ps, xts, wt, ocols):
            """One QKV-projection matmul chain (DR fp8 or bf16)."""
            if FP8_QKV:
                for dp in range(2):
                    nc.tensor.matmul(
                        ps[:], wt[:, 2 * dp:2 * dp + 2, ocols],
                        xts[:, 2 * dp:2 * dp + 2, :],
                        start=dp == 0, stop=dp == 1,
                        perf_mode=mybir.MatmulPerfMode.DoubleRow)
            else:
                for dt in range(ND):
                    nc.tensor.matmul(
                        ps[:], wt[:, dt, ocols], xts[:, :, :, dt, :],
                        start=dt == 0, stop=dt == ND - 1)

        def qkv_pair(pr, w):
            """qT/kT feature-major bf16 for both seqs."""
            nt = pn[pr]
            sw = 2 * nt * 128
            xts = xT[pr]
            qT = work.tile([128, ND, sw], BF16, tag="qT", name=f"qT{pr}")
            kT = work.tile([128, ND, sw], BF16, tag="kT", name=f"kT{pr}")
            for dst_t, wt, which in ((qT, w["wq"], "q"), (kT, w["wk"], "k")):
                sc = Q_SC if which == "q" else P_SC
                for ot in range(ND):
                    ps = ps_mm.tile([128, sw], F32, tag="mm")
                    proj_mm(ps, xts, wt, slice(ot * 128, (ot + 1) * 128))
                    if fl["bqk"]:
                        # split PSUM drain across ACT and DVE so neither
                        # engine's queue gates PSUM recycling
                        if which == "q":
                            nc.scalar.mul(out=dst_t[:, ot, :], in_=ps[:],
                                          mul=sc)
                        elif sc == 1.0:
                            nc.vector.tensor_copy(out=dst_t[:, ot, :],
                                                  in_=ps[:])
                        else:
                            nc.vector.tensor_scalar_mul(
                                out=dst_t[:, ot, :], in0=ps[:], scalar1=sc)
                    else:
                        bt = w["bq"] if which == "q" else w["bk"]
                        nc.vector.tensor_scalar(
                            out=dst_t[:, ot, :], in0=ps[:], scalar1=sc,
                            scalar2=bt[:, ot:ot + 1], op0=AL.mult, op1=AL.add)
            return qT, kT

        def v_pair(pr, w):
            """V token-major bf16 with a ones column per head, both seqs."""
            nt = pn[pr]
            xts = xT[pr]
            vAs = []
            for si in range(2):
                vA = work.tile([128, nt, H, DH + 1], BF16, tag="vA",
                               name=f"vA{pr}_{si}")
                nc.vector.memset(vA[:, :, :, DH:DH + 1], 1.0)
                for tt in range(nt):
                    tok = (si * nt + tt) * 128
                    ps = ps_mm.tile([128, D], F32, tag="mm")
                    if FP8_QKV:
                        for dp in range(2):
                            nc.tensor.matmul(
                                ps[:], xts[:, 2 * dp:2 * dp + 2, tok:tok + 128],
                                w["wv"][:, 2 * dp:2 * dp + 2, :],
                                start=dp == 0, stop=dp == 1,
                                perf_mode=mybir.MatmulPerfMode.DoubleRow)
                    else:
                        for dt in range(ND):
                            nc.tensor.matmul(
                                ps[:], xts[:, si, tt, dt, :],
                                w["wv"][:, dt, :], start=dt == 0,
                                stop=dt == ND - 1)
                    if fl["bv"]:
                        if P_SC == 1.0:
                            nc.vector.tensor_copy(
                                out=vA[:, tt, :, 0:DH],
                                in_=ps.rearrange("p (h d) -> p h d", h=H))
                        else:
                            nc.vector.tensor_scalar_mul(
                                out=vA[:, tt, :, 0:DH],
                                in0=ps.rearrange("p (h d) -> p h d", h=H),
                                scalar1=P_SC)
                    else:
                        nc.vector.scalar_tensor_tensor(
                            out=vA[:, tt, :, 0:DH],
                            in0=ps.rearrange("p (h d) -> p h d", h=H),
                            scalar=P_SC,
                            in1=w["bv"].rearrange("p (h d) -> p h d", h=H),
                            op0=AL.mult, op1=AL.add)
                vAs.append(vA)
            return vAs

        att_state = {}

        def att_scores(pr, g_, qk):
            """S^T + exp for head-group g_ of both seqs of pair pr."""
            nt = pn[pr]
            sq = nt * 128       # queries per seq
            qT, kT = qk
            for si in range(2):
                seq = 2 * pr + si
                so = si * sq
                eT = work.tile([128, nt, 4, sq], BF16, tag="eT",
                               name=f"eT{pr}_{g_}_{si}", bufs=4)
                att_state[(pr, g_, si)] = eT
                for kt in range(nt):
                    ti = int(tbase[seq]) + kt
                    for hi in range(4):
                        h = g_ * 4 + hi
                        ot, hh = h // 2, (h % 2) * DH
                        ps = ps_mm.tile([128, sq], F32, tag="mm")
                        nc.tensor.matmul(
                            ps[:],
                            kT[hh:hh + DH, ot, so + kt * 128:so + (kt + 1) * 128],
                            qT[hh:hh + DH, ot, so:so + sq],
                            start=True, stop=True)
                        nc.scalar.activation(
                            out=eT[:, kt, hi, :], in_=ps[:],
                            func=AF.Exp, bias=mb_sb[:, ti:ti + 1], scale=1.0)

        def att_ctx(pr, g_, vAs, ctxb):
            """ctx (+denominator) matmuls and DVE normalization for group g_."""
            nt = pn[pr]
            for si in range(2):
                eT, vA = att_state.pop((pr, g_, si)), vAs[si]
                for qt in range(nt):
                    cps = ps_ctx.tile([128, 4 * (DH + 1)], F32, tag="ctx",
                                      name=f"ctx{si}_{qt}_{g_}")
                    for hi in range(4):
                        h = g_ * 4 + hi
                        sl = slice(hi * (DH + 1), (hi + 1) * (DH + 1))
                        for kt in range(nt):
                            nc.tensor.matmul(
                                cps[:, sl],
                                eT[:, kt, hi, qt * 128:(qt + 1) * 128],
                                vA[:, kt, h, :], start=kt == 0,
                                stop=kt == nt - 1)
                    cph = cps.rearrange("p (h c) -> p h c", c=DH + 1)
                    if BCAST_NORM:
                        rcp = small.tile([128, 4, 1], F32, tag="rcp")
                        nc.vector.reciprocal(out=rcp[:],
                                             in_=cph[:, :, DH:DH + 1])
                        nc.vector.scalar_tensor_tensor(
                            out=ctxb[:, si, qt,
                                     g_ * 4 * DH:(g_ + 1) * 4 * DH].rearrange(
                                         "p (h d) -> p h d", d=DH),
                            in0=cph[:, :, 0:DH], scalar=1.0,
                            in1=rcp[:, :, 0:1].broadcast_to([128, 4, DH]),
                            op0=AL.mult, op1=AL.mult)
                    else:
                        rcp = small.tile([128, 4], F32, tag="rcp")
                        nc.vector.reciprocal(out=rcp[:], in_=cph[:, :, DH])
                        for hi in range(4):
                            h = g_ * 4 + hi
                            base = hi * (DH + 1)
                            nc.vector.tensor_scalar_mul(
                                out=ctxb[:, si, qt, h * DH:(h + 1) * DH],
                                in0=cps[:, base:base + DH],
                                scalar1=rcp[:, hi:hi + 1])

        def residual_ln(rs, dsts, s_tile, b_tile):
            rcp, nmb = ln_stats_batch(rs)
            ln_apply_batch(rs, dsts, rcp, nmb, s_tile, b_tile)

        def ctx_half(pr, g_, vAs, ctxb, ct):
            """ctx matmuls + norm for head-group g_, then PE-transpose that
            group's two feature tiles into ct with ACT copy drains (copy is
            in every act table -> no table load, ~1us latency to WO)."""
            nt = pn[pr]
            sw = 2 * nt * 128
            att_ctx(pr, g_, vAs, ctxb)
            for dt in (2 * g_, 2 * g_ + 1):
                tpx = ps_mm.tile([128, sw], BF16, tag="mm",
                                 name=f"ctp{pr}_{dt}")
                for i in range(2 * nt):
                    si, tt = i // nt, i % nt
                    nc.tensor.transpose(
                        tpx[:, i * 128:(i + 1) * 128],
                        ctxb[:, si, tt, dt * 128:(dt + 1) * 128],
                        ident[:])
                if FP8_QKV:
                    nc.scalar.mul(out=ct[:, dt, :], in_=tpx[:], mul=CTX_S)
                else:
                    nc.scalar.copy(out=ct[:, dt, :], in_=tpx[:])

        def ctx_start(pr, vAs):
            """Allocate this pair's ctx tiles and run head-group 0."""
            nt = pn[pr]
            ctxb = work.tile([128, 2, nt, D], BF16, tag="ctxb",
                             name=f"cb{pr}")
            ct = work.tile([128, ND, 2 * nt * 128], XDT, tag="cT",
                           name=f"cT{pr}")
            ctx_half(pr, 0, vAs, ctxb, ct)
            return ctxb, ct

        cw = {}

        def load_conv_weights(q):
            ti = 0
            for ki, k in enumerate((1, 2, 3)):
                for j in range(k):
                    t = wts.tile([128, ND, NF], BF16, tag=f"cw{ti}",
                                 name=f"cwt{k}_{j}")
                    q.dma_start(out=t[:], in_=cw_d[ki][j])
                    cw[(k, j)] = t
                    ti += 1
            fcw = consts.tile([128, 3, NCH, NCLS], F32, tag="fcw")
            q.dma_start(out=fcw[:], in_=fcw_d)
            cb = None
            if not fl["cb"]:
                cb = consts.tile([128, 3, 2], F32, tag="cb")
                q.dma_start(out=cb[:], in_=cb_d.rearrange("k t p -> p k t"))
            return fcw, cb

        def conv_prefetch(pr):
            """Issue the conv mask / window-penalty DMAs for pair pr early."""
            nt = pn[pr]
            swc = nt * 128 + 1
            cms, pens = [], []
            for si in range(2):
                seq = 2 * pr + si
                cm = work.tile([128, swc], BF16, tag="cm", name=f"cm{seq}")
                nc.gpsimd.dma_start(
                    out=cm[:],
                    in_=cmask_d[seq, 0:swc][None, :].to_broadcast([128, swc]))
                cms.append(cm)
                ps_ = []
                for ki in range(3):
                    pen = work.tile([128, swc], F32, tag="pen",
                                    name=f"pen{seq}_{ki}", bufs=6)
                    nc.gpsimd.dma_start(
                        out=pen[:],
                        in_=cpen_d[seq, ki, 0:swc][None, :].to_broadcast(
                            [128, swc]))
                    ps_.append(pen)
                pens.append(ps_)
            return cms, pens

        def conv_pair(pr, cms, pens):
            """Conv head for both seqs of pair pr straight off x_p: PE
            transposes feature tiles into PSUM, the drain fuses the
            token-mask multiply (no DMA transpose on the conv path)."""
            nt = pn[pr]
            swc = nt * 128 + 1
            for si in range(2):
                seq = 2 * pr + si
                xcv = work.tile([128, ND, swc], BF16, tag="xcv",
                                name=f"xcv{seq}")
                nc.vector.memset(xcv[:, :, nt * 128:nt * 128 + 1], 0.0)
                for dt in range(ND):
                    tpc = ps_mm.tile([128, nt * 128], BF16, tag="mm",
                                     name=f"cvtp{seq}_{dt}")
                    for tt in range(nt):
                        nc.tensor.transpose(
                            tpc[:, tt * 128:(tt + 1) * 128],
                            x_p[pr][:, si, tt, dt * 128:(dt + 1) * 128],
                            ident[:])
                    nc.vector.tensor_tensor(out=xcv[:, dt, 0:nt * 128],
                                            in0=tpc[:],
                                            in1=cms[si][:, 0:nt * 128],
                                            op=AL.mult)
                for ki, k in enumerate((1, 2, 3)):
                    nw = swc - k + 1
                    for ft in range(2):
                        ps = ps_mm.tile([128, swc], F32, tag="mm")
                        idx = 0
                        for dt in range(ND):
                            for j in range(k):
                                nc.tensor.matmul(
                                    ps[:, 0:nw],
                                    cw[(k, j)][:, dt, ft * 128:(ft + 1) * 128],
                                    xcv[:, dt, j:j + nw],
                                    start=idx == 0, stop=idx == ND * k - 1)
                                idx += 1
                        cvt = work.tile([128, swc], F32, tag="cvt",
                                        name=f"cv{seq}_{k}_{ft}")
                        nc.vector.tensor_tensor(out=cvt[:, 0:nw],
                                                in0=ps[:, 0:nw],
                                                in1=pens[si][ki][:, 0:nw],
                                                op=AL.add)
                        nc.vector.tensor_reduce(
                            out=rep[:, ki * 2 + ft, seq:seq + 1],
                            in_=cvt[:, 0:nw],
                            axis=mybir.AxisListType.X, op=AL.max)

        # ---- main schedule ----
        # prologue: embed gathers lead the DGE, layer-0 weights trail them
        # on the gpsimd queue in need order (wq/wk -> wv -> wo/wi/wo2)
        make_identity(nc, ident[:])
        # p-state warmup: keep the PE streaming while the embed/weight
        # chain runs so the first real matmuls start at full clock
        warm = consts.tile([128, 512], BF16, tag="warm")
        nc.vector.memset(warm[:], 0.5)
        wps = ps_mm.tile([128, 512], F32, tag="mm", name="warm")
        for _ in range(30):
            nc.tensor.matmul(wps[:], ident[:], warm[:], start=True, stop=True)
        embed_pair(0, pt_eng=nc.scalar)
        w_cur = {}
        for nm, dd in (("wq", wq_d), ("wk", wk_d)):
            w_cur[nm] = wts.tile([128, ND, D], PDT, tag=nm, name=f"{nm}_0")
            for dt in range(0, ND, 2):
                nc.gpsimd.dma_start(out=w_cur[nm][:, dt:dt + 2, :],
                                    in_=dd[0][:, dt:dt + 2, :])
        embed_pair(1, pt_eng=nc.scalar)
        w_cur["wv"] = wts.tile([128, ND, D], PDT, tag="wv", name="wv_0")
        for dt in range(0, ND, 2):
            nc.gpsimd.dma_start(out=w_cur["wv"][:, dt:dt + 2, :],
                                in_=wv_d[0][:, dt:dt + 2, :])
        if not fl["bqk"]:
            w_cur["bq"] = consts.tile([128, ND], F32, tag="bq", name="bq_0")
            nc.gpsimd.dma_start(out=w_cur["bq"][:],
                                in_=bq_d[0].rearrange("t p -> p t"))
            w_cur["bk"] = consts.tile([128, ND], F32, tag="bk", name="bk_0")
            nc.gpsimd.dma_start(out=w_cur["bk"][:],
                                in_=bk_d[0].rearrange("t p -> p t"))
        if not fl["bv"]:
            w_cur["bv"] = consts.tile([128, D], F32, tag="bv", name="bv_0")
            nc.gpsimd.dma_start(out=w_cur["bv"][:], in_=bcast(bv_d[0], D))
        if not fl["ln"]:
            for nm, dd in (("ln1s", ln1s_d), ("ln1b", ln1b_d),
                           ("ln2s", ln2s_d), ("ln2b", ln2b_d)):
                w_cur[nm] = consts.tile([128, D], F32, tag=nm, name=f"{nm}_0")
                nc.gpsimd.dma_start(out=w_cur[nm][:], in_=bcast(dd[0], D))
        w_cur = load_layer_weights_B(0, w_cur, nc.gpsimd)
        qk_cur = qkv_pair(0, w_cur)
        v_cur = v_pair(0, w_cur)
        att_scores(0, 0, qk_cur)
        att_scores(0, 1, qk_cur)
        fcw = cb = None
        for l in range(NL):
            for pr in range(NPAIR):
                nt = pn[pr]
                w = w_cur
                if pr + 1 < NPAIR:
                    nxt_l, nxt_pr = l, pr + 1
                elif l + 1 < NL:
                    nxt_l, nxt_pr = l + 1, 0
                else:
                    nxt_l = nxt_pr = None
                cross = nxt_pr is not None and nxt_l != l

                if cross:
                    # QKV weights of the next layer: all layer-l readers of
                    # wq/wk/wv were emitted by the previous iteration
                    w_nxt = load_layer_weights_A(nxt_l, nc.gpsimd)
                elif nxt_pr is not None:
                    w_nxt = w
                if l == NL - 1:
                    cms, pens = conv_prefetch(pr)

                sw = 2 * nt * 128
                ctxb, ct = ctx_start(pr, v_cur)
                ctx_half(pr, 1, v_cur, ctxb, ct)
                # PE backfill: the next pair's QKV projections
                if nxt_pr is not None:
                    qk_nxt = qkv_pair(nxt_pr, w_nxt)
                # attention out projection + residual
                rs = []
                for i in range(2 * nt):
                    si, tt = i // nt, i % nt
                    ps = ps_mm.tile([128, D], F32, tag="mm")
                    if FP8_QKV:
                        for dp in range(2):
                            nc.tensor.matmul(
                                ps[:], ct[:, 2 * dp:2 * dp + 2,
                                           i * 128:(i + 1) * 128],
                                w["wo"][:, 2 * dp:2 * dp + 2, :],
                                start=dp == 0, stop=dp == 1,
                                perf_mode=mybir.MatmulPerfMode.DoubleRow)
                    else:
                        for dt in range(ND):
                            nc.tensor.matmul(
                                ps[:], ct[:, dt, i * 128:(i + 1) * 128],
                                w["wo"][:, dt, :], start=dt == 0,
                                stop=dt == ND - 1)
                    r = work.tile([128, D], RDT, tag="rln", name=f"r{i}", bufs=4)
                    if O_SC == 1.0:
                        nc.vector.tensor_tensor(out=r[:], in0=ps[:],
                                                in1=x_p[pr][:, si, tt, :],
                                                op=AL.add)
                    else:
                        nc.vector.scalar_tensor_tensor(
                            out=r[:], in0=ps[:], scalar=O_SC,
                            in1=x_p[pr][:, si, tt, :], op0=AL.mult, op1=AL.add)
                    if not fl["bo"]:
                        nc.vector.tensor_tensor(out=r[:], in0=r[:],
                                                in1=w["bo"][:], op=AL.add)
                    rs.append(r[:])
                # more PE backfill: next pair's V and both score groups run
                # while the LN1 chain (pure DVE now) drains
                if nxt_pr is not None:
                    v_nxt = v_pair(nxt_pr, w_nxt)
                    att_scores(nxt_pr, 0, qk_nxt)
                residual_ln(rs, [x_p[pr][:, i // nt, i % nt, :]
                                 for i in range(2 * nt)],
                            None if fl["ln"] else w["ln1s"],
                            None if fl["ln"] else w["ln1b"])
                # PE-transpose the LN1 output straight into PSUM (bf16),
                # then one drain per dt does the fp8 cast + scale
                y1 = work.tile([128, ND, sw], WDT, tag="y1f8",
                               name=f"y1f8{pr}")
                feat_major(pr, y1, FFN_XS if FP8_FFN else 1.0)
                # FFN1: hidden feature-major, gelu fused with bias
                hT = big.tile([128, NFT, sw], F8D if FP8_FFN else BF16,
                              tag="hT")
                for ft in range(NFT):
                    ps = ps_mm.tile([128, sw], F32, tag="mm")
                    if FP8_FFN:
                        for dp in range(2):
                            nc.tensor.matmul(
                                ps[:],
                                w["wi"][:, 2 * dp:2 * dp + 2,
                                        ft * 128:(ft + 1) * 128],
                                y1[:, 2 * dp:2 * dp + 2, :],
                                start=dp == 0, stop=dp == 1,
                                perf_mode=mybir.MatmulPerfMode.DoubleRow)
                    else:
                        for dt in range(ND):
                            nc.tensor.matmul(
                                ps[:], w["wi"][:, dt, ft * 128:(ft + 1) * 128],
                                y1[:, dt, :], start=dt == 0,
                                stop=dt == ND - 1)
                    nc.scalar.activation(
                        out=hT[:, ft, :], in_=ps[:], func=AF.Gelu,
                        bias=0.0 if fl["bi"] else w["bi"][:, ft:ft + 1],
                        scale=1.0 / (FFN_XS * FFN_WS) if FP8_FFN else 1.0)
                # embeds of the remaining pairs ride the FFN window (their
                # DVE chain slots between the y1f8 and FFN2 drains)
                if l == 0 and pr < 4:
                    embed_pair(pr + 2)
                # FFN2 + residual: ft-outer with per-token-tile PSUM so the
                # first matmuls chase the Gelu chain instead of waiting on it
                pss = [ps_mm.tile([128, D], F32, tag="mm",
                                  name=f"f2_{l}_{pr}_{i}")
                       for i in range(2 * nt)]
                if FP8_FFN:
                    for fp_ in range(NFT // 2):
                        for i in range(2 * nt):
                            si, tt = i // nt, i % nt
                            so = si * nt * 128
                            nc.tensor.matmul(
                                pss[i][:],
                                hT[:, 2 * fp_:2 * fp_ + 2,
                                   so + tt * 128:so + (tt + 1) * 128],
                                w["wo2"][:, 2 * fp_:2 * fp_ + 2, :],
                                start=fp_ == 0, stop=fp_ == NFT // 2 - 1,
                                perf_mode=mybir.MatmulPerfMode.DoubleRow)
                else:
                    for ft in range(NFT):
                        for i in range(2 * nt):
                            si, tt = i // nt, i % nt
                            so = si * nt * 128
                            nc.tensor.matmul(
                                pss[i][:],
                                hT[:, ft, so + tt * 128:so + (tt + 1) * 128],
                                w["wo2"][:, ft, :], start=ft == 0,
                                stop=ft == NFT - 1)
                rs = []
                for i in range(2 * nt):
                    si, tt = i // nt, i % nt
                    r = work.tile([128, D], RDT, tag="rln", name=f"r2{i}", bufs=4)
                    if FP8_FFN:
                        nc.vector.scalar_tensor_tensor(
                            out=r[:], in0=pss[i][:], scalar=1.0 / FFN_WS,
                            in1=x_p[pr][:, si, tt, :], op0=AL.mult, op1=AL.add)
                    else:
                        nc.vector.tensor_tensor(out=r[:], in0=pss[i][:],
                                                in1=x_p[pr][:, si, tt, :],
                                                op=AL.add)
                    if not fl["bo2"]:
                        nc.vector.tensor_tensor(out=r[:], in0=r[:],
                                                in1=w["bo2"][:], op=AL.add)
                    rs.append(r[:])
                if cross:
                    # WO/FFN weights of the next layer: all layer-l readers
                    # of wo/wi/wo2 are emitted above
                    w_nxt = load_layer_weights_B(nxt_l, w_nxt, nc.gpsimd)
                if l == 1 and pr == 0:
                    fcw, cb = load_conv_weights(nc.gpsimd)
                # PE backfill while the LN2 chain runs: second score group
                if nxt_pr is not None:
                    att_scores(nxt_pr, 1, qk_nxt)
                residual_ln(rs, [x_p[pr][:, i // nt, i % nt, :]
                                 for i in range(2 * nt)],
                            None if fl["ln"] else w["ln2s"],
                            None if fl["ln"] else w["ln2b"])
                if l < NL - 1:
                    to_feat(pr)
                if nxt_pr is not None:
                    qk_cur, v_cur, w_cur = qk_nxt, v_nxt, w_nxt
                if l == NL - 1:
                    conv_pair(pr, cms, pens)

        if not fl["cb"]:
            for ki in range(3):
                for ft in range(2):
                    co = ki * 2 + ft
                    nc.vector.tensor_scalar_add(
                        out=rep[:, co, :], in0=rep[:, co, :],
                        scalar1=cb[:, ki, ft:ft + 1])
        nc.scalar.activation(out=rep[:], in_=rep[:], func=AF.Relu)

        # partial logits per branch hypothesis: fps[:, b, :] = fcw_b^T @ rep
        fps = ps_mm.tile([128, 3, NSEQ], F32, tag="mm", name="fps")
        for b_ in range(3):
            for co in range(NCH):
                nc.tensor.matmul(fps[0:NCLS, b_, :], fcw[:, b_, co, :],
                                 rep[:, co, :],
                                 start=co == 0, stop=co == NCH - 1)
        ob = small.tile([NCLS, 3, NSEQ], F32, tag="ob")
        nc.scalar.copy(out=ob[:], in_=fps[0:NCLS, :, :])
        nc.sync.dma_start(out=out_d[:], in_=ob[:])

    nc.compile()
    return nc


def _classify(inputs):
    """Compute per-core composition and the seq->(core, slot) assignment.

    Returns (ns, assign) where assign[core] is a list of NSEQ global
    sequence ids (branch*32 + sample) in slot order."""
    lens = []
    for p in ("q", "a", "b"):
        lens.append(np.asarray(inputs[p + "_attention_mask"]).sum(1))
    lens = np.concatenate(lens)          # [96], id = branch*32+sample
    short_ids = np.where(lens <= 128)[0]
    ns = min(len(short_ids) // NCORES, NSEQ)
    ns -= ns % 2
    n_short = ns * NCORES
    order = np.argsort(lens, kind="stable")
    shorts = [i for i in order if lens[i] <= 128][:n_short]
    short_set = set(shorts)
    longs = [i for i in order[::-1] if i not in short_set]
    pnt = _pair_nts(ns)
    assign = []
    for c in range(NCORES):
        my_s = shorts[c * ns:(c + 1) * ns]
        my_l = longs[c * (NSEQ - ns):(c + 1) * (NSEQ - ns)]
        si, li = 0, 0
        slots = []
        for p in range(NPAIR):
            for _ in range(2):
                if pnt[p] == 1:
                    slots.append(my_s[si]); si += 1
                else:
                    slots.append(my_l[li]); li += 1
        assign.append(slots)
    return ns, assign


def _core_inputs(inputs, fl, ns, assign):
    f32 = lambda a: np.ascontiguousarray(np.asarray(a, dtype=np.float32))
    tile_w = lambda w: np.ascontiguousarray(
        f32(w).reshape(w.shape[0] // 128, 128, w.shape[1])
        .transpose(1, 0, 2).astype(BF))
    tile_w8 = lambda w: np.ascontiguousarray(
        (f32(w) * FFN_WS).reshape(w.shape[0] // 128, 128, w.shape[1])
        .transpose(1, 0, 2).astype(F8))

    pnt = _pair_nts(ns)
    seq_nt = []
    for p in range(NPAIR):
        seq_nt += [pnt[p], pnt[p]]

    shared = {}
    shared["posty"] = np.ascontiguousarray(
        (f32(inputs["pos_emb"][:S]) + f32(inputs["type_emb"][0])).reshape(
            2, 128, D))
    for l in range(NL):
        if FP8_QKV:
            # no host /8 on Wq in fp8 (subnormal risk); folded in the drain
            shared[f"wq{l}"] = tile_w8(inputs["Wq"][l])
            shared[f"wk{l}"] = tile_w8(inputs["Wk"][l])
            shared[f"wv{l}"] = tile_w8(inputs["Wv"][l])
            shared[f"wo{l}"] = tile_w8(inputs["Wo"][l])
        else:
            shared[f"wq{l}"] = tile_w(f32(inputs["Wq"][l]) / 8.0)
            shared[f"wk{l}"] = tile_w(inputs["Wk"][l])
            shared[f"wv{l}"] = tile_w(inputs["Wv"][l])
            shared[f"wo{l}"] = tile_w(inputs["Wo"][l])
        if FP8_FFN:
            shared[f"wi{l}"] = tile_w8(inputs["Wi"][l])
            shared[f"wo2{l}"] = tile_w8(inputs["Wo2"][l])
        else:
            shared[f"wi{l}"] = tile_w(inputs["Wi"][l])
            shared[f"wo2{l}"] = tile_w(inputs["Wo2"][l])
        if not fl["bqk"]:
            shared[f"bq{l}"] = f32(inputs["bq"][l]).reshape(ND, 128) / 8.0
            shared[f"bk{l}"] = f32(inputs["bk"][l]).reshape(ND, 128)
        if not fl["bv"]:
            shared[f"bv{l}"] = f32(inputs["bv"][l])
        if not fl["bo"]:
            shared[f"bo{l}"] = f32(inputs["bo"][l])
        if not fl["bi"]:
            shared[f"bi{l}"] = f32(inputs["bi"][l]).reshape(NFT, 128)
        if not fl["bo2"]:
            shared[f"bo2{l}"] = f32(inputs["bo2"][l])
        if not fl["ln"]:
            shared[f"ln1s{l}"] = f32(inputs["ln1_s"][l])
            shared[f"ln1b{l}"] = f32(inputs["ln1_b"][l])
            shared[f"ln2s{l}"] = f32(inputs["ln2_s"][l])
            shared[f"ln2b{l}"] = f32(inputs["ln2_b"][l])
    if not fl["ln"]:
        shared["lnes"] = f32(inputs["emb_ln_s"])
        shared["lneb"] = f32(inputs["emb_ln_b"])
    for ki, k in enumerate((1, 2, 3)):
        w = f32(inputs[f"conv_w{k}"])          # [NF, k, D]
        wt = np.ascontiguousarray(w.transpose(1, 2, 0))  # [k, D, NF]
        shared[f"cw{k}"] = np.ascontiguousarray(
            wt.reshape(k, ND, 128, NF).transpose(0, 2, 1, 3).astype(BF))
    # fc blocks in reference concat order (q_rep, b_rep, a_rep); index by
    # input branch id 0=q 1=a 2=b
    fcw = f32(inputs["fc_w"]).reshape(3, NCH, 128, NCLS)
    shared["fcw"] = np.ascontiguousarray(fcw[[0, 2, 1]].transpose(2, 0, 1, 3))
    if not fl["cb"]:
        shared["convb"] = np.stack(
            [f32(inputs[f"conv_b{k}"]).reshape(2, 128) for k in (1, 2, 3)])
    shared["word_emb"] = f32(inputs["word_emb"])

    all_ids = np.concatenate([np.asarray(inputs[p + "_input_ids"])
                              for p in ("q", "a", "b")]).astype(np.int32)
    all_masks = np.concatenate([np.asarray(inputs[p + "_attention_mask"])
                                for p in ("q", "a", "b")]).astype(np.int32)
    all_lens = all_masks.sum(1)

    in_maps = []
    for c in range(NCORES):
        sids = assign[c]
        ids_tiles, mb_tiles = [], []
        cmask = np.zeros((NSEQ, S + 1), dtype=np.float32)
        pen = np.zeros((NSEQ, 3, S + 1), dtype=np.float32)
        for j, gid in enumerate(sids):
            nt = seq_nt[j]
            for tt in range(nt):
                ids_tiles.append(all_ids[gid, tt * 128:(tt + 1) * 128])
                mb_tiles.append(
                    (all_masks[gid, tt * 128:(tt + 1) * 128] - 1) * 10000.0)
            cmask[j, 0:S] = all_masks[gid]
            L = all_lens[gid]
            swj = nt * 128 + 1
            for ki, k in enumerate((1, 2, 3)):
                valid = (np.arange(S + 1) + k - 1) <= L
                valid[swj - k + 1:] = False
                pen[j, ki] = np.where(valid, 0.0, -1e30)
        m = dict(shared)
        m["ids"] = np.ascontiguousarray(
            np.stack(ids_tiles).astype(np.int32).T)
        m["maskbias"] = np.ascontiguousarray(
            np.stack(mb_tiles).astype(np.float32).T)
        m["convmask"] = np.ascontiguousarray(cmask.astype(BF))
        m["convpen"] = np.ascontiguousarray(pen)
        in_maps.append(m)
    return in_maps


def _get_program(fl, pnt):
    key = (tuple(sorted(fl.items())), pnt)
    if key not in _CACHE:
        _CACHE[key] = _build_program(fl, pnt)
    return _CACHE[key]


def run_sharded(inputs, debug=False, **run_kwargs):
    """Shard, run on 8 cores, gather. Returns (output, BassKernelResults)."""
    from concourse.bass_utils import run_bass_kernel_spmd
    fl = _flags(inputs)
    ns, assign = _classify(inputs)
    nc = _get_program(fl, _pair_nts(ns))
    in_maps = _core_inputs(inputs, fl, ns, assign)
    res = run_bass_kernel_spmd(nc, in_maps, core_ids=list(range(NCORES)),
                               **run_kwargs)
    out = np.zeros((B, NCLS), dtype=np.float32)
    for c in range(NCORES):
        part = np.asarray(res.results[c]["out"], dtype=np.float32)  # [4,3,12]
        for j, gid in enumerate(assign[c]):
            br, sample = gid // B, gid % B
            out[sample] += part[:, br, j]
    out += np.asarray(inputs["fc_b"], dtype=np.float32)[None, :]
    return out, res


def kernel(**inputs):
    out, _ = run_sharded(inputs)
    return out


# revision 66
# speedup vs baseline: 1.0172x; 1.0148x over previous
"""Trainium2 Bass kernel for nn_BertCNN (3x BERT-small encoder + CNN maxpool head).

Strategy: data-parallel over *sequences* across 8 NeuronCores. The 96
sequences (32 samples x 3 branches) are classified by ragged length into
"short" (fits one 128-token tile) and "long" (two tiles), then dealt to
cores so every core gets the same composition (e.g. 6 short + 6 long)
and runs an identical SPMD program. Each core runs the 4-layer BERT
encoder + conv/maxpool head on its 12 sequences and emits per-sequence
partial logits [4cls, 3branch-hypotheses, 12seq]; the host selects the
real branch row per sequence, sums the 3 branch contributions of every
sample and adds fc_b (pure gather/unshard arithmetic).

Dataflow per core (all big matmuls bf16 operands, fp32 PSUM accumulation):
  - token embeddings gathered on-device via indirect DMA from word_emb
  - residual stream token-major fp32->bf16 in SBUF; a feature-major bf16
    transposed copy (PE-free DMA transpose, one XBAR call per pair)
    feeds the QKV / FFN matmuls
  - sequences processed in pairs (short pair = 2x1 tile, long = 2x2) so
    projection / FFN1 matmuls run at the widest moving-operand width
  - attention in S^T = [key, query] layout: the ragged-length mask folds
    into the Exp activation as a per-partition bias; exp(S^T) is the
    lhsT of the context matmul; softmax denominators come from a ones
    column appended to V
  - LayerNorm rstd via Quake-style bit-trick rsqrt + 2 Newton steps on
    the DVE: keeps Sqrt off the ACT engine so the only ACT table loads
    are the per-iteration Exp<->Gelu switch
  - conv head runs feature-major ([filters, windows]); global maxpool is
    a per-partition free-axis reduce_max; ragged window validity is a
    -1e30 penalty added before the max; short sequences use 129-wide
    windows instead of 257

Engine-queue discipline (from trace analysis of the v0 kernel):
  - sync queue: ONLY DMA transposes (+ final output store)
  - scalar queue (ACT HWDGE): prologue constants + layer-0 weights
  - gpsimd queue (SWDGE): embedding gathers, layer 1..3 / conv / fc
    weight prefetch, conv masks
  - per-iteration emission order keeps the PE queue dependency-clean:
    ctx -> [next-pair QKV backfill] -> WO -> [next V] -> [next scores g0]
    -> LN1 -> FFN1 -> FFN2 (ft-outer, chases the Gelu chain) ->
    [next scores g1] -> LN2 -> pair transpose -> conv (last layer)
"""

import numpy as np
import ml_dtypes

V, D, H, DH, NL, FF = 30522, 512, 8, 64, 4, 2048
NF, NCLS, B, S = 256, 4, 32, 256
NCORES, SPC = 8, 4
NSEQ = 3 * SPC          # 12 sequences per core
NPAIR = NSEQ // 2       # 6 sequence pairs
ND = D // 128            # 4 feature tiles
NFT = FF // 128          # 16 FFN hidden tiles
NCH = 6                  # per-branch fc chunks of 128 (3 kernels x 2 ftiles)

BF = ml_dtypes.bfloat16
F8 = ml_dtypes.float8_e4m3
_CACHE = {}
FP8_FFN = True           # fp8e4 DoubleRow matmuls for FFN1/FFN2
FFN_XS, FFN_WS = 8.0, 16.0   # fp8 quantization scales (powers of 2)
FP8_QKV = False          # fp8e4 DoubleRow for the QKV/WO projections too
#   (tried: saves ~90us PE but congests DVE/ACT and costs 0.4e-2 rel err)
#   (scores / softmax / ctx stay bf16; only the projections quantize)
QKV_XS, CTX_S = 16.0, 32.0   # x / ctx activation scales for fp8
QUAKE_LN = True          # DVE-only rsqrt (bit trick + Newton steps)
QUAKE_ITERS = 1          # Newton steps (1 -> 1.8e-3 rstd rel err, plenty here)
BCAST_NORM = True        # one stride-0-broadcast DVE op per ctx 4-head group
BF16_R = True            # residual/LN scratch tiles in bf16 (2x DVE modes)


def _flags(inputs):
    z = lambda a: bool(np.all(np.asarray(a) == 0))
    o = lambda a: bool(np.all(np.asarray(a) == 1))
    return {
        "bqk": z(inputs["bq"]) and z(inputs["bk"]),
        "bv": z(inputs["bv"]),
        "bo": z(inputs["bo"]),
        "bi": z(inputs["bi"]),
        "bo2": z(inputs["bo2"]),
        "ln": all(o(inputs[k]) for k in ("emb_ln_s", "ln1_s", "ln2_s"))
        and all(z(inputs[k]) for k in ("emb_ln_b", "ln1_b", "ln2_b")),
        "cb": z(inputs["conv_b1"]) and z(inputs["conv_b2"]) and z(inputs["conv_b3"]),
    }


def _pair_nts(ns):
    """Pair tile-counts for a core with ns short seqs: one short pair
    first (fast PE start), then alternate so every short-pair iteration
    gets a long next-pair to backfill its LN windows; a short pair last
    (small final conv)."""
    nps = ns // 2
    npl = NPAIR - nps
    pn = []
    s_left, l_left = nps, npl
    want_short = True
    while s_left + l_left > 0:
        if want_short and s_left > 0:
            pn.append(1); s_left -= 1
        elif l_left > 0:
            pn.append(2); l_left -= 1
        else:
            pn.append(1); s_left -= 1
        # keep one short for the tail if possible
        want_short = not want_short if s_left > 1 or l_left == 0 else False
        if s_left == 1 and l_left == 0:
            want_short = True
    return tuple(pn)


def _build_program(fl, pn):
    import contextlib
    import concourse.bass as bass
    import concourse.mybir as mybir
    import concourse.tile as tile
    from concourse import bacc
    from concourse.masks import make_identity

    F32, BF16, I32 = mybir.dt.float32, mybir.dt.bfloat16, mybir.dt.int32
    U32 = mybir.dt.uint32
    AL, AF = mybir.AluOpType, mybir.ActivationFunctionType

    seq_nt = []
    for p in range(NPAIR):
        seq_nt += [pn[p], pn[p]]
    tbase = np.concatenate([[0], np.cumsum(seq_nt)]).astype(int)
    NT_TOT = int(tbase[-1])

    nc = bacc.Bacc("TRN2", target_bir_lowering=False, debug=False,
                   num_devices=NCORES)

    di = lambda n, s, d: nc.dram_tensor(n, s, d, kind="ExternalInput").ap()
    F8D = mybir.dt.float8e4
    PDT = F8D if FP8_QKV else BF16
    word = di("word_emb", [V, D], F32)
    ids_d = di("ids", [128, NT_TOT], I32)
    mb_d = di("maskbias", [128, NT_TOT], F32)
    posty_d = di("posty", [2, 128, D], F32)
    cmask_d = di("convmask", [NSEQ, S + 1], BF16)
    cpen_d = di("convpen", [NSEQ, 3, S + 1], F32)
    wq_d = [di(f"wq{l}", [128, ND, D], PDT) for l in range(NL)]
    wk_d = [di(f"wk{l}", [128, ND, D], PDT) for l in range(NL)]
    wv_d = [di(f"wv{l}", [128, ND, D], PDT) for l in range(NL)]
    wo_d = [di(f"wo{l}", [128, ND, D], PDT) for l in range(NL)]
    WDT = F8D if FP8_FFN else BF16
    wi_d = [di(f"wi{l}", [128, ND, FF], WDT) for l in range(NL)]
    wo2_d = [di(f"wo2{l}", [128, NFT, D], WDT) for l in range(NL)]
    cw_d = [di(f"cw{k}", [k, 128, ND, NF], BF16) for k in (1, 2, 3)]
    fcw_d = di("fcw", [128, 3, NCH, NCLS], F32)
    if not fl["bqk"]:
        bq_d = [di(f"bq{l}", [ND, 128], F32) for l in range(NL)]
        bk_d = [di(f"bk{l}", [ND, 128], F32) for l in range(NL)]
    if not fl["bv"]:
        bv_d = [di(f"bv{l}", [D], F32) for l in range(NL)]
    if not fl["bo"]:
        bo_d = [di(f"bo{l}", [D], F32) for l in range(NL)]
    if not fl["bi"]:
        bi_d = [di(f"bi{l}", [NFT, 128], F32) for l in range(NL)]
    if not fl["bo2"]:
        bo2_d = [di(f"bo2{l}", [D], F32) for l in range(NL)]
    if not fl["ln"]:
        elns_d = di("lnes", [D], F32)
        elnb_d = di("lneb", [D], F32)
        ln1s_d = [di(f"ln1s{l}", [D], F32) for l in range(NL)]
        ln1b_d = [di(f"ln1b{l}", [D], F32) for l in range(NL)]
        ln2s_d = [di(f"ln2s{l}", [D], F32) for l in range(NL)]
        ln2b_d = [di(f"ln2b{l}", [D], F32) for l in range(NL)]
    if not fl["cb"]:
        cb_d = di("convb", [3, 2, 128], F32)

    out_d = nc.dram_tensor("out", [NCLS, 3, NSEQ], F32,
                           kind="ExternalOutput").ap()

    with tile.TileContext(nc) as tc, contextlib.ExitStack() as ctx:
        consts = ctx.enter_context(tc.tile_pool(name="consts", bufs=1))
        state = ctx.enter_context(tc.tile_pool(name="state", bufs=1))
        wts = ctx.enter_context(tc.tile_pool(name="wts", bufs=1))
        big = ctx.enter_context(tc.tile_pool(name="big", bufs=1))
        work = ctx.enter_context(tc.tile_pool(name="work", bufs=2))
        small = ctx.enter_context(tc.tile_pool(name="small", bufs=4))
        ps_mm = ctx.enter_context(tc.tile_pool(name="ps_mm", bufs=6, space="PSUM"))
        ps_ctx = ctx.enter_context(tc.tile_pool(name="ps_ctx", bufs=2, space="PSUM"))

        # ---- prologue constants: scalar (ACT) HWDGE queue, critical first ----
        ids_sb = consts.tile([128, NT_TOT], I32, tag="ids")
        nc.scalar.dma_start(out=ids_sb[:], in_=ids_d)
        posty = consts.tile([128, 2, D], F32, tag="posty")
        nc.scalar.dma_start(out=posty[:], in_=posty_d.rearrange("t p d -> p t d"))
        mb_sb = consts.tile([128, NT_TOT], F32, tag="mb")
        nc.scalar.dma_start(out=mb_sb[:], in_=mb_d)
        ident = consts.tile([128, 128], BF16, tag="ident")
        magic = consts.tile([128, 8], I32, tag="magic")
        nc.vector.memset(magic[:], 0x5F3759DF)
        if not QUAKE_LN:
            eps_t = consts.tile([128, 1], F32, tag="eps")
            nc.vector.memset(eps_t[:], 1e-12)

        bcast = lambda ap, n: ap[None, :].to_broadcast([128, n])
        if not fl["ln"]:
            elns = consts.tile([128, D], F32, tag="elns")
            nc.scalar.dma_start(out=elns[:], in_=bcast(elns_d, D))
            elnb = consts.tile([128, D], F32, tag="elnb")
            nc.scalar.dma_start(out=elnb[:], in_=bcast(elnb_d, D))

        # persistent per-pair state: token-major residual + feature-major copy
        RDT = BF16 if BF16_R else F32
        XDT = F8D if FP8_QKV else BF16
        x_p = [state.tile([128, 2, pn[q], D], BF16, tag=f"xp{q}",
                          name=f"xp{q}") for q in range(NPAIR)]
        if FP8_QKV:
            xT = [state.tile([128, ND, 2 * pn[q] * 128], F8D, tag=f"xT{q}",
                             name=f"xT{q}") for q in range(NPAIR)]
        else:
            xT = [state.tile([128, 2, pn[q], ND, 128], BF16, tag=f"xT{q}",
                             name=f"xT{q}") for q in range(NPAIR)]
        rep = state.tile([128, NCH, NSEQ], F32, tag="rep")

        def ln_stats_batch(rs):
            """Pipelined LN stats for a list of [128, D] sources; returns
            (rcp, nmb) where rcp[:, i] = rstd_i, nmb[:, i] = -mean_i*rstd_i."""
            n = len(rs)
            mvt = small.tile([128, n, 2], F32, tag="mvt")
            for i, r in enumerate(rs):
                st = small.tile([128, 6], F32, tag="st", name=f"st{i}")
                nc.vector.bn_stats(out=st[:], in_=r)
                nc.vector.bn_aggr(out=mvt[:, i, :], in_=st[:])
            rcp = small.tile([128, n], F32, tag="rcp2")
            if QUAKE_LN:
                # rstd = rsqrt(var + eps): Quake bit-trick + 2 Newton steps,
                # all on the DVE (no ACT Sqrt -> no act-table thrash)
                vv = small.tile([128, n], F32, tag="vv")
                nc.vector.tensor_scalar_add(out=vv[:], in0=mvt[:, :, 1],
                                            scalar1=1e-12)
                nc.vector.tensor_scalar(
                    out=rcp[:].bitcast(I32), in0=vv[:].bitcast(I32),
                    scalar1=1, scalar2=0, op0=AL.logical_shift_right,
                    op1=AL.bypass)
                nc.vector.tensor_tensor(
                    out=rcp[:].bitcast(I32), in0=magic[:, 0:n],
                    in1=rcp[:].bitcast(I32), op=AL.subtract)
                t = small.tile([128, n], F32, tag="qt")
                for _ in range(QUAKE_ITERS):
                    nc.vector.tensor_tensor(out=t[:], in0=rcp[:], in1=rcp[:],
                                            op=AL.mult)
                    nc.vector.tensor_tensor(out=t[:], in0=t[:], in1=vv[:],
                                            op=AL.mult)
                    nc.vector.tensor_scalar(out=t[:], in0=t[:], scalar1=-0.5,
                                            scalar2=1.5, op0=AL.mult,
                                            op1=AL.add)
                    nc.vector.tensor_tensor(out=rcp[:], in0=rcp[:], in1=t[:],
                                            op=AL.mult)
            else:
                sd = small.tile([128, n], F32, tag="sd")
                nc.scalar.activation(out=sd[:], in_=mvt[:, :, 1],
                                     func=AF.Sqrt, bias=eps_t[:], scale=1.0)
                nc.vector.reciprocal(out=rcp[:], in_=sd[:])
            nmb = small.tile([128, n], F32, tag="nmb")
            # nmb = (mean * -1) * rstd
            nc.vector.scalar_tensor_tensor(
                out=nmb[:], in0=mvt[:, :, 0], scalar=-1.0, in1=rcp[:],
                op0=AL.mult, op1=AL.mult)
            return rcp, nmb

        def ln_apply_batch(rs, dsts, rcp, nmb, s_tile, b_tile):
            for i in range(len(rs)):
                if False and s_tile is None and b_tile is None and i % 2 == 1:
                    # odd tiles apply on ACT (Copy: in every table, no load)
                    # so the two engines drain the batch in parallel
                    nc.scalar.activation(
                        out=dsts[i], in_=rs[i], func=AF.Identity,
                        bias=nmb[:, i:i + 1], scale=rcp[:, i:i + 1])
                    continue
                nc.vector.tensor_scalar(
                    out=dsts[i], in0=rs[i], scalar1=rcp[:, i:i + 1],
                    scalar2=nmb[:, i:i + 1], op0=AL.mult, op1=AL.add)
                if s_tile is not None:
                    nc.vector.tensor_tensor(out=dsts[i], in0=dsts[i],
                                            in1=s_tile[:], op=AL.mult)
                if b_tile is not None:
                    nc.vector.tensor_tensor(out=dsts[i], in0=dsts[i],
                                            in1=b_tile[:], op=AL.add)

        def feat_major(pr, dst, scale):
            """PE-transpose x_p[pr] into a feature-major copy dst with a
            fused scale+cast drain. Transposes go tile-outer so they chase
            the LN applies tile-by-tile instead of waiting for the batch."""
            nt = pn[pr]
            sw = 2 * nt * 128
            for dt in range(ND):
                tps = ps_mm.tile([128, sw], BF16, tag="mm",
                                 name=f"fm{pr}_{dt}")
                for i in range(2 * nt):
                    si, tt = i // nt, i % nt
                    nc.tensor.transpose(
                        tps[:, i * 128:(i + 1) * 128],
                        x_p[pr][:, si, tt, dt * 128:(dt + 1) * 128],
                        ident[:])
                if scale == 1.0:
                    nc.vector.tensor_copy(out=dst[:, dt, :], in_=tps[:])
                else:
                    nc.vector.tensor_scalar_mul(out=dst[:, dt, :],
                                                in0=tps[:], scalar1=scale)

        def to_feat(pr, eng=None):
            """Refresh the feature-major x copy after an LN2 update."""
            if FP8_QKV:
                feat_major(pr, xT[pr], QKV_XS)
            else:
                (eng or nc.sync).dma_start_transpose(xT[pr][:, :, :, :, :],
                                                     x_p[pr][:, :, :, :])

        def embed_pair(p, pt_eng=None):
            nt = pn[p]
            t0 = int(tbase[2 * p])
            gb = work.tile([128, 2 * nt, D], F32, tag="r", name=f"g{p}",
                           bufs=2)
            tiles = []
            for si in range(2):
                for tt in range(nt):
                    j = si * nt + tt
                    nc.gpsimd.indirect_dma_start(
                        out=gb[:, j, :], out_offset=None, in_=word[:],
                        in_offset=bass.IndirectOffsetOnAxis(
                            ap=ids_sb[:, t0 + j:t0 + j + 1], axis=0))
                    nc.vector.tensor_tensor(out=gb[:, j, :], in0=gb[:, j, :],
                                            in1=posty[:, tt, :], op=AL.add)
                    tiles.append((si, tt))
            rcp, nmb = ln_stats_batch([gb[:, si * nt + tt, :]
                                       for si, tt in tiles])
            ln_apply_batch([gb[:, si * nt + tt, :] for si, tt in tiles],
                           [x_p[p][:, si, tt, :] for si, tt in tiles],
                           rcp, nmb,
                           None if fl["ln"] else elns,
                           None if fl["ln"] else elnb)
            to_feat(p, pt_eng)

        def load_layer_weights_A(l, q):
            """QKV weights (+ small per-layer consts) for layer l."""
            w = {}
            for nm, dd in (("wq", wq_d), ("wk", wk_d), ("wv", wv_d)):
                w[nm] = wts.tile([128, ND, D], PDT, tag=nm, name=f"{nm}_{l}")
                for dt in range(0, ND, 2):
                    q.dma_start(out=w[nm][:, dt:dt + 2, :],
                                in_=dd[l][:, dt:dt + 2, :])
            if not fl["bqk"]:
                w["bq"] = consts.tile([128, ND], F32, tag="bq", name=f"bq_{l}")
                q.dma_start(out=w["bq"][:], in_=bq_d[l].rearrange("t p -> p t"))
                w["bk"] = consts.tile([128, ND], F32, tag="bk", name=f"bk_{l}")
                q.dma_start(out=w["bk"][:], in_=bk_d[l].rearrange("t p -> p t"))
            if not fl["bv"]:
                w["bv"] = consts.tile([128, D], F32, tag="bv", name=f"bv_{l}")
                q.dma_start(out=w["bv"][:], in_=bcast(bv_d[l], D))
            if not fl["ln"]:
                for nm, dd in (("ln1s", ln1s_d), ("ln1b", ln1b_d),
                               ("ln2s", ln2s_d), ("ln2b", ln2b_d)):
                    w[nm] = consts.tile([128, D], F32, tag=nm, name=f"{nm}_{l}")
                    q.dma_start(out=w[nm][:], in_=bcast(dd[l], D))
            return w

        def load_layer_weights_B(l, w, q):
            """WO / FFN weights for layer l (emit after last layer-(l-1) use)."""
            w["wo"] = wts.tile([128, ND, D], PDT, tag="wo", name=f"wo_{l}")
            for dt in range(0, ND, 2):
                q.dma_start(out=w["wo"][:, dt:dt + 2, :],
                            in_=wo_d[l][:, dt:dt + 2, :])
            w["wi"] = wts.tile([128, ND, FF], WDT, tag="wi", name=f"wi_{l}")
            for dt in range(0, ND, 2):
                q.dma_start(out=w["wi"][:, dt:dt + 2, :],
                            in_=wi_d[l][:, dt:dt + 2, :])
            w["wo2"] = wts.tile([128, NFT, D], WDT, tag="wo2", name=f"wo2_{l}")
            for ft in range(0, NFT, 8):
                q.dma_start(out=w["wo2"][:, ft:ft + 8, :],
                            in_=wo2_d[l][:, ft:ft + 8, :])
            if not fl["bo"]:
                w["bo"] = consts.tile([128, D], F32, tag="bo", name=f"bo_{l}")
                q.dma_start(out=w["bo"][:], in_=bcast(bo_d[l], D))
            if not fl["bi"]:
                w["bi"] = consts.tile([128, NFT], F32, tag="bi", name=f"bi_{l}")
                q.dma_start(out=w["bi"][:], in_=bi_d[l].rearrange("t p -> p t"))
            if not fl["bo2"]:
                w["bo2"] = consts.tile([128, D], F32, tag="bo2", name=f"bo2_{l}")
                q.dma_start(out=w["bo2"][:], in_=bcast(bo2_d[l], D))
            return w

        # fp8 dequant scales for the projection drains (wq carries no /8 on
        # the host in fp8 mode; the score scaling folds in here instead)
        P_SC = 1.0 / (QKV_XS * FFN_WS) if FP8_QKV else 1.0
        Q_SC = P_SC * 0.125 if FP8_QKV else 1.0
        O_SC = 1.0 / (CTX_S * FFN_WS) if FP8_QKV else 1.0

        def proj_mm(ps, xts, wt, ocols):
            """One QKV-projection matmul chain (DR fp8 or bf16)."""
            if FP8_QKV:
                for dp in range(2):
                    nc.tensor.matmul(
                        ps[:], wt[:, 2 * dp:2 * dp + 2, ocols],
                        xts[:, 2 * dp:2 * dp + 2, :],
                        start=dp == 0, stop=dp == 1,
                        perf_mode=mybir.MatmulPerfMode.DoubleRow)
            else:
                for dt in range(ND):
                    nc.tensor.matmul(
                        ps[:], wt[:, dt, ocols], xts[:, :, :, dt, :],
                        start=dt == 0, stop=dt == ND - 1)

        def qkv_pair(pr, w):
            """qT/kT feature-major bf16 for both seqs."""
            nt = pn[pr]
            sw = 2 * nt * 128
            xts = xT[pr]
            qT = work.tile([128, ND, sw], BF16, tag="qT", name=f"qT{pr}")
            kT = work.tile([128, ND, sw], BF16, tag="kT", name=f"kT{pr}")
            for dst_t, wt, which in ((qT, w["wq"], "q"), (kT, w["wk"], "k")):
                sc = Q_SC if which == "q" else P_SC
                for ot in range(ND):
                    ps = ps_mm.tile([128, sw], F32, tag="mm")
                    proj_mm(ps, xts, wt, slice(ot * 128, (ot + 1) * 128))
                    if fl["bqk"]:
                        # split PSUM drain across ACT and DVE so neither
                        # engine's queue gates PSUM recycling
                        if which == "q":
                            nc.scalar.mul(out=dst_t[:, ot, :], in_=ps[:],
                                          mul=sc)
                        elif sc == 1.0:
                            nc.vector.tensor_copy(out=dst_t[:, ot, :],
                                                  in_=ps[:])
                        else:
                            nc.vector.tensor_scalar_mul(
                                out=dst_t[:, ot, :], in0=ps[:], scalar1=sc)
                    else:
                        bt = w["bq"] if which == "q" else w["bk"]
                        nc.vector.tensor_scalar(
                            out=dst_t[:, ot, :], in0=ps[:], scalar1=sc,
                            scalar2=bt[:, ot:ot + 1], op0=AL.mult, op1=AL.add)
            return qT, kT

        def v_pair(pr, w):
            """V token-major bf16 with a ones column per head, both seqs."""
            nt = pn[pr]
            xts = xT[pr]
            vAs = []
            for si in range(2):
                vA = work.tile([128, nt, H, DH + 1], BF16, tag="vA",
                               name=f"vA{pr}_{si}")
                nc.vector.memset(vA[:, :, :, DH:DH + 1], 1.0)
                for tt in range(nt):
                    tok = (si * nt + tt) * 128
                    ps = ps_mm.tile([128, D], F32, tag="mm")
                    if FP8_QKV:
                        for dp in range(2):
                            nc.tensor.matmul(
                                ps[:], xts[:, 2 * dp:2 * dp + 2, tok:tok + 128],
                                w["wv"][:, 2 * dp:2 * dp + 2, :],
                                start=dp == 0, stop=dp == 1,
                                perf_mode=mybir.MatmulPerfMode.DoubleRow)
                    else:
                        for dt in range(ND):
                            nc.tensor.matmul(
                                ps[:], xts[:, si, tt, dt, :],
                                w["wv"][:, dt, :], start=dt == 0,
                                stop=dt == ND - 1)
                    if fl["bv"]:
                        if P_SC == 1.0:
                            nc.vector.tensor_copy(
                                out=vA[:, tt, :, 0:DH],
                                in_=ps.rearrange("p (h d) -> p h d", h=H))
                        else:
                            nc.vector.tensor_scalar_mul(
                                out=vA[:, tt, :, 0:DH],
                                in0=ps.rearrange("p (h d) -> p h d", h=H),
                                scalar1=P_SC)
                    else:
                        nc.vector.scalar_tensor_tensor(
                            out=vA[:, tt, :, 0:DH],
                            in0=ps.rearrange("p (h d) -> p h d", h=H),
                            scalar=P_SC,
                            in1=w["bv"].rearrange("p (h d) -> p h d", h=H),
                            op0=AL.mult, op1=AL.add)
                vAs.append(vA)
            return vAs

        att_state = {}

        def att_scores(pr, g_, qk):
            """S^T + exp for head-group g_ of both seqs of pair pr."""
            nt = pn[pr]
            sq = nt * 128       # queries per seq
            qT, kT = qk
            for si in range(2):
                seq = 2 * pr + si
                so = si * sq
                eT = work.tile([128, nt, 4, sq], BF16, tag="eT",
                               name=f"eT{pr}_{g_}_{si}", bufs=4)
                att_state[(pr, g_, si)] = eT
                for kt in range(nt):
                    ti = int(tbase[seq]) + kt
                    for hi in range(4):
                        h = g_ * 4 + hi
                        ot, hh = h // 2, (h % 2) * DH
                        ps = ps_mm.tile([128, sq], F32, tag="mm")
                        nc.tensor.matmul(
                            ps[:],
                            kT[hh:hh + DH, ot, so + kt * 128:so + (kt + 1) * 128],
                            qT[hh:hh + DH, ot, so:so + sq],
                            start=True, stop=True)
                        nc.scalar.activation(
                            out=eT[:, kt, hi, :], in_=ps[:],
                            func=AF.Exp, bias=mb_sb[:, ti:ti + 1], scale=1.0)

        def att_ctx(pr, g_, vAs, ctxb):
            """ctx (+denominator) matmuls and DVE normalization for group g_."""
            nt = pn[pr]
            for si in range(2):
                eT, vA = att_state.pop((pr, g_, si)), vAs[si]
                for qt in range(nt):
                    cps = ps_ctx.tile([128, 4 * (DH + 1)], F32, tag="ctx",
                                      name=f"ctx{si}_{qt}_{g_}")
                    for hi in range(4):
                        h = g_ * 4 + hi
                        sl = slice(hi * (DH + 1), (hi + 1) * (DH + 1))
                        for kt in range(nt):
                            nc.tensor.matmul(
                                cps[:, sl],
                                eT[:, kt, hi, qt * 128:(qt + 1) * 128],
                                vA[:, kt, h, :], start=kt == 0,
                                stop=kt == nt - 1)
                    cph = cps.rearrange("p (h c) -> p h c", c=DH + 1)
                    if BCAST_NORM:
                        rcp = small.tile([128, 4, 1], F32, tag="rcp")
                        nc.vector.reciprocal(out=rcp[:],
                                             in_=cph[:, :, DH:DH + 1])
                        nc.vector.scalar_tensor_tensor(
                            out=ctxb[:, si, qt,
                                     g_ * 4 * DH:(g_ + 1) * 4 * DH].rearrange(
                                         "p (h d) -> p h d", d=DH),
                            in0=cph[:, :, 0:DH], scalar=1.0,
                            in1=rcp[:, :, 0:1].broadcast_to([128, 4, DH]),
                            op0=AL.mult, op1=AL.mult)
                    else:
                        rcp = small.tile([128, 4], F32, tag="rcp")
                        nc.vector.reciprocal(out=rcp[:], in_=cph[:, :, DH])
                        for hi in range(4):
                            h = g_ * 4 + hi
                            base = hi * (DH + 1)
                            nc.vector.tensor_scalar_mul(
                                out=ctxb[:, si, qt, h * DH:(h + 1) * DH],
                                in0=cps[:, base:base + DH],
                                scalar1=rcp[:, hi:hi + 1])

        def residual_ln(rs, dsts, s_tile, b_tile):
            rcp, nmb = ln_stats_batch(rs)
            ln_apply_batch(rs, dsts, rcp, nmb, s_tile, b_tile)

        def ctx_half(pr, g_, vAs, ctxb, ct):
            """ctx matmuls + norm for head-group g_, then PE-transpose that
            group's two feature tiles into ct with ACT copy drains (copy is
            in every act table -> no table load, ~1us latency to WO)."""
            nt = pn[pr]
            sw = 2 * nt * 128
            att_ctx(pr, g_, vAs, ctxb)
            for dt in (2 * g_, 2 * g_ + 1):
                tpx = ps_mm.tile([128, sw], BF16, tag="mm",
                                 name=f"ctp{pr}_{dt}")
                for i in range(2 * nt):
                    si, tt = i // nt, i % nt
                    nc.tensor.transpose(
                        tpx[:, i * 128:(i + 1) * 128],
                        ctxb[:, si, tt, dt * 128:(dt + 1) * 128],
                        ident[:])
                if FP8_QKV:
                    nc.scalar.mul(out=ct[:, dt, :], in_=tpx[:], mul=CTX_S)
                else:
                    nc.scalar.copy(out=ct[:, dt, :], in_=tpx[:])

        def ctx_start(pr, vAs):
            """Allocate this pair's ctx tiles and run head-group 0."""
            nt = pn[pr]
            ctxb = work.tile([128, 2, nt, D], BF16, tag="ctxb",
                             name=f"cb{pr}")
            ct = work.tile([128, ND, 2 * nt * 128], XDT, tag="cT",
                           name=f"cT{pr}")
            ctx_half(pr, 0, vAs, ctxb, ct)
            return ctxb, ct

        cw = {}

        def load_conv_weights(q):
            ti = 0
            for ki, k in enumerate((1, 2, 3)):
                for j in range(k):
                    t = wts.tile([128, ND, NF], BF16, tag=f"cw{ti}",
                                 name=f"cwt{k}_{j}")
                    q.dma_start(out=t[:], in_=cw_d[ki][j])
                    cw[(k, j)] = t
                    ti += 1
            fcw = consts.tile([128, 3, NCH, NCLS], F32, tag="fcw")
            q.dma_start(out=fcw[:], in_=fcw_d)
            cb = None
            if not fl["cb"]:
                cb = consts.tile([128, 3, 2], F32, tag="cb")
                q.dma_start(out=cb[:], in_=cb_d.rearrange("k t p -> p k t"))
            return fcw, cb

        def conv_prefetch(pr):
            """Issue the conv mask / window-penalty DMAs for pair pr early."""
            nt = pn[pr]
            swc = nt * 128 + 1
            cms, pens = [], []
            for si in range(2):
                seq = 2 * pr + si
                cm = work.tile([128, swc], BF16, tag="cm", name=f"cm{seq}")
                nc.gpsimd.dma_start(
                    out=cm[:],
                    in_=cmask_d[seq, 0:swc][None, :].to_broadcast([128, swc]))
                cms.append(cm)
                ps_ = []
                for ki in range(3):
                    pen = work.tile([128, swc], F32, tag="pen",
                                    name=f"pen{seq}_{ki}", bufs=6)
                    nc.gpsimd.dma_start(
                        out=pen[:],
                        in_=cpen_d[seq, ki, 0:swc][None, :].to_broadcast(
                            [128, swc]))
                    ps_.append(pen)
                pens.append(ps_)
            return cms, pens

        def conv_pair(pr, cms, pens):
            """Conv head for both seqs of pair pr straight off x_p: PE
            transposes feature tiles into PSUM, the drain fuses the
            token-mask multiply (no DMA transpose on the conv path)."""
            nt = pn[pr]
            swc = nt * 128 + 1
            for si in range(2):
                seq = 2 * pr + si
                xcv = work.tile([128, ND, swc], BF16, tag="xcv",
                                name=f"xcv{seq}")
                nc.vector.memset(xcv[:, :, nt * 128:nt * 128 + 1], 0.0)
                for dt in range(ND):
                    tpc = ps_mm.tile([128, nt * 128], BF16, tag="mm",
                                     name=f"cvtp{seq}_{dt}")
                    for tt in range(nt):
                        nc.tensor.transpose(
                            tpc[:, tt * 128:(tt + 1) * 128],
                            x_p[pr][:, si, tt, dt * 128:(dt + 1) * 128],
                            ident[:])
                    nc.vector.tensor_tensor(out=xcv[:, dt, 0:nt * 128],
                                            in0=tpc[:],
                                            in1=cms[si][:, 0:nt * 128],
                                            op=AL.mult)
                for ki, k in enumerate((1, 2, 3)):
                    nw = swc - k + 1
                    for ft in range(2):
                        ps = ps_mm.tile([128, swc], F32, tag="mm")
                        idx = 0
                        for dt in range(ND):
                            for j in range(k):
                                nc.tensor.matmul(
                                    ps[:, 0:nw],
                                    cw[(k, j)][:, dt, ft * 128:(ft + 1) * 128],
                                    xcv[:, dt, j:j + nw],
                                    start=idx == 0, stop=idx == ND * k - 1)
                                idx += 1
                        cvt = work.tile([128, swc], F32, tag="cvt",
                                        name=f"cv{seq}_{k}_{ft}")
                        nc.vector.tensor_tensor(out=cvt[:, 0:nw],
                                                in0=ps[:, 0:nw],
                                                in1=pens[si][ki][:, 0:nw],
                                                op=AL.add)
                        nc.vector.tensor_reduce(
                            out=rep[:, ki * 2 + ft, seq:seq + 1],
                            in_=cvt[:, 0:nw],
                            axis=mybir.AxisListType.X, op=AL.max)

        # ---- main schedule ----
        # prologue: embed gathers lead the DGE, layer-0 weights trail them
        # on the gpsimd queue in need order (wq/wk -> wv -> wo/wi/wo2)
        make_identity(nc, ident[:])
        # p-state warmup: keep the PE streaming while the embed/weight
        # chain runs so the first real matmuls start at full clock
        warm = consts.tile([128, 512], BF16, tag="warm")
        nc.vector.memset(warm[:], 0.5)
        wps = ps_mm.tile([128, 512], F32, tag="mm", name="warm")
        for _ in range(30):
            nc.tensor.matmul(wps[:], ident[:], warm[:], start=True, stop=True)
        embed_pair(0, pt_eng=nc.scalar)
        w_cur = {}
        for nm, dd in (("wq", wq_d), ("wk", wk_d)):
            w_cur[nm] = wts.tile([128, ND, D], PDT, tag=nm, name=f"{nm}_0")
            for dt in range(0, ND, 2):
                nc.gpsimd.dma_start(out=w_cur[nm][:, dt:dt + 2, :],
                                    in_=dd[0][:, dt:dt + 2, :])
        embed_pair(1, pt_eng=nc.scalar)
        w_cur["wv"] = wts.tile([128, ND, D], PDT, tag="wv", name="wv_0")
        for dt in range(0, ND, 2):
            nc.gpsimd.dma_start(out=w_cur["wv"][:, dt:dt + 2, :],
                                in_=wv_d[0][:, dt:dt + 2, :])
        if not fl["bqk"]:
            w_cur["bq"] = consts.tile([128, ND], F32, tag="bq", name="bq_0")
            nc.gpsimd.dma_start(out=w_cur["bq"][:],
                                in_=bq_d[0].rearrange("t p -> p t"))
            w_cur["bk"] = consts.tile([128, ND], F32, tag="bk", name="bk_0")
            nc.gpsimd.dma_start(out=w_cur["bk"][:],
                                in_=bk_d[0].rearrange("t p -> p t"))
        if not fl["bv"]:
            w_cur["bv"] = consts.tile([128, D], F32, tag="bv", name="bv_0")
            nc.gpsimd.dma_start(out=w_cur["bv"][:], in_=bcast(bv_d[0], D))
        if not fl["ln"]:
            for nm, dd in (("ln1s", ln1s_d), ("ln1b", ln1b_d),
                           ("ln2s", ln2s_d), ("ln2b", ln2b_d)):
                w_cur[nm] = consts.tile([128, D], F32, tag=nm, name=f"{nm}_0")
                nc.gpsimd.dma_start(out=w_cur[nm][:], in_=bcast(dd[0], D))
        w_cur = load_layer_weights_B(0, w_cur, nc.gpsimd)
        qk_cur = qkv_pair(0, w_cur)
        v_cur = v_pair(0, w_cur)
        att_scores(0, 0, qk_cur)
        att_scores(0, 1, qk_cur)
        fcw = cb = None
        for l in range(NL):
            for pr in range(NPAIR):
                nt = pn[pr]
                w = w_cur
                if pr + 1 < NPAIR:
                    nxt_l, nxt_pr = l, pr + 1
                elif l + 1 < NL:
                    nxt_l, nxt_pr = l + 1, 0
                else:
                    nxt_l = nxt_pr = None
                cross = nxt_pr is not None and nxt_l != l

                if cross:
                    # QKV weights of the next layer: all layer-l readers of
                    # wq/wk/wv were emitted by the previous iteration
                    w_nxt = load_layer_weights_A(nxt_l, nc.gpsimd)
                elif nxt_pr is not None:
                    w_nxt = w
                if l == NL - 1:
                    cms, pens = conv_prefetch(pr)

                sw = 2 * nt * 128
                ctxb, ct = ctx_start(pr, v_cur)
                ctx_half(pr, 1, v_cur, ctxb, ct)
                # PE backfill: the next pair's QKV projections
                if nxt_pr is not None:
                    qk_nxt = qkv_pair(nxt_pr, w_nxt)
                # attention out projection + residual
                rs = []
                for i in range(2 * nt):
                    si, tt = i // nt, i % nt
                    ps = ps_mm.tile([128, D], F32, tag="mm")
                    if FP8_QKV:
                        for dp in range(2):
                            nc.tensor.matmul(
                                ps[:], ct[:, 2 * dp:2 * dp + 2,
                                           i * 128:(i + 1) * 128],
                                w["wo"][:, 2 * dp:2 * dp + 2, :],
                                start=dp == 0, stop=dp == 1,
                                perf_mode=mybir.MatmulPerfMode.DoubleRow)
                    else:
                        for dt in range(ND):
                            nc.tensor.matmul(
                                ps[:], ct[:, dt, i * 128:(i + 1) * 128],
                                w["wo"][:, dt, :], start=dt == 0,
                                stop=dt == ND - 1)
                    r = work.tile([128, D], RDT, tag="rln", name=f"r{i}", bufs=4)
                    if O_SC == 1.0:
                        nc.vector.tensor_tensor(out=r[:], in0=ps[:],
                                                in1=x_p[pr][:, si, tt, :],
                                                op=AL.add)
                    else:
                        nc.vector.scalar_tensor_tensor(
                            out=r[:], in0=ps[:], scalar=O_SC,
                            in1=x_p[pr][:, si, tt, :], op0=AL.mult, op1=AL.add)
                    if not fl["bo"]:
                        nc.vector.tensor_tensor(out=r[:], in0=r[:],
                                                in1=w["bo"][:], op=AL.add)
                    rs.append(r[:])
                # more PE backfill: next pair's V and both score groups run
                # while the LN1 chain (pure DVE now) drains
                if nxt_pr is not None:
                    v_nxt = v_pair(nxt_pr, w_nxt)
                    att_scores(nxt_pr, 0, qk_nxt)
                residual_ln(rs, [x_p[pr][:, i // nt, i % nt, :]
                                 for i in range(2 * nt)],
                            None if fl["ln"] else w["ln1s"],
                            None if fl["ln"] else w["ln1b"])
                # PE-transpose the LN1 output straight into PSUM (bf16),
                # then one drain per dt does the fp8 cast + scale
                y1 = work.tile([128, ND, sw], WDT, tag="y1f8",
                               name=f"y1f8{pr}")
                feat_major(pr, y1, FFN_XS if FP8_FFN else 1.0)
                # FFN1: hidden feature-major, gelu fused with bias
                hT = big.tile([128, NFT, sw], F8D if FP8_FFN else BF16,
                              tag="hT")
                for ft in range(NFT):
                    ps = ps_mm.tile([128, sw], F32, tag="mm")
                    if FP8_FFN:
                        for dp in range(2):
                            nc.tensor.matmul(
                                ps[:],
                                w["wi"][:, 2 * dp:2 * dp + 2,
                                        ft * 128:(ft + 1) * 128],
                                y1[:, 2 * dp:2 * dp + 2, :],
                                start=dp == 0, stop=dp == 1,
                                perf_mode=mybir.MatmulPerfMode.DoubleRow)
                    else:
                        for dt in range(ND):
                            nc.tensor.matmul(
                                ps[:], w["wi"][:, dt, ft * 128:(ft + 1) * 128],
                                y1[:, dt, :], start=dt == 0,
                                stop=dt == ND - 1)
                    nc.scalar.activation(
                        out=hT[:, ft, :], in_=ps[:], func=AF.Gelu,
                        bias=0.0 if fl["bi"] else w["bi"][:, ft:ft + 1],
                        scale=1.0 / (FFN_XS * FFN_WS) if FP8_FFN else 1.0)
                # embeds of the remaining pairs ride the FFN window (their
                # DVE chain slots between the y1f8 and FFN2 drains)
                if l == 0 and pr < 4:
                    embed_pair(pr + 2)
                # FFN2 + residual: ft-outer with per-token-tile PSUM so the
                # first matmuls chase the Gelu chain instead of waiting on it
                pss = [ps_mm.tile([128, D], F32, tag="mm",
                                  name=f"f2_{l}_{pr}_{i}")
                       for i in range(2 * nt)]
                if FP8_FFN:
                    for fp_ in range(NFT // 2):
                        for i in range(2 * nt):
                            si, tt = i // nt, i % nt
                            so = si * nt * 128
                            nc.tensor.matmul(
                                pss[i][:],
                                hT[:, 2 * fp_:2 * fp_ + 2,
                                   so + tt * 128:so + (tt + 1) * 128],
                                w["wo2"][:, 2 * fp_:2 * fp_ + 2, :],
                                start=fp_ == 0, stop=fp_ == NFT // 2 - 1,
                                perf_mode=mybir.MatmulPerfMode.DoubleRow)
                else:
                    for ft in range(NFT):
                        for i in range(2 * nt):
                            si, tt = i // nt, i % nt
                            so = si * nt * 128
                            nc.tensor.matmul(
                                pss[i][:],
                                hT[:, ft, so + tt * 128:so + (tt + 1) * 128],
                                w["wo2"][:, ft, :], start=ft == 0,
                                stop=ft == NFT - 1)
                rs = []
                for i in range(2 * nt):
                    si, tt = i // nt, i % nt
                    r = work.tile([128, D], RDT, tag="rln", name=f"r2{i}", bufs=4)
                    if FP8_FFN:
                        nc.vector.scalar_tensor_tensor(
                            out=r[:], in0=pss[i][:], scalar=1.0 / FFN_WS,
                            in1=x_p[pr][:, si, tt, :], op0=AL.mult, op1=AL.add)
                    else:
                        nc.vector.tensor_tensor(out=r[:], in0=pss[i][:],
                                                in1=x_p[pr][:, si, tt, :],
                                                op=AL.add)
                    if not fl["bo2"]:
                        nc.vector.tensor_tensor(out=r[:], in0=r[:],
                                                in1=w["bo2"][:], op=AL.add)
                    rs.append(r[:])
                if cross:
                    # WO/FFN weights of the next layer: all layer-l readers
                    # of wo/wi/wo2 are emitted above
                    w_nxt = load_layer_weights_B(nxt_l, w_nxt, nc.gpsimd)
                if l == 1 and pr == 0:
                    fcw, cb = load_conv_weights(nc.gpsimd)
                # PE backfill while the LN2 chain runs: second score group
                if nxt_pr is not None:
                    att_scores(nxt_pr, 1, qk_nxt)
                residual_ln(rs, [x_p[pr][:, i // nt, i % nt, :]
                                 for i in range(2 * nt)],
                            None if fl["ln"] else w["ln2s"],
                            None if fl["ln"] else w["ln2b"])
                if l < NL - 1:
                    to_feat(pr)
                if nxt_pr is not None:
                    qk_cur, v_cur, w_cur = qk_nxt, v_nxt, w_nxt
                if l == NL - 1:
                    conv_pair(pr, cms, pens)

        if not fl["cb"]:
            for ki in range(3):
                for ft in range(2):
                    co = ki * 2 + ft
                    nc.vector.tensor_scalar_add(
                        out=rep[:, co, :], in0=rep[:, co, :],
                        scalar1=cb[:, ki, ft:ft + 1])
        nc.scalar.activation(out=rep[:], in_=rep[:], func=AF.Relu)

        # partial logits per branch hypothesis: fps[:, b, :] = fcw_b^T @ rep
        fps = ps_mm.tile([128, 3, NSEQ], F32, tag="mm", name="fps")
        for b_ in range(3):
            for co in range(NCH):
                nc.tensor.matmul(fps[0:NCLS, b_, :], fcw[:, b_, co, :],
                                 rep[:, co, :],
                                 start=co == 0, stop=co == NCH - 1)
        ob = small.tile([NCLS, 3, NSEQ], F32, tag="ob")
        nc.scalar.copy(out=ob[:], in_=fps[0:NCLS, :, :])
        nc.sync.dma_start(out=out_d[:], in_=ob[:])

    nc.compile()
    return nc


def _classify(inputs):
    """Compute per-core composition and the seq->(core, slot) assignment.

    Returns (ns, assign) where assign[core] is a list of NSEQ global
    sequence ids (branch*32 + sample) in slot order."""
    lens = []
    for p in ("q", "a", "b"):
        lens.append(np.asarray(inputs[p + "_attention_mask"]).sum(1))
    lens = np.concatenate(lens)          # [96], id = branch*32+sample
    short_ids = np.where(lens <= 128)[0]
    ns = min(len(short_ids) // NCORES, NSEQ)
    ns -= ns % 2
    n_short = ns * NCORES
    order = np.argsort(lens, kind="stable")
    shorts = [i for i in order if lens[i] <= 128][:n_short]
    short_set = set(shorts)
    longs = [i for i in order[::-1] if i not in short_set]
    pnt = _pair_nts(ns)
    assign = []
    for c in range(NCORES):
        my_s = shorts[c * ns:(c + 1) * ns]
        my_l = longs[c * (NSEQ - ns):(c + 1) * (NSEQ - ns)]
        si, li = 0, 0
        slots = []
        for p in range(NPAIR):
            for _ in range(2):
                if pnt[p] == 1:
                    slots.append(my_s[si]); si += 1
                else:
                    slots.append(my_l[li]); li += 1
        assign.append(slots)
    return ns, assign


def _core_inputs(inputs, fl, ns, assign):
    f32 = lambda a: np.ascontiguousarray(np.asarray(a, dtype=np.float32))
    tile_w = lambda w: np.ascontiguousarray(
        f32(w).reshape(w.shape[0] // 128, 128, w.shape[1])
        .transpose(1, 0, 2).astype(BF))
    tile_w8 = lambda w: np.ascontiguousarray(
        (f32(w) * FFN_WS).reshape(w.shape[0] // 128, 128, w.shape[1])
        .transpose(1, 0, 2).astype(F8))

    pnt = _pair_nts(ns)
    seq_nt = []
    for p in range(NPAIR):
        seq_nt += [pnt[p], pnt[p]]

    shared = {}
    shared["posty"] = np.ascontiguousarray(
        (f32(inputs["pos_emb"][:S]) + f32(inputs["type_emb"][0])).reshape(
            2, 128, D))
    for l in range(NL):
        if FP8_QKV:
            # no host /8 on Wq in fp8 (subnormal risk); folded in the drain
            shared[f"wq{l}"] = tile_w8(inputs["Wq"][l])
            shared[f"wk{l}"] = tile_w8(inputs["Wk"][l])
            shared[f"wv{l}"] = tile_w8(inputs["Wv"][l])
            shared[f"wo{l}"] = tile_w8(inputs["Wo"][l])
        else:
            shared[f"wq{l}"] = tile_w(f32(inputs["Wq"][l]) / 8.0)
            shared[f"wk{l}"] = tile_w(inputs["Wk"][l])
            shared[f"wv{l}"] = tile_w(inputs["Wv"][l])
            shared[f"wo{l}"] = tile_w(inputs["Wo"][l])
        if FP8_FFN:
            shared[f"wi{l}"] = tile_w8(inputs["Wi"][l])
            shared[f"wo2{l}"] = tile_w8(inputs["Wo2"][l])
        else:
            shared[f"wi{l}"] = tile_w(inputs["Wi"][l])
            shared[f"wo2{l}"] = tile_w(inputs["Wo2"][l])
        if not fl["bqk"]:
            shared[f"bq{l}"] = f32(inputs["bq"][l]).reshape(ND, 128) / 8.0
            shared[f"bk{l}"] = f32(inputs["bk"][l]).reshape(ND, 128)
        if not fl["bv"]:
            shared[f"bv{l}"] = f32(inputs["bv"][l])
        if not fl["bo"]:
            shared[f"bo{l}"] = f32(inputs["bo"][l])
        if not fl["bi"]:
            shared[f"bi{l}"] = f32(inputs["bi"][l]).reshape(NFT, 128)
        if not fl["bo2"]:
            shared[f"bo2{l}"] = f32(inputs["bo2"][l])
        if not fl["ln"]:
            shared[f"ln1s{l}"] = f32(inputs["ln1_s"][l])
            shared[f"ln1b{l}"] = f32(inputs["ln1_b"][l])
            shared[f"ln2s{l}"] = f32(inputs["ln2_s"][l])
            shared[f"ln2b{l}"] = f32(inputs["ln2_b"][l])
    if not fl["ln"]:
        shared["lnes"] = f32(inputs["emb_ln_s"])
        shared["lneb"] = f32(inputs["emb_ln_b"])
    for ki, k in enumerate((1, 2, 3)):
        w = f32(inputs[f"conv_w{k}"])          # [NF, k, D]
        wt = np.ascontiguousarray(w.transpose(1, 2, 0))  # [k, D, NF]
        shared[f"cw{k}"] = np.ascontiguousarray(
            wt.reshape(k, ND, 128, NF).transpose(0, 2, 1, 3).astype(BF))
    # fc blocks in reference concat order (q_rep, b_rep, a_rep); index by
    # input branch id 0=q 1=a 2=b
    fcw = f32(inputs["fc_w"]).reshape(3, NCH, 128, NCLS)
    shared["fcw"] = np.ascontiguousarray(fcw[[0, 2, 1]].transpose(2, 0, 1, 3))
    if not fl["cb"]:
        shared["convb"] = np.stack(
            [f32(inputs[f"conv_b{k}"]).reshape(2, 128) for k in (1, 2, 3)])
    shared["word_emb"] = f32(inputs["word_emb"])

    all_ids = np.concatenate([np.asarray(inputs[p + "_input_ids"])
                              for p in ("q", "a", "b")]).astype(np.int32)
    all_masks = np.concatenate([np.asarray(inputs[p + "_attention_mask"])
                                for p in ("q", "a", "b")]).astype(np.int32)
    all_lens = all_masks.sum(1)

    in_maps = []
    for c in range(NCORES):
        sids = assign[c]
        ids_tiles, mb_tiles = [], []
        cmask = np.zeros((NSEQ, S + 1), dtype=np.float32)
        pen = np.zeros((NSEQ, 3, S + 1), dtype=np.float32)
        for j, gid in enumerate(sids):
            nt = seq_nt[j]
            for tt in range(nt):
                ids_tiles.append(all_ids[gid, tt * 128:(tt + 1) * 128])
                mb_tiles.append(
                    (all_masks[gid, tt * 128:(tt + 1) * 128] - 1) * 10000.0)
            cmask[j, 0:S] = all_masks[gid]
            L = all_lens[gid]
            swj = nt * 128 + 1
            for ki, k in enumerate((1, 2, 3)):
                valid = (np.arange(S + 1) + k - 1) <= L
                valid[swj - k + 1:] = False
                pen[j, ki] = np.where(valid, 0.0, -1e30)
        m = dict(shared)
        m["ids"] = np.ascontiguousarray(
            np.stack(ids_tiles).astype(np.int32).T)
        m["maskbias"] = np.ascontiguousarray(
            np.stack(mb_tiles).astype(np.float32).T)
        m["convmask"] = np.ascontiguousarray(cmask.astype(BF))
        m["convpen"] = np.ascontiguousarray(pen)
        in_maps.append(m)
    return in_maps


def _get_program(fl, pnt):
    key = (tuple(sorted(fl.items())), pnt)
    if key not in _CACHE:
        _CACHE[key] = _build_program(fl, pnt)
    return _CACHE[key]


def run_sharded(inputs, debug=False, **run_kwargs):
    """Shard, run on 8 cores, gather. Returns (output, BassKernelResults)."""
    from concourse.bass_utils import run_bass_kernel_spmd
    fl = _flags(inputs)
    ns, assign = _classify(inputs)
    nc = _get_program(fl, _pair_nts(ns))
    in_maps = _core_inputs(inputs, fl, ns, assign)
    res = run_bass_kernel_spmd(nc, in_maps, core_ids=list(range(NCORES)),
                               **run_kwargs)
    out = np.zeros((B, NCLS), dtype=np.float32)
    for c in range(NCORES):
        part = np.asarray(res.results[c]["out"], dtype=np.float32)  # [4,3,12]
        for j, gid in enumerate(assign[c]):
            br, sample = gid // B, gid % B
            out[sample] += part[:, br, j]
    out += np.asarray(inputs["fc_b"], dtype=np.float32)[None, :]
    return out, res


def kernel(**inputs):
    out, _ = run_sharded(inputs)
    return out


# revision 68
# speedup vs baseline: 1.0179x; 1.0007x over previous
"""Trainium2 Bass kernel for nn_BertCNN (3x BERT-small encoder + CNN maxpool head).

Strategy: data-parallel over *sequences* across 8 NeuronCores. The 96
sequences (32 samples x 3 branches) are classified by ragged length into
"short" (fits one 128-token tile) and "long" (two tiles), then dealt to
cores so every core gets the same composition (e.g. 6 short + 6 long)
and runs an identical SPMD program. Each core runs the 4-layer BERT
encoder + conv/maxpool head on its 12 sequences and emits per-sequence
partial logits [4cls, 3branch-hypotheses, 12seq]; the host selects the
real branch row per sequence, sums the 3 branch contributions of every
sample and adds fc_b (pure gather/unshard arithmetic).

Dataflow per core (all big matmuls bf16 operands, fp32 PSUM accumulation):
  - token embeddings gathered on-device via indirect DMA from word_emb
  - residual stream token-major fp32->bf16 in SBUF; a feature-major bf16
    transposed copy (PE-free DMA transpose, one XBAR call per pair)
    feeds the QKV / FFN matmuls
  - sequences processed in pairs (short pair = 2x1 tile, long = 2x2) so
    projection / FFN1 matmuls run at the widest moving-operand width
  - attention in S^T = [key, query] layout: the ragged-length mask folds
    into the Exp activation as a per-partition bias; exp(S^T) is the
    lhsT of the context matmul; softmax denominators come from a ones
    column appended to V
  - LayerNorm rstd via Quake-style bit-trick rsqrt + 2 Newton steps on
    the DVE: keeps Sqrt off the ACT engine so the only ACT table loads
    are the per-iteration Exp<->Gelu switch
  - conv head runs feature-major ([filters, windows]); global maxpool is
    a per-partition free-axis reduce_max; ragged window validity is a
    -1e30 penalty added before the max; short sequences use 129-wide
    windows instead of 257

Engine-queue discipline (from trace analysis of the v0 kernel):
  - sync queue: ONLY DMA transposes (+ final output store)
  - scalar queue (ACT HWDGE): prologue constants + layer-0 weights
  - gpsimd queue (SWDGE): embedding gathers, layer 1..3 / conv / fc
    weight prefetch, conv masks
  - per-iteration emission order keeps the PE queue dependency-clean:
    ctx -> [next-pair QKV backfill] -> WO -> [next V] -> [next scores g0]
    -> LN1 -> FFN1 -> FFN2 (ft-outer, chases the Gelu chain) ->
    [next scores g1] -> LN2 -> pair transpose -> conv (last layer)
"""

import numpy as np
import ml_dtypes

V, D, H, DH, NL, FF = 30522, 512, 8, 64, 4, 2048
NF, NCLS, B, S = 256, 4, 32, 256
NCORES, SPC = 8, 4
NSEQ = 3 * SPC          # 12 sequences per core
NPAIR = NSEQ // 2       # 6 sequence pairs
ND = D // 128            # 4 feature tiles
NFT = FF // 128          # 16 FFN hidden tiles
NCH = 6                  # per-branch fc chunks of 128 (3 kernels x 2 ftiles)

BF = ml_dtypes.bfloat16
F8 = ml_dtypes.float8_e4m3
_CACHE = {}
FP8_FFN = True           # fp8e4 DoubleRow matmuls for FFN1/FFN2
FFN_XS, FFN_WS = 8.0, 16.0   # fp8 quantization scales (powers of 2)
FP8_QKV = False          # fp8e4 DoubleRow for the QKV/WO projections too
#   (tried: saves ~90us PE but congests DVE/ACT and costs 0.4e-2 rel err)
#   (scores / softmax / ctx stay bf16; only the projections quantize)
QKV_XS, CTX_S = 16.0, 32.0   # x / ctx activation scales for fp8
QUAKE_LN = True          # DVE-only rsqrt (bit trick + Newton steps)
QUAKE_ITERS = 1          # Newton steps (1 -> 1.8e-3 rstd rel err, plenty here)
BCAST_NORM = True        # one stride-0-broadcast DVE op per ctx 4-head group
BF16_R = True            # residual/LN scratch tiles in bf16 (2x DVE modes)


def _flags(inputs):
    z = lambda a: bool(np.all(np.asarray(a) == 0))
    o = lambda a: bool(np.all(np.asarray(a) == 1))
    return {
        "bqk": z(inputs["bq"]) and z(inputs["bk"]),
        "bv": z(inputs["bv"]),
        "bo": z(inputs["bo"]),
        "bi": z(inputs["bi"]),
        "bo2": z(inputs["bo2"]),
        "ln": all(o(inputs[k]) for k in ("emb_ln_s", "ln1_s", "ln2_s"))
        and all(z(inputs[k]) for k in ("emb_ln_b", "ln1_b", "ln2_b")),
        "cb": z(inputs["conv_b1"]) and z(inputs["conv_b2"]) and z(inputs["conv_b3"]),
    }


def _pair_nts(ns):
    """Pair tile-counts for a core with ns short seqs: one short pair
    first (fast PE start), then alternate so every short-pair iteration
    gets a long next-pair to backfill its LN windows; a short pair last
    (small final conv)."""
    nps = ns // 2
    npl = NPAIR - nps
    pn = []
    s_left, l_left = nps, npl
    want_short = True
    while s_left + l_left > 0:
        if want_short and s_left > 0:
            pn.append(1); s_left -= 1
        elif l_left > 0:
            pn.append(2); l_left -= 1
        else:
            pn.append(1); s_left -= 1
        # keep one short for the tail if possible
        want_short = not want_short if s_left > 1 or l_left == 0 else False
        if s_left == 1 and l_left == 0:
            want_short = True
    return tuple(pn)


def _build_program(fl, pn):
    import contextlib
    import concourse.bass as bass
    import concourse.mybir as mybir
    import concourse.tile as tile
    from concourse import bacc
    from concourse.masks import make_identity

    F32, BF16, I32 = mybir.dt.float32, mybir.dt.bfloat16, mybir.dt.int32
    U32 = mybir.dt.uint32
    AL, AF = mybir.AluOpType, mybir.ActivationFunctionType

    seq_nt = []
    for p in range(NPAIR):
        seq_nt += [pn[p], pn[p]]
    tbase = np.concatenate([[0], np.cumsum(seq_nt)]).astype(int)
    NT_TOT = int(tbase[-1])

    nc = bacc.Bacc("TRN2", target_bir_lowering=False, debug=False,
                   num_devices=NCORES)

    di = lambda n, s, d: nc.dram_tensor(n, s, d, kind="ExternalInput").ap()
    F8D = mybir.dt.float8e4
    PDT = F8D if FP8_QKV else BF16
    word = di("word_emb", [V, D], F32)
    ids_d = di("ids", [128, NT_TOT], I32)
    mb_d = di("maskbias", [128, NT_TOT], F32)
    posty_d = di("posty", [2, 128, D], F32)
    cmask_d = di("convmask", [NSEQ, S + 1], BF16)
    cpen_d = di("convpen", [NSEQ, 3, S + 1], F32)
    wq_d = [di(f"wq{l}", [128, ND, D], PDT) for l in range(NL)]
    wk_d = [di(f"wk{l}", [128, ND, D], PDT) for l in range(NL)]
    wv_d = [di(f"wv{l}", [128, ND, D], PDT) for l in range(NL)]
    wo_d = [di(f"wo{l}", [128, ND, D], PDT) for l in range(NL)]
    WDT = F8D if FP8_FFN else BF16
    wi_d = [di(f"wi{l}", [128, ND, FF], WDT) for l in range(NL)]
    wo2_d = [di(f"wo2{l}", [128, NFT, D], WDT) for l in range(NL)]
    cw_d = [di(f"cw{k}", [k, 128, ND, NF], BF16) for k in (1, 2, 3)]
    fcw_d = di("fcw", [128, 3, NCH, NCLS], F32)
    if not fl["bqk"]:
        bq_d = [di(f"bq{l}", [ND, 128], F32) for l in range(NL)]
        bk_d = [di(f"bk{l}", [ND, 128], F32) for l in range(NL)]
    if not fl["bv"]:
        bv_d = [di(f"bv{l}", [D], F32) for l in range(NL)]
    if not fl["bo"]:
        bo_d = [di(f"bo{l}", [D], F32) for l in range(NL)]
    if not fl["bi"]:
        bi_d = [di(f"bi{l}", [NFT, 128], F32) for l in range(NL)]
    if not fl["bo2"]:
        bo2_d = [di(f"bo2{l}", [D], F32) for l in range(NL)]
    if not fl["ln"]:
        elns_d = di("lnes", [D], F32)
        elnb_d = di("lneb", [D], F32)
        ln1s_d = [di(f"ln1s{l}", [D], F32) for l in range(NL)]
        ln1b_d = [di(f"ln1b{l}", [D], F32) for l in range(NL)]
        ln2s_d = [di(f"ln2s{l}", [D], F32) for l in range(NL)]
        ln2b_d = [di(f"ln2b{l}", [D], F32) for l in range(NL)]
    if not fl["cb"]:
        cb_d = di("convb", [3, 2, 128], F32)

    out_d = nc.dram_tensor("out", [NCLS, 3, NSEQ], F32,
                           kind="ExternalOutput").ap()

    with tile.TileContext(nc) as tc, contextlib.ExitStack() as ctx:
        consts = ctx.enter_context(tc.tile_pool(name="consts", bufs=1))
        state = ctx.enter_context(tc.tile_pool(name="state", bufs=1))
        wts = ctx.enter_context(tc.tile_pool(name="wts", bufs=1))
        big = ctx.enter_context(tc.tile_pool(name="big", bufs=1))
        work = ctx.enter_context(tc.tile_pool(name="work", bufs=2))
        small = ctx.enter_context(tc.tile_pool(name="small", bufs=4))
        ps_mm = ctx.enter_context(tc.tile_pool(name="ps_mm", bufs=6, space="PSUM"))
        ps_ctx = ctx.enter_context(tc.tile_pool(name="ps_ctx", bufs=2, space="PSUM"))

        # ---- prologue constants: scalar (ACT) HWDGE queue, critical first ----
        ids_sb = consts.tile([128, NT_TOT], I32, tag="ids")
        nc.scalar.dma_start(out=ids_sb[:], in_=ids_d)
        posty = consts.tile([128, 2, D], F32, tag="posty")
        nc.scalar.dma_start(out=posty[:], in_=posty_d.rearrange("t p d -> p t d"))
        mb_sb = consts.tile([128, NT_TOT], F32, tag="mb")
        nc.scalar.dma_start(out=mb_sb[:], in_=mb_d)
        ident = consts.tile([128, 128], BF16, tag="ident")
        magic = consts.tile([128, 8], I32, tag="magic")
        nc.vector.memset(magic[:], 0x5F3759DF)
        if not QUAKE_LN:
            eps_t = consts.tile([128, 1], F32, tag="eps")
            nc.vector.memset(eps_t[:], 1e-12)

        bcast = lambda ap, n: ap[None, :].to_broadcast([128, n])
        if not fl["ln"]:
            elns = consts.tile([128, D], F32, tag="elns")
            nc.scalar.dma_start(out=elns[:], in_=bcast(elns_d, D))
            elnb = consts.tile([128, D], F32, tag="elnb")
            nc.scalar.dma_start(out=elnb[:], in_=bcast(elnb_d, D))

        # persistent per-pair state: token-major residual + feature-major copy
        RDT = BF16 if BF16_R else F32
        XDT = F8D if FP8_QKV else BF16
        x_p = [state.tile([128, 2, pn[q], D], BF16, tag=f"xp{q}",
                          name=f"xp{q}") for q in range(NPAIR)]
        if FP8_QKV:
            xT = [state.tile([128, ND, 2 * pn[q] * 128], F8D, tag=f"xT{q}",
                             name=f"xT{q}") for q in range(NPAIR)]
        else:
            xT = [state.tile([128, 2, pn[q], ND, 128], BF16, tag=f"xT{q}",
                             name=f"xT{q}") for q in range(NPAIR)]
        rep = state.tile([128, NCH, NSEQ], F32, tag="rep")

        def ln_stats_batch(rs):
            """Pipelined LN stats for a list of [128, D] sources; returns
            (rcp, nmb) where rcp[:, i] = rstd_i, nmb[:, i] = -mean_i*rstd_i."""
            n = len(rs)
            mvt = small.tile([128, n, 2], F32, tag="mvt")
            for i, r in enumerate(rs):
                st = small.tile([128, 6], F32, tag="st", name=f"st{i}")
                nc.vector.bn_stats(out=st[:], in_=r)
                nc.vector.bn_aggr(out=mvt[:, i, :], in_=st[:])
            rcp = small.tile([128, n], F32, tag="rcp2")
            if QUAKE_LN:
                # rstd = rsqrt(var + eps): Quake bit-trick + 2 Newton steps,
                # all on the DVE (no ACT Sqrt -> no act-table thrash)
                vv = small.tile([128, n], F32, tag="vv")
                nc.vector.tensor_scalar_add(out=vv[:], in0=mvt[:, :, 1],
                                            scalar1=1e-12)
                nc.vector.tensor_scalar(
                    out=rcp[:].bitcast(I32), in0=vv[:].bitcast(I32),
                    scalar1=1, scalar2=0, op0=AL.logical_shift_right,
                    op1=AL.bypass)
                nc.vector.tensor_tensor(
                    out=rcp[:].bitcast(I32), in0=magic[:, 0:n],
                    in1=rcp[:].bitcast(I32), op=AL.subtract)
                t = small.tile([128, n], F32, tag="qt")
                for _ in range(QUAKE_ITERS):
                    nc.vector.tensor_tensor(out=t[:], in0=rcp[:], in1=rcp[:],
                                            op=AL.mult)
                    nc.vector.tensor_tensor(out=t[:], in0=t[:], in1=vv[:],
                                            op=AL.mult)
                    nc.vector.tensor_scalar(out=t[:], in0=t[:], scalar1=-0.5,
                                            scalar2=1.5, op0=AL.mult,
                                            op1=AL.add)
                    nc.vector.tensor_tensor(out=rcp[:], in0=rcp[:], in1=t[:],
                                            op=AL.mult)
            else:
                sd = small.tile([128, n], F32, tag="sd")
                nc.scalar.activation(out=sd[:], in_=mvt[:, :, 1],
                                     func=AF.Sqrt, bias=eps_t[:], scale=1.0)
                nc.vector.reciprocal(out=rcp[:], in_=sd[:])
            nmb = small.tile([128, n], F32, tag="nmb")
            # nmb = (mean * -1) * rstd
            nc.vector.scalar_tensor_tensor(
                out=nmb[:], in0=mvt[:, :, 0], scalar=-1.0, in1=rcp[:],
                op0=AL.mult, op1=AL.mult)
            return rcp, nmb

        def ln_apply_batch(rs, dsts, rcp, nmb, s_tile, b_tile):
            for i in range(len(rs)):
                if False and s_tile is None and b_tile is None and i % 2 == 1:
                    # odd tiles apply on ACT (Copy: in every table, no load)
                    # so the two engines drain the batch in parallel
                    nc.scalar.activation(
                        out=dsts[i], in_=rs[i], func=AF.Identity,
                        bias=nmb[:, i:i + 1], scale=rcp[:, i:i + 1])
                    continue
                nc.vector.tensor_scalar(
                    out=dsts[i], in0=rs[i], scalar1=rcp[:, i:i + 1],
                    scalar2=nmb[:, i:i + 1], op0=AL.mult, op1=AL.add)
                if s_tile is not None:
                    nc.vector.tensor_tensor(out=dsts[i], in0=dsts[i],
                                            in1=s_tile[:], op=AL.mult)
                if b_tile is not None:
                    nc.vector.tensor_tensor(out=dsts[i], in0=dsts[i],
                                            in1=b_tile[:], op=AL.add)

        def feat_major(pr, dst, scale):
            """PE-transpose x_p[pr] into a feature-major copy dst with a
            fused scale+cast drain. Transposes go tile-outer so they chase
            the LN applies tile-by-tile instead of waiting for the batch."""
            nt = pn[pr]
            sw = 2 * nt * 128
            for dt in range(ND):
                tps = ps_mm.tile([128, sw], BF16, tag="mm",
                                 name=f"fm{pr}_{dt}")
                for i in range(2 * nt):
                    si, tt = i // nt, i % nt
                    nc.tensor.transpose(
                        tps[:, i * 128:(i + 1) * 128],
                        x_p[pr][:, si, tt, dt * 128:(dt + 1) * 128],
                        ident[:])
                if scale == 1.0:
                    nc.vector.tensor_copy(out=dst[:, dt, :], in_=tps[:])
                else:
                    nc.vector.tensor_scalar_mul(out=dst[:, dt, :],
                                                in0=tps[:], scalar1=scale)

        def to_feat(pr, eng=None):
            """Refresh the feature-major x copy after an LN2 update."""
            if FP8_QKV:
                feat_major(pr, xT[pr], QKV_XS)
            else:
                (eng or nc.sync).dma_start_transpose(xT[pr][:, :, :, :, :],
                                                     x_p[pr][:, :, :, :])

        def embed_pair(p, pt_eng=None):
            nt = pn[p]
            t0 = int(tbase[2 * p])
            gb = work.tile([128, 2 * nt, D], F32, tag="r", name=f"g{p}",
                           bufs=2)
            tiles = []
            for si in range(2):
                for tt in range(nt):
                    j = si * nt + tt
                    nc.gpsimd.indirect_dma_start(
                        out=gb[:, j, :], out_offset=None, in_=word[:],
                        in_offset=bass.IndirectOffsetOnAxis(
                            ap=ids_sb[:, t0 + j:t0 + j + 1], axis=0))
                    nc.vector.tensor_tensor(out=gb[:, j, :], in0=gb[:, j, :],
                                            in1=posty[:, tt, :], op=AL.add)
                    tiles.append((si, tt))
            rcp, nmb = ln_stats_batch([gb[:, si * nt + tt, :]
                                       for si, tt in tiles])
            ln_apply_batch([gb[:, si * nt + tt, :] for si, tt in tiles],
                           [x_p[p][:, si, tt, :] for si, tt in tiles],
                           rcp, nmb,
                           None if fl["ln"] else elns,
                           None if fl["ln"] else elnb)
            to_feat(p, pt_eng)

        def load_layer_weights_A(l, q):
            """QKV weights (+ small per-layer consts) for layer l."""
            w = {}
            for nm, dd in (("wq", wq_d), ("wk", wk_d), ("wv", wv_d)):
                w[nm] = wts.tile([128, ND, D], PDT, tag=nm, name=f"{nm}_{l}")
                for dt in range(0, ND, 2):
                    q.dma_start(out=w[nm][:, dt:dt + 2, :],
                                in_=dd[l][:, dt:dt + 2, :])
            if not fl["bqk"]:
                w["bq"] = consts.tile([128, ND], F32, tag="bq", name=f"bq_{l}")
                q.dma_start(out=w["bq"][:], in_=bq_d[l].rearrange("t p -> p t"))
                w["bk"] = consts.tile([128, ND], F32, tag="bk", name=f"bk_{l}")
                q.dma_start(out=w["bk"][:], in_=bk_d[l].rearrange("t p -> p t"))
            if not fl["bv"]:
                w["bv"] = consts.tile([128, D], F32, tag="bv", name=f"bv_{l}")
                q.dma_start(out=w["bv"][:], in_=bcast(bv_d[l], D))
            if not fl["ln"]:
                for nm, dd in (("ln1s", ln1s_d), ("ln1b", ln1b_d),
                               ("ln2s", ln2s_d), ("ln2b", ln2b_d)):
                    w[nm] = consts.tile([128, D], F32, tag=nm, name=f"{nm}_{l}")
                    q.dma_start(out=w[nm][:], in_=bcast(dd[l], D))
            return w

        def load_layer_weights_B(l, w, q):
            """WO / FFN weights for layer l (emit after last layer-(l-1) use)."""
            w["wo"] = wts.tile([128, ND, D], PDT, tag="wo", name=f"wo_{l}")
            for dt in range(0, ND, 2):
                q.dma_start(out=w["wo"][:, dt:dt + 2, :],
                            in_=wo_d[l][:, dt:dt + 2, :])
            w["wi"] = wts.tile([128, ND, FF], WDT, tag="wi", name=f"wi_{l}")
            for dt in range(0, ND, 2):
                q.dma_start(out=w["wi"][:, dt:dt + 2, :],
                            in_=wi_d[l][:, dt:dt + 2, :])
            w["wo2"] = wts.tile([128, NFT, D], WDT, tag="wo2", name=f"wo2_{l}")
            for ft in range(0, NFT, 8):
                q.dma_start(out=w["wo2"][:, ft:ft + 8, :],
                            in_=wo2_d[l][:, ft:ft + 8, :])
            if not fl["bo"]:
                w["bo"] = consts.tile([128, D], F32, tag="bo", name=f"bo_{l}")
                q.dma_start(out=w["bo"][:], in_=bcast(bo_d[l], D))
            if not fl["bi"]:
                w["bi"] = consts.tile([128, NFT], F32, tag="bi", name=f"bi_{l}")
                q.dma_start(out=w["bi"][:], in_=bi_d[l].rearrange("t p -> p t"))
            if not fl["bo2"]:
                w["bo2"] = consts.tile([128, D], F32, tag="bo2", name=f"bo2_{l}")
                q.dma_start(out=w["bo2"][:], in_=bcast(bo2_d[l], D))
            return w

        # fp8 dequant scales for the projection drains (wq carries no /8 on
        # the host in fp8 mode; the score scaling folds in here instead)
        P_SC = 1.0 / (QKV_XS * FFN_WS) if FP8_QKV else 1.0
        Q_SC = P_SC * 0.125 if FP8_QKV else 1.0
        O_SC = 1.0 / (CTX_S * FFN_WS) if FP8_QKV else 1.0

        def proj_mm(ps, xts, wt, ocols):
            """One QKV-projection matmul chain (DR fp8 or bf16)."""
            if FP8_QKV:
                for dp in range(2):
                    nc.tensor.matmul(
                        ps[:], wt[:, 2 * dp:2 * dp + 2, ocols],
                        xts[:, 2 * dp:2 * dp + 2, :],
                        start=dp == 0, stop=dp == 1,
                        perf_mode=mybir.MatmulPerfMode.DoubleRow)
            else:
                for dt in range(ND):
                    nc.tensor.matmul(
                        ps[:], wt[:, dt, ocols], xts[:, :, :, dt, :],
                        start=dt == 0, stop=dt == ND - 1)

        def qkv_pair(pr, w):
            """qT/kT feature-major bf16 for both seqs."""
            nt = pn[pr]
            sw = 2 * nt * 128
            xts = xT[pr]
            qT = work.tile([128, ND, sw], BF16, tag="qT", name=f"qT{pr}")
            kT = work.tile([128, ND, sw], BF16, tag="kT", name=f"kT{pr}")
            for dst_t, wt, which in ((qT, w["wq"], "q"), (kT, w["wk"], "k")):
                sc = Q_SC if which == "q" else P_SC
                for ot in range(ND):
                    ps = ps_mm.tile([128, sw], F32, tag="mm")
                    proj_mm(ps, xts, wt, slice(ot * 128, (ot + 1) * 128))
                    if fl["bqk"]:
                        # split PSUM drain across ACT and DVE so neither
                        # engine's queue gates PSUM recycling
                        if which == "q":
                            nc.scalar.mul(out=dst_t[:, ot, :], in_=ps[:],
                                          mul=sc)
                        elif sc == 1.0:
                            nc.vector.tensor_copy(out=dst_t[:, ot, :],
                                                  in_=ps[:])
                        else:
                            nc.vector.tensor_scalar_mul(
                                out=dst_t[:, ot, :], in0=ps[:], scalar1=sc)
                    else:
                        bt = w["bq"] if which == "q" else w["bk"]
                        nc.vector.tensor_scalar(
                            out=dst_t[:, ot, :], in0=ps[:], scalar1=sc,
                            scalar2=bt[:, ot:ot + 1], op0=AL.mult, op1=AL.add)
            return qT, kT

        def v_pair(pr, w):
            """V token-major bf16 with a ones column per head, both seqs."""
            nt = pn[pr]
            xts = xT[pr]
            vAs = []
            for si in range(2):
                vA = work.tile([128, nt, H, DH + 1], BF16, tag="vA",
                               name=f"vA{pr}_{si}")
                nc.vector.memset(vA[:, :, :, DH:DH + 1], 1.0)
                for tt in range(nt):
                    tok = (si * nt + tt) * 128
                    ps = ps_mm.tile([128, D], F32, tag="mm")
                    if FP8_QKV:
                        for dp in range(2):
                            nc.tensor.matmul(
                                ps[:], xts[:, 2 * dp:2 * dp + 2, tok:tok + 128],
                                w["wv"][:, 2 * dp:2 * dp + 2, :],
                                start=dp == 0, stop=dp == 1,
                                perf_mode=mybir.MatmulPerfMode.DoubleRow)
                    else:
                        for dt in range(ND):
                            nc.tensor.matmul(
                                ps[:], xts[:, si, tt, dt, :],
                                w["wv"][:, dt, :], start=dt == 0,
                                stop=dt == ND - 1)
                    if fl["bv"]:
                        if P_SC == 1.0:
                            nc.vector.tensor_copy(
                                out=vA[:, tt, :, 0:DH],
                                in_=ps.rearrange("p (h d) -> p h d", h=H))
                        else:
                            nc.vector.tensor_scalar_mul(
                                out=vA[:, tt, :, 0:DH],
                                in0=ps.rearrange("p (h d) -> p h d", h=H),
                                scalar1=P_SC)
                    else:
                        nc.vector.scalar_tensor_tensor(
                            out=vA[:, tt, :, 0:DH],
                            in0=ps.rearrange("p (h d) -> p h d", h=H),
                            scalar=P_SC,
                            in1=w["bv"].rearrange("p (h d) -> p h d", h=H),
                            op0=AL.mult, op1=AL.add)
                vAs.append(vA)
            return vAs

        att_state = {}

        def att_scores(pr, g_, qk):
            """S^T + exp for head-group g_ of both seqs of pair pr."""
            nt = pn[pr]
            sq = nt * 128       # queries per seq
            qT, kT = qk
            for si in range(2):
                seq = 2 * pr + si
                so = si * sq
                eT = work.tile([128, nt, 4, sq], BF16, tag="eT",
                               name=f"eT{pr}_{g_}_{si}", bufs=4)
                att_state[(pr, g_, si)] = eT
                for kt in range(nt):
                    ti = int(tbase[seq]) + kt
                    for hi in range(4):
                        h = g_ * 4 + hi
                        ot, hh = h // 2, (h % 2) * DH
                        ps = ps_mm.tile([128, sq], F32, tag="mm")
                        nc.tensor.matmul(
                            ps[:],
                            kT[hh:hh + DH, ot, so + kt * 128:so + (kt + 1) * 128],
                            qT[hh:hh + DH, ot, so:so + sq],
                            start=True, stop=True)
                        nc.scalar.activation(
                            out=eT[:, kt, hi, :], in_=ps[:],
                            func=AF.Exp, bias=mb_sb[:, ti:ti + 1], scale=1.0)

        def att_ctx(pr, g_, vAs, ctxb):
            """ctx (+denominator) matmuls and DVE normalization for group g_."""
            nt = pn[pr]
            for si in range(2):
                eT, vA = att_state.pop((pr, g_, si)), vAs[si]
                for qt in range(nt):
                    cps = ps_ctx.tile([128, 4 * (DH + 1)], F32, tag="ctx",
                                      name=f"ctx{si}_{qt}_{g_}")
                    for hi in range(4):
                        h = g_ * 4 + hi
                        sl = slice(hi * (DH + 1), (hi + 1) * (DH + 1))
                        for kt in range(nt):
                            nc.tensor.matmul(
                                cps[:, sl],
                                eT[:, kt, hi, qt * 128:(qt + 1) * 128],
                                vA[:, kt, h, :], start=kt == 0,
                                stop=kt == nt - 1)
                    cph = cps.rearrange("p (h c) -> p h c", c=DH + 1)
                    if BCAST_NORM:
                        rcp = small.tile([128, 4, 1], F32, tag="rcp")
                        nc.vector.reciprocal(out=rcp[:],
                                             in_=cph[:, :, DH:DH + 1])
                        nc.vector.scalar_tensor_tensor(
                            out=ctxb[:, si, qt,
                                     g_ * 4 * DH:(g_ + 1) * 4 * DH].rearrange(
                                         "p (h d) -> p h d", d=DH),
                            in0=cph[:, :, 0:DH], scalar=1.0,
                            in1=rcp[:, :, 0:1].broadcast_to([128, 4, DH]),
                            op0=AL.mult, op1=AL.mult)
                    else:
                        rcp = small.tile([128, 4], F32, tag="rcp")
                        nc.vector.reciprocal(out=rcp[:], in_=cph[:, :, DH])
                        for hi in range(4):
                            h = g_ * 4 + hi
                            base = hi * (DH + 1)
                            nc.vector.tensor_scalar_mul(
                                out=ctxb[:, si, qt, h * DH:(h + 1) * DH],
                                in0=cps[:, base:base + DH],
                                scalar1=rcp[:, hi:hi + 1])

        def residual_ln(rs, dsts, s_tile, b_tile):
            rcp, nmb = ln_stats_batch(rs)
            ln_apply_batch(rs, dsts, rcp, nmb, s_tile, b_tile)

        def ctx_half(pr, g_, vAs, ctxb, ct):
            """ctx matmuls + norm for head-group g_, then PE-transpose that
            group's two feature tiles into ct with ACT copy drains (copy is
            in every act table -> no table load, ~1us latency to WO)."""
            nt = pn[pr]
            sw = 2 * nt * 128
            att_ctx(pr, g_, vAs, ctxb)
            for dt in (2 * g_, 2 * g_ + 1):
                tpx = ps_mm.tile([128, sw], BF16, tag="mm",
                                 name=f"ctp{pr}_{dt}")
                for i in range(2 * nt):
                    si, tt = i // nt, i % nt
                    nc.tensor.transpose(
                        tpx[:, i * 128:(i + 1) * 128],
                        ctxb[:, si, tt, dt * 128:(dt + 1) * 128],
                        ident[:])
                if FP8_QKV:
                    nc.scalar.mul(out=ct[:, dt, :], in_=tpx[:], mul=CTX_S)
                else:
                    nc.scalar.copy(out=ct[:, dt, :], in_=tpx[:])

        def ctx_start(pr, vAs):
            """Allocate this pair's ctx tiles and run head-group 0."""
            nt = pn[pr]
            ctxb = work.tile([128, 2, nt, D], BF16, tag="ctxb",
                             name=f"cb{pr}")
            ct = work.tile([128, ND, 2 * nt * 128], XDT, tag="cT",
                           name=f"cT{pr}")
            ctx_half(pr, 0, vAs, ctxb, ct)
            return ctxb, ct

        cw = {}

        def load_conv_weights(q):
            ti = 0
            for ki, k in enumerate((1, 2, 3)):
                for j in range(k):
                    t = wts.tile([128, ND, NF], BF16, tag=f"cw{ti}",
                                 name=f"cwt{k}_{j}")
                    q.dma_start(out=t[:], in_=cw_d[ki][j])
                    cw[(k, j)] = t
                    ti += 1
            fcw = consts.tile([128, 3, NCH, NCLS], F32, tag="fcw")
            q.dma_start(out=fcw[:], in_=fcw_d)
            cb = None
            if not fl["cb"]:
                cb = consts.tile([128, 3, 2], F32, tag="cb")
                q.dma_start(out=cb[:], in_=cb_d.rearrange("k t p -> p k t"))
            return fcw, cb

        def conv_prefetch(pr):
            """Issue the conv mask / window-penalty DMAs for pair pr early."""
            nt = pn[pr]
            swc = nt * 128 + 1
            cms, pens = [], []
            for si in range(2):
                seq = 2 * pr + si
                cm = work.tile([128, swc], BF16, tag="cm", name=f"cm{seq}")
                nc.gpsimd.dma_start(
                    out=cm[:],
                    in_=cmask_d[seq, 0:swc][None, :].to_broadcast([128, swc]))
                cms.append(cm)
                ps_ = []
                for ki in range(3):
                    pen = work.tile([128, swc], F32, tag="pen",
                                    name=f"pen{seq}_{ki}", bufs=6)
                    nc.gpsimd.dma_start(
                        out=pen[:],
                        in_=cpen_d[seq, ki, 0:swc][None, :].to_broadcast(
                            [128, swc]))
                    ps_.append(pen)
                pens.append(ps_)
            return cms, pens

        def conv_pair(pr, cms, pens):
            """Conv head for both seqs of pair pr straight off x_p: PE
            transposes feature tiles into PSUM, the drain fuses the
            token-mask multiply (no DMA transpose on the conv path)."""
            nt = pn[pr]
            swc = nt * 128 + 1
            for si in range(2):
                seq = 2 * pr + si
                xcv = work.tile([128, ND, swc], BF16, tag="xcv",
                                name=f"xcv{seq}")
                nc.vector.memset(xcv[:, :, nt * 128:nt * 128 + 1], 0.0)
                for dt in range(ND):
                    tpc = ps_mm.tile([128, nt * 128], BF16, tag="mm",
                                     name=f"cvtp{seq}_{dt}")
                    for tt in range(nt):
                        nc.tensor.transpose(
                            tpc[:, tt * 128:(tt + 1) * 128],
                            x_p[pr][:, si, tt, dt * 128:(dt + 1) * 128],
                            ident[:])
                    nc.vector.tensor_tensor(out=xcv[:, dt, 0:nt * 128],
                                            in0=tpc[:],
                                            in1=cms[si][:, 0:nt * 128],
                                            op=AL.mult)
                for ki, k in enumerate((1, 2, 3)):
                    nw = swc - k + 1
                    for ft in range(2):
                        ps = ps_mm.tile([128, swc], F32, tag="mm")
                        idx = 0
                        for dt in range(ND):
                            for j in range(k):
                                nc.tensor.matmul(
                                    ps[:, 0:nw],
                                    cw[(k, j)][:, dt, ft * 128:(ft + 1) * 128],
                                    xcv[:, dt, j:j + nw],
                                    start=idx == 0, stop=idx == ND * k - 1)
                                idx += 1
                        cvt = work.tile([128, swc], F32, tag="cvt",
                                        name=f"cv{seq}_{k}_{ft}")
                        nc.vector.tensor_tensor(out=cvt[:, 0:nw],
                                                in0=ps[:, 0:nw],
                                                in1=pens[si][ki][:, 0:nw],
                                                op=AL.add)
                        nc.vector.tensor_reduce(
                            out=rep[:, ki * 2 + ft, seq:seq + 1],
                            in_=cvt[:, 0:nw],
                            axis=mybir.AxisListType.X, op=AL.max)

        # ---- main schedule ----
        # prologue: embed gathers lead the DGE, layer-0 weights trail them
        # on the gpsimd queue in need order (wq/wk -> wv -> wo/wi/wo2)
        make_identity(nc, ident[:])
        # p-state warmup: keep the PE streaming while the embed/weight
        # chain runs so the first real matmuls start at full clock
        warm = consts.tile([128, 512], BF16, tag="warm")
        nc.vector.memset(warm[:], 0.5)
        wps = ps_mm.tile([128, 512], F32, tag="mm", name="warm")
        for _ in range(30):
            nc.tensor.matmul(wps[:], ident[:], warm[:], start=True, stop=True)
        embed_pair(0, pt_eng=nc.scalar)
        w_cur = {}
        for nm, dd in (("wq", wq_d), ("wk", wk_d)):
            w_cur[nm] = wts.tile([128, ND, D], PDT, tag=nm, name=f"{nm}_0")
            for dt in range(0, ND, 2):
                nc.gpsimd.dma_start(out=w_cur[nm][:, dt:dt + 2, :],
                                    in_=dd[0][:, dt:dt + 2, :])
        embed_pair(1, pt_eng=nc.scalar)
        w_cur["wv"] = wts.tile([128, ND, D], PDT, tag="wv", name="wv_0")
        for dt in range(0, ND, 2):
            nc.gpsimd.dma_start(out=w_cur["wv"][:, dt:dt + 2, :],
                                in_=wv_d[0][:, dt:dt + 2, :])
        if not fl["bqk"]:
            w_cur["bq"] = consts.tile([128, ND], F32, tag="bq", name="bq_0")
            nc.gpsimd.dma_start(out=w_cur["bq"][:],
                                in_=bq_d[0].rearrange("t p -> p t"))
            w_cur["bk"] = consts.tile([128, ND], F32, tag="bk", name="bk_0")
            nc.gpsimd.dma_start(out=w_cur["bk"][:],
                                in_=bk_d[0].rearrange("t p -> p t"))
        if not fl["bv"]:
            w_cur["bv"] = consts.tile([128, D], F32, tag="bv", name="bv_0")
            nc.gpsimd.dma_start(out=w_cur["bv"][:], in_=bcast(bv_d[0], D))
        if not fl["ln"]:
            for nm, dd in (("ln1s", ln1s_d), ("ln1b", ln1b_d),
                           ("ln2s", ln2s_d), ("ln2b", ln2b_d)):
                w_cur[nm] = consts.tile([128, D], F32, tag=nm, name=f"{nm}_0")
                nc.gpsimd.dma_start(out=w_cur[nm][:], in_=bcast(dd[0], D))
        w_cur = load_layer_weights_B(0, w_cur, nc.gpsimd)
        qk_cur = qkv_pair(0, w_cur)
        v_cur = v_pair(0, w_cur)
        att_scores(0, 0, qk_cur)
        att_scores(0, 1, qk_cur)
        fcw = cb = None
        for l in range(NL):
            for pr in range(NPAIR):
                nt = pn[pr]
                w = w_cur
                if pr + 1 < NPAIR:
                    nxt_l, nxt_pr = l, pr + 1
                elif l + 1 < NL:
                    nxt_l, nxt_pr = l + 1, 0
                else:
                    nxt_l = nxt_pr = None
                cross = nxt_pr is not None and nxt_l != l

                if cross:
                    # QKV weights of the next layer: all layer-l readers of
                    # wq/wk/wv were emitted by the previous iteration
                    w_nxt = load_layer_weights_A(nxt_l, nc.gpsimd)
                elif nxt_pr is not None:
                    w_nxt = w
                if l == NL - 1:
                    cms, pens = conv_prefetch(pr)

                sw = 2 * nt * 128
                ctxb, ct = ctx_start(pr, v_cur)
                ctx_half(pr, 1, v_cur, ctxb, ct)
                # PE backfill: the next pair's QKV projections
                if nxt_pr is not None:
                    qk_nxt = qkv_pair(nxt_pr, w_nxt)
                # attention out projection + residual
                rs = []
                for i in range(2 * nt):
                    si, tt = i // nt, i % nt
                    ps = ps_mm.tile([128, D], F32, tag="mm")
                    if FP8_QKV:
                        for dp in range(2):
                            nc.tensor.matmul(
                                ps[:], ct[:, 2 * dp:2 * dp + 2,
                                           i * 128:(i + 1) * 128],
                                w["wo"][:, 2 * dp:2 * dp + 2, :],
                                start=dp == 0, stop=dp == 1,
                                perf_mode=mybir.MatmulPerfMode.DoubleRow)
                    else:
                        for dt in range(ND):
                            nc.tensor.matmul(
                                ps[:], ct[:, dt, i * 128:(i + 1) * 128],
                                w["wo"][:, dt, :], start=dt == 0,
                                stop=dt == ND - 1)
                    r = work.tile([128, D], RDT, tag="rln", name=f"r{i}", bufs=4)
                    if O_SC == 1.0:
                        nc.vector.tensor_tensor(out=r[:], in0=ps[:],
                                                in1=x_p[pr][:, si, tt, :],
                                                op=AL.add)
                    else:
                        nc.vector.scalar_tensor_tensor(
                            out=r[:], in0=ps[:], scalar=O_SC,
                            in1=x_p[pr][:, si, tt, :], op0=AL.mult, op1=AL.add)
                    if not fl["bo"]:
                        nc.vector.tensor_tensor(out=r[:], in0=r[:],
                                                in1=w["bo"][:], op=AL.add)
                    rs.append(r[:])
                # more PE backfill: next pair's V and both score groups run
                # while the LN1 chain (pure DVE now) drains
                if nxt_pr is not None:
                    v_nxt = v_pair(nxt_pr, w_nxt)
                    att_scores(nxt_pr, 0, qk_nxt)
                residual_ln(rs, [x_p[pr][:, i // nt, i % nt, :]
                                 for i in range(2 * nt)],
                            None if fl["ln"] else w["ln1s"],
                            None if fl["ln"] else w["ln1b"])
                # PE-transpose the LN1 output straight into PSUM (bf16),
                # then one drain per dt does the fp8 cast + scale
                y1 = work.tile([128, ND, sw], WDT, tag="y1f8",
                               name=f"y1f8{pr}")
                feat_major(pr, y1, FFN_XS if FP8_FFN else 1.0)
                # FFN1: hidden feature-major, gelu fused with bias
                hT = big.tile([128, NFT, sw], F8D if FP8_FFN else BF16,
                              tag="hT")
                for ft in range(NFT):
                    ps = ps_mm.tile([128, sw], F32, tag="mm")
                    if FP8_FFN:
                        for dp in range(2):
                            nc.tensor.matmul(
                                ps[:],
                                w["wi"][:, 2 * dp:2 * dp + 2,
                                        ft * 128:(ft + 1) * 128],
                                y1[:, 2 * dp:2 * dp + 2, :],
                                start=dp == 0, stop=dp == 1,
                                perf_mode=mybir.MatmulPerfMode.DoubleRow)
                    else:
                        for dt in range(ND):
                            nc.tensor.matmul(
                                ps[:], w["wi"][:, dt, ft * 128:(ft + 1) * 128],
                                y1[:, dt, :], start=dt == 0,
                                stop=dt == ND - 1)
                    nc.scalar.activation(
                        out=hT[:, ft, :], in_=ps[:], func=AF.Gelu,
                        bias=0.0 if fl["bi"] else w["bi"][:, ft:ft + 1],
                        scale=1.0 / (FFN_XS * FFN_WS) if FP8_FFN else 1.0)
                # embeds of the remaining pairs ride the FFN window (their
                # DVE chain slots between the y1f8 and FFN2 drains)
                if l == 0 and pr < 4:
                    embed_pair(pr + 2)
                # FFN2 + residual: ft-outer with per-token-tile PSUM so the
                # first matmuls chase the Gelu chain instead of waiting on it
                pss = [ps_mm.tile([128, D], F32, tag="mm",
                                  name=f"f2_{l}_{pr}_{i}")
                       for i in range(2 * nt)]
                if FP8_FFN:
                    for fp_ in range(NFT // 2):
                        for i in range(2 * nt):
                            si, tt = i // nt, i % nt
                            so = si * nt * 128
                            nc.tensor.matmul(
                                pss[i][:],
                                hT[:, 2 * fp_:2 * fp_ + 2,
                                   so + tt * 128:so + (tt + 1) * 128],
                                w["wo2"][:, 2 * fp_:2 * fp_ + 2, :],
                                start=fp_ == 0, stop=fp_ == NFT // 2 - 1,
                                perf_mode=mybir.MatmulPerfMode.DoubleRow)
                else:
                    for ft in range(NFT):
                        for i in range(2 * nt):
                            si, tt = i // nt, i % nt
                            so = si * nt * 128
                            nc.tensor.matmul(
                                pss[i][:],
                                hT[:, ft, so + tt * 128:so + (tt + 1) * 128],
                                w["wo2"][:, ft, :], start=ft == 0,
                                stop=ft == NFT - 1)
                rs = []
                for i in range(2 * nt):
                    si, tt = i // nt, i % nt
                    r = work.tile([128, D], RDT, tag="rln", name=f"r2{i}", bufs=4)
                    if FP8_FFN:
                        nc.vector.scalar_tensor_tensor(
                            out=r[:], in0=pss[i][:], scalar=1.0 / FFN_WS,
                            in1=x_p[pr][:, si, tt, :], op0=AL.mult, op1=AL.add)
                    else:
                        nc.vector.tensor_tensor(out=r[:], in0=pss[i][:],
                                                in1=x_p[pr][:, si, tt, :],
                                                op=AL.add)
                    if not fl["bo2"]:
                        nc.vector.tensor_tensor(out=r[:], in0=r[:],
                                                in1=w["bo2"][:], op=AL.add)
                    rs.append(r[:])
                if cross:
                    # WO/FFN weights of the next layer: all layer-l readers
                    # of wo/wi/wo2 are emitted above
                    w_nxt = load_layer_weights_B(nxt_l, w_nxt, nc.gpsimd)
                if l == 1 and pr == 0:
                    fcw, cb = load_conv_weights(nc.gpsimd)
                # PE backfill while the LN2 chain runs: second score group
                if nxt_pr is not None:
                    att_scores(nxt_pr, 1, qk_nxt)
                residual_ln(rs, [x_p[pr][:, i // nt, i % nt, :]
                                 for i in range(2 * nt)],
                            None if fl["ln"] else w["ln2s"],
                            None if fl["ln"] else w["ln2b"])
                if l < NL - 1:
                    to_feat(pr)
                if nxt_pr is not None:
                    qk_cur, v_cur, w_cur = qk_nxt, v_nxt, w_nxt
                if l == NL - 1:
                    conv_pair(pr, cms, pens)

        if not fl["cb"]:
            for ki in range(3):
                for ft in range(2):
                    co = ki * 2 + ft
                    nc.vector.tensor_scalar_add(
                        out=rep[:, co, :], in0=rep[:, co, :],
                        scalar1=cb[:, ki, ft:ft + 1])
        nc.scalar.activation(out=rep[:], in_=rep[:], func=AF.Relu)

        # partial logits per branch hypothesis: fps[:, b, :] = fcw_b^T @ rep
        fps = ps_mm.tile([128, 3, NSEQ], F32, tag="mm", name="fps")
        for b_ in range(3):
            for co in range(NCH):
                nc.tensor.matmul(fps[0:NCLS, b_, :], fcw[:, b_, co, :],
                                 rep[:, co, :],
                                 start=co == 0, stop=co == NCH - 1)
        ob = small.tile([NCLS, 3, NSEQ], F32, tag="ob")
        nc.scalar.copy(out=ob[:], in_=fps[0:NCLS, :, :])
        nc.sync.dma_start(out=out_d[:], in_=ob[:])

    nc.compile()
    return nc


def _classify(inputs):
    """Compute per-core composition and the seq->(core, slot) assignment.

    Returns (ns, assign) where assign[core] is a list of NSEQ global
    sequence ids (branch*32 + sample) in slot order."""
    lens = []
    for p in ("q", "a", "b"):
        lens.append(np.asarray(inputs[p + "_attention_mask"]).sum(1))
    lens = np.concatenate(lens)          # [96], id = branch*32+sample
    short_ids = np.where(lens <= 128)[0]
    ns = min(len(short_ids) // NCORES, NSEQ)
    ns -= ns % 2
    n_short = ns * NCORES
    order = np.argsort(lens, kind="stable")
    shorts = [i for i in order if lens[i] <= 128][:n_short]
    short_set = set(shorts)
    longs = [i for i in order[::-1] if i not in short_set]
    pnt = _pair_nts(ns)
    assign = []
    for c in range(NCORES):
        my_s = shorts[c * ns:(c + 1) * ns]
        my_l = longs[c * (NSEQ - ns):(c + 1) * (NSEQ - ns)]
        si, li = 0, 0
        slots = []
        for p in range(NPAIR):
            for _ in range(2):
                if pnt[p] == 1:
                    slots.append(my_s[si]); si += 1
                else:
                    slots.append(my_l[li]); li += 1
        assign.append(slots)
    return ns, assign


def _core_inputs(inputs, fl, ns, assign):
    f32 = lambda a: np.ascontiguousarray(np.asarray(a, dtype=np.float32))
    tile_w = lambda w: np.ascontiguousarray(
        f32(w).reshape(w.shape[0] // 128, 128, w.shape[1])
        .transpose(1, 0, 2).astype(BF))
    tile_w8 = lambda w: np.ascontiguousarray(
        (f32(w) * FFN_WS).reshape(w.shape[0] // 128, 128, w.shape[1])
        .transpose(1, 0, 2).astype(F8))

    pnt = _pair_nts(ns)
    seq_nt = []
    for p in range(NPAIR):
        seq_nt += [pnt[p], pnt[p]]

    shared = {}
    shared["posty"] = np.ascontiguousarray(
        (f32(inputs["pos_emb"][:S]) + f32(inputs["type_emb"][0])).reshape(
            2, 128, D))
    for l in range(NL):
        if FP8_QKV:
            # no host /8 on Wq in fp8 (subnormal risk); folded in the drain
            shared[f"wq{l}"] = tile_w8(inputs["Wq"][l])
            shared[f"wk{l}"] = tile_w8(inputs["Wk"][l])
            shared[f"wv{l}"] = tile_w8(inputs["Wv"][l])
            shared[f"wo{l}"] = tile_w8(inputs["Wo"][l])
        else:
            shared[f"wq{l}"] = tile_w(f32(inputs["Wq"][l]) / 8.0)
            shared[f"wk{l}"] = tile_w(inputs["Wk"][l])
            shared[f"wv{l}"] = tile_w(inputs["Wv"][l])
            shared[f"wo{l}"] = tile_w(inputs["Wo"][l])
        if FP8_FFN:
            shared[f"wi{l}"] = tile_w8(inputs["Wi"][l])
            shared[f"wo2{l}"] = tile_w8(inputs["Wo2"][l])
        else:
            shared[f"wi{l}"] = tile_w(inputs["Wi"][l])
            shared[f"wo2{l}"] = tile_w(inputs["Wo2"][l])
        if not fl["bqk"]:
            shared[f"bq{l}"] = f32(inputs["bq"][l]).reshape(ND, 128) / 8.0
            shared[f"bk{l}"] = f32(inputs["bk"][l]).reshape(ND, 128)
        if not fl["bv"]:
            shared[f"bv{l}"] = f32(inputs["bv"][l])
        if not fl["bo"]:
            shared[f"bo{l}"] = f32(inputs["bo"][l])
        if not fl["bi"]:
            shared[f"bi{l}"] = f32(inputs["bi"][l]).reshape(NFT, 128)
        if not fl["bo2"]:
            shared[f"bo2{l}"] = f32(inputs["bo2"][l])
        if not fl["ln"]:
            shared[f"ln1s{l}"] = f32(inputs["ln1_s"][l])
            shared[f"ln1b{l}"] = f32(inputs["ln1_b"][l])
            shared[f"ln2s{l}"] = f32(inputs["ln2_s"][l])
            shared[f"ln2b{l}"] = f32(inputs["ln2_b"][l])
    if not fl["ln"]:
        shared["lnes"] = f32(inputs["emb_ln_s"])
        shared["lneb"] = f32(inputs["emb_ln_b"])
    for ki, k in enumerate((1, 2, 3)):
        w = f32(inputs[f"conv_w{k}"])          # [NF, k, D]
        wt = np.ascontiguousarray(w.transpose(1, 2, 0))  # [k, D, NF]
        shared[f"cw{k}"] = np.ascontiguousarray(
            wt.reshape(k, ND, 128, NF).transpose(0, 2, 1, 3).astype(BF))
    # fc blocks in reference concat order (q_rep, b_rep, a_rep); index by
    # input branch id 0=q 1=a 2=b
    fcw = f32(inputs["fc_w"]).reshape(3, NCH, 128, NCLS)
    shared["fcw"] = np.ascontiguousarray(fcw[[0, 2, 1]].transpose(2, 0, 1, 3))
    if not fl["cb"]:
        shared["convb"] = np.stack(
            [f32(inputs[f"conv_b{k}"]).reshape(2, 128) for k in (1, 2, 3)])
    shared["word_emb"] = f32(inputs["word_emb"])

    all_ids = np.concatenate([np.asarray(inputs[p + "_input_ids"])
                              for p in ("q", "a", "b")]).astype(np.int32)
    all_masks = np.concatenate([np.asarray(inputs[p + "_attention_mask"])
                                for p in ("q", "a", "b")]).astype(np.int32)
    all_lens = all_masks.sum(1)

    in_maps = []
    for c in range(NCORES):
        sids = assign[c]
        ids_tiles, mb_tiles = [], []
        cmask = np.zeros((NSEQ, S + 1), dtype=np.float32)
        pen = np.zeros((NSEQ, 3, S + 1), dtype=np.float32)
        for j, gid in enumerate(sids):
            nt = seq_nt[j]
            for tt in range(nt):
                ids_tiles.append(all_ids[gid, tt * 128:(tt + 1) * 128])
                mb_tiles.append(
                    (all_masks[gid, tt * 128:(tt + 1) * 128] - 1) * 10000.0)
            cmask[j, 0:S] = all_masks[gid]
            L = all_lens[gid]
            swj = nt * 128 + 1
            for ki, k in enumerate((1, 2, 3)):
                valid = (np.arange(S + 1) + k - 1) <= L
                valid[swj - k + 1:] = False
                pen[j, ki] = np.where(valid, 0.0, -1e30)
        m = dict(shared)
        m["ids"] = np.ascontiguousarray(
            np.stack(ids_tiles).astype(np.int32).T)
        m["maskbias"] = np.ascontiguousarray(
            np.stack(mb_tiles).astype(np.float32).T)
        m["convmask"] = np.ascontiguousarray(cmask.astype(BF))
        m["convpen"] = np.ascontiguousarray(pen)
        in_maps.append(m)
    return in_maps


def _get_program(fl, pnt):
    key = (tuple(sorted(fl.items())), pnt)
    if key not in _CACHE:
        _CACHE[key] = _build_program(fl, pnt)
    return _CACHE[key]


def run_sharded(inputs, debug=False, **run_kwargs):
    """Shard, run on 8 cores, gather. Returns (output, BassKernelResults)."""
    from concourse.bass_utils import run_bass_kernel_spmd
    fl = _flags(inputs)
    ns, assign = _classify(inputs)
    nc = _get_program(fl, _pair_nts(ns))
    in_maps = _core_inputs(inputs, fl, ns, assign)
    res = run_bass_kernel_spmd(nc, in_maps, core_ids=list(range(NCORES)),
                               **run_kwargs)
    out = np.zeros((B, NCLS), dtype=np.float32)
    for c in range(NCORES):
        part = np.asarray(res.results[c]["out"], dtype=np.float32)  # [4,3,12]
        for j, gid in enumerate(assign[c]):
            br, sample = gid // B, gid % B
            out[sample] += part[:, br, j]
    out += np.asarray(inputs["fc_b"], dtype=np.float32)[None, :]
    return out, res


def kernel(**inputs):
    out, _ = run_sharded(inputs)
    return out
